# revision 15
# baseline (speedup 1.0000x reference)
"""MegrezMoE MoE layer on 8 Trainium2 NeuronCores.

Strategy (expert-parallel, host-routed dispatch):
  - Host computes the (tiny) router: logits -> grouped top-k ids/weights,
    exactly mirroring the reference's noaux_tc selection.
  - 32 experts are assigned 4-per-core, balanced by routed-token count.
    Tokens are gathered per expert on the host (transposed: [H, rows],
    rows padded to a per-slot static capacity) so the device kernel is a
    fully static SPMD program: per expert slot, gate_up matmul ->
    silu*mul -> down matmul, streaming the expert weight bank from HBM
    exactly once per core.
  - The shared-expert MLP is tensor-parallel across the 8 cores (each
    core owns a 128-wide slice of the shared intermediate dim) and its
    partial outputs are summed on the host.
  - All device tensors use a partition-major layout ([128, ...] with
    k-tiles concatenated along the free dim) so every DMA is a plain 2D
    contiguous transfer with minimal descriptor overhead.
  - Matmuls run in bf16 with fp32 PSUM accumulation (KERNEL_DTYPE=f32r
    selects a float32r variant that keeps fp32 data in HBM).
  - Host combines: out[t] = sum_k w[t,k]*SCALE * y_col(t,k) + shared[t].

kernel() takes the full unsharded inputs, returns the full [1024, 2048]
fp32 output.
"""

import os

import ml_dtypes
import numpy as np

import concourse.bass as bass
import concourse.bass_utils as _bass_utils
import concourse.tile as tile
from concourse import bacc, mybir
from concourse.bass_utils import run_bass_kernel_spmd

# The stock compile driver pins --enable-ldw-opt=false; with plain bf16
# LDWEIGHTS (no FWL) the PE spends ~102ns per weight load, which is the
# kernel's critical path. Rewrite the flag on the walrus command line.
# walrus rejects STANDALONE InstLdweights under ldw-opt, so
# _fuse_ldweights() folds each one into its matmul (self-loading form)
# before compile.
_LDW_OPT = os.environ.get("KERNEL_LDW_OPT", "1") == "1"
if _LDW_OPT and not getattr(_bass_utils, "_ldw_opt_patched", False):
    _orig_run_command = _bass_utils.run_command

    def _run_command_ldw(cmd, **kw):
        if isinstance(cmd, list):
            cmd = ["--enable-ldw-opt=true" if c == "--enable-ldw-opt=false"
                   else c for c in cmd]
        return _orig_run_command(cmd, **kw)

    _bass_utils.run_command = _run_command_ldw
    _bass_utils._ldw_opt_patched = True

# Model dims (hardcoded per problem spec)
H = 2048
E = 32
I = 1024
TOPK = 6
NGROUP = 8
TOPKG = 4
SCALE = 2.5
T = 1024

N_CORES = 8
EPC = 4          # experts per core
KT_H = H // 128  # 16 k-tiles over hidden dim
KT_I = I // 128  # 8 k-tiles over intermediate dim
WLOAD_K = 8      # k-tiles per weight DMA
CH = 256         # shared-expert token chunk

F32 = mybir.dt.float32
F32R = mybir.dt.float32r
BF16 = mybir.dt.bfloat16

_PROGRAM_CACHE = {}
LAST_RESULTS = None  # BassKernelResults from the most recent run (for harness)


def _mode():
    return os.environ.get("KERNEL_DTYPE", "bf16")


# ---------------------------------------------------------------------------
# Host-side routing (mirrors reference._grouped_topk in fp32 numpy)
# ---------------------------------------------------------------------------

def _host_routing(x, gate_w, e_bias):
    logits = x @ gate_w                                   # [T, E] fp32
    scores = 1.0 / (1.0 + np.exp(-logits, dtype=np.float32))
    scores_choice = scores + e_bias[None, :]
    gsize = E // NGROUP
    grp = scores_choice.reshape(T, NGROUP, gsize)
    top2 = np.sort(grp, axis=-1)[:, :, -2:]
    group_scores = top2.sum(-1)                           # [T, G]
    gidx = np.argsort(-group_scores, axis=-1, kind="stable")[:, :TOPKG]
    gmask = np.zeros((T, NGROUP), bool)
    np.put_along_axis(gmask, gidx, True, axis=1)
    emask = np.repeat(gmask, gsize, axis=1)
    masked = np.where(emask, scores_choice, -np.inf)
    topk_ids = np.argsort(-masked, axis=-1, kind="stable")[:, :TOPK]
    topk_w = np.take_along_axis(scores, topk_ids, axis=1)
    topk_w = topk_w / topk_w.sum(-1, keepdims=True)
    return topk_w.astype(np.float32), topk_ids.astype(np.int64)


# ---------------------------------------------------------------------------
# Dispatch plan: expert -> (core, slot), per-slot static row capacities
# ---------------------------------------------------------------------------

def _make_plan(topk_ids):
    counts = np.bincount(topk_ids.ravel(), minlength=E)
    # slot capacity = max routed count in the slot's expert group, rounded
    # up to 8 (DMA alignment)
    padded = np.maximum(16, ((counts + 7) // 8) * 8)
    order = np.argsort(-padded, kind="stable")            # experts, big first
    expert_of = []      # per slot: experts per core
    slot_rows = []
    slot_piece = []     # (piece index, piece capacity) per slot
    for s in range(EPC):
        chunk = order[s * N_CORES:(s + 1) * N_CORES]
        cap = int(padded[chunk].max())
        if cap <= 512:
            expert_of.append(list(chunk))
            slot_rows.append(cap)
            slot_piece.append((0, cap))
        else:
            # an expert group too wide for one PSUM bank: split into
            # pieces of <=512 rows (same expert, partitioned token list)
            n_p = -(-cap // 512)
            pcap = ((-(-cap // n_p) + 7) // 8) * 8
            for j in range(n_p):
                expert_of.append(list(chunk))
                slot_rows.append(pcap)
                slot_piece.append((j, pcap))
    expert_of = np.asarray(expert_of).T                   # [N_CORES, n_slots]
    offs = np.concatenate([[0], np.cumsum(slot_rows)])
    return {
        "expert_of": expert_of,
        "slot_rows": tuple(slot_rows),
        "slot_piece": slot_piece,
        "slot_offs": offs[:-1],
        "rtotal": int(offs[-1]),
        "counts": counts,
    }


# ---------------------------------------------------------------------------
# Bass program (SPMD; one program, per-core data)
# ---------------------------------------------------------------------------

def _fuse_ldweights(nc):
    """Fold each standalone InstLdweights into the following InstMatmult
    (ldweights=True, self-loading) so walrus --enable-ldw-opt (FWL) can
    compile the program. Tile legalization always splits matmuls into
    LDW+MM pairs; walrus errors on any standalone LDW under ldw-opt.
    LDW waits move onto the matmul (or an event-semaphore right before
    it when the matmul already waits on a different semaphore)."""
    PE = mybir.EngineType.PE
    n_fused = 0
    for fn in nc.m.functions:
        for blk in fn.blocks:
            pending = None
            out = []
            changed = False
            for inst in blk.instructions:
                if isinstance(inst, mybir.InstLdweights) and inst.engine == PE:
                    assert pending is None, "LDW with no consuming matmul"
                    pending = inst
                    changed = True
                    continue
                if isinstance(inst, mybir.InstMatmult) and inst.engine == PE:
                    assert pending is not None, "matmul without its LDW"
                    ldw, pending = pending, None
                    wap, lap = inst.ins[1], ldw.ins[0]
                    assert (wap.memref, wap.offset, str(wap.ap)) == \
                           (lap.memref, lap.offset, str(lap.ap))
                    inst.ldweights = True
                    lsync = ldw.sync_info
                    lw = list(lsync.on_wait) if lsync else []
                    assert not (lsync and lsync.on_update)
                    if lw:
                        msync = inst.sync_info
                        mw = list(msync.on_wait) if msync else []
                        mu = list(msync.on_update) if msync else []
                        extra = []
                        for w in lw:
                            dup = next((x for x in mw if x.id == w.id and
                                        x.wait_mode == w.wait_mode ==
                                        "sem-ge-imm"), None)
                            if dup is not None:
                                if w.wait_value > dup.wait_value:
                                    mw[mw.index(dup)] = w
                            elif not mw:
                                mw.append(w)
                            else:
                                extra.append(w)
                        if extra:
                            ev = mybir.InstEventSemaphore(
                                name=nc.get_next_instruction_name(),
                                ins=[], outs=[])
                            ev.engine = PE
                            ev.sync_info = mybir.SyncInfo(
                                on_wait=extra, on_update=[])
                            nc.register_instruction(ev)
                            out.append(ev)
                        inst.sync_info = mybir.SyncInfo(
                            on_wait=mw, on_update=mu)
                    n_fused += 1
                out.append(inst)
            assert pending is None
            if changed:
                blk.instructions = out
    return n_fused


def _build_program(slot_rows, mode):
    rtotal = sum(slot_rows)
    f32r = mode == "f32r"
    DTD = F32 if f32r else BF16      # dram dtype of matmul operands
    DTS = F32R if f32r else BF16     # sbuf dtype of matmul operands
    DTO = F32 if f32r else BF16      # output dtype

    nc = bacc.Bacc("TRN2", target_bir_lowering=False, debug=False,
                   num_devices=N_CORES)

    # DRAM I/O, all partition-major ([128 partitions, free]):
    #   xg : slot-blocked gathered tokens; slot s at cols KT_H*off_s,
    #        k-tile k of slot s at [KT_H*off_s + k*R_s, +R_s]
    #   xf : chunk-blocked all tokens (for the shared expert)
    #   wgu/wd : per (slot, half): k-tiles concatenated along free dim
    #   yr/ys : per (slot/chunk, half): 8 output m-tiles concatenated
    n_slots = len(slot_rows)
    xg = nc.dram_tensor("xg", [128, KT_H * rtotal], DTD, kind="ExternalInput")
    xf = nc.dram_tensor("xf", [128, KT_H * T], DTD, kind="ExternalInput")
    wgu = nc.dram_tensor("wgu", [n_slots, 2, 128, KT_H * I], DTD,
                         kind="ExternalInput")
    wd = nc.dram_tensor("wd", [n_slots, 2, 128, KT_I * I], DTD,
                        kind="ExternalInput")
    wsg = nc.dram_tensor("wsg", [128, KT_H * 256], DTD, kind="ExternalInput")
    wsd = nc.dram_tensor("wsd", [128, H], DTD, kind="ExternalInput")
    nc.dram_tensor(f"cfgldw{int(_LDW_OPT)}", [1, 1], F32, kind="Internal")
    yr = nc.dram_tensor("yr", [128, 16 * rtotal], DTO, kind="ExternalOutput")
    ys = nc.dram_tensor("ys", [128, 16 * T], DTO, kind="ExternalOutput")

    # casting DMA (fp32 dram -> f32r sbuf) must go via SWDGE
    ldma = nc.gpsimd.dma_start if f32r else nc.sync.dma_start
    xdma = ldma

    slot_offs = [0]
    for R in slot_rows[:-1]:
        slot_offs.append(slot_offs[-1] + R)

    # Input loads (weights + x) all ride the Sync HWDGE queue — triggers
    # run well ahead of compute there. Output stores go on the Activation
    # HWDGE queue: their deps (the drain copies) are scalar/vector-local,
    # so a store trigger never blocks that queue's forward progress.
    sdma = ldma
    odma = nc.gpsimd.dma_start if f32r else nc.scalar.dma_start

    with tile.TileContext(nc) as tc:
        with tc.tile_pool(name="psum_e", bufs=5, space="PSUM") as pe_pool, \
             tc.tile_pool(name="psum_s", bufs=3, space="PSUM") as psh_pool, \
             tc.tile_pool(name="swg", bufs=1) as swg_pool, \
             tc.tile_pool(name="swd", bufs=1) as swd_pool, \
             tc.tile_pool(name="sxf", bufs=3) as sxf_pool, \
             tc.tile_pool(name="sact", bufs=2) as sact_pool, \
             tc.tile_pool(name="sout", bufs=2) as sout_pool, \
             tc.tile_pool(name="wsl", bufs=4) as w_pool, \
             tc.tile_pool(name="xs", bufs=3) as x_pool, \
             tc.tile_pool(name="gs", bufs=2) as g_pool, \
             tc.tile_pool(name="at", bufs=2) as a_pool, \
             tc.tile_pool(name="ost", bufs=2) as o_pool:

            wsg_sb = swg_pool.tile([128, KT_H * 256], DTS)
            wsd_sb = swd_pool.tile([128, H], DTS)

            def shared_weights():
                sdma(wsg_sb[:], wsg.ap())
                sdma(wsd_sb[:], wsd.ap())

            def shared_chunk(ch):
                xf_sb = sxf_pool.tile([128, KT_H * CH], DTS, name="xf_sb")
                sdma(xf_sb[:], xf.ap()[:, ch * KT_H * CH:(ch + 1) * KT_H * CH])
                ps_g = psh_pool.tile([128, CH], F32, tag="ps", name="ps_g")
                ps_u = psh_pool.tile([128, CH], F32, tag="ps", name="ps_u")
                for k in range(KT_H):
                    lg = wsg_sb[:, k * 256:k * 256 + 128]
                    lu = wsg_sb[:, k * 256 + 128:k * 256 + 256]
                    rx = xf_sb[:, k * CH:(k + 1) * CH]
                    nc.tensor.matmul(ps_g[:], lg, rx,
                                     start=(k == 0), stop=(k == KT_H - 1))
                    nc.tensor.matmul(ps_u[:], lu, rx,
                                     start=(k == 0), stop=(k == KT_H - 1))
                gss = sact_pool.tile([128, CH], F32, tag="sgs", name="gss")
                nc.scalar.activation(gss[:], ps_g[:],
                                     mybir.ActivationFunctionType.Sigmoid)
                nc.vector.tensor_mul(gss[:], gss[:], ps_g[:])
                a_s = sact_pool.tile([128, CH], DTS, tag="sas", name="a_s")
                nc.vector.tensor_mul(a_s[:], gss[:], ps_u[:])
                # down: 16 output m-tiles, single k (the 128-slice of I);
                # vector does the PSUM drain copies (scalar is busy with
                # the expert-slot drains and DMA triggers)
                for half in range(2):
                    stg = sout_pool.tile([128, 8 * CH], DTO, tag="sstg",
                                         name="stg")
                    for m in range(8):
                        pd = psh_pool.tile([128, CH], F32, tag="ps",
                                           name="pd")
                        lw = wsd_sb[:, (half * 8 + m) * 128:
                                    (half * 8 + m + 1) * 128]
                        nc.tensor.matmul(pd[:], lw, a_s[:],
                                         start=True, stop=True)
                        nc.vector.tensor_copy(stg[:, m * CH:(m + 1) * CH],
                                              pd[:])
                    odma(ys.ap()[:, (ch * 2 + half) * 8 * CH:
                                 (ch * 2 + half + 1) * 8 * CH],
                         stg[:])

            def expert_slot(s):
                R = slot_rows[s]
                off = slot_offs[s]
                xbase = KT_H * off
                xs = x_pool.tile([128, KT_H * R], DTS, tag="xs", name="xs")
                s0_wts = []
                if s == 0:
                    # ramp: interleave the phase-0 weight-group DMAs with
                    # the xs k-range pieces they need, smallest first, so
                    # the PE starts early and never outruns the stream
                    for (k0, k1) in [(0, 1), (1, 2), (2, 8), (8, 16)]:
                        wt = w_pool.tile([128, (k1 - k0) * I], DTS,
                                         tag="wsl", name="wt")
                        ldma(wt[:], wgu.ap()[0, 0][:, k0 * I:k1 * I])
                        s0_wts.append(wt)
                        sdma(xs[:, k0 * R:k1 * R],
                             xg.ap()[:, xbase + k0 * R:xbase + k1 * R])
                else:
                    sdma(xs[:], xg.ap()[:, xbase:xbase + KT_H * R])

                gs = g_pool.tile([128, KT_I * R], F32, tag="gs", name="gs")
                at = a_pool.tile([128, KT_I * R], DTS, tag="at", name="at")

                for phase in range(2):  # 0 = gate, 1 = up
                    def gu_drain(m, ps_m):
                        if phase == 0:
                            nc.scalar.activation(
                                gs[:, m * R:(m + 1) * R], ps_m[:],
                                mybir.ActivationFunctionType.Sigmoid)
                            nc.vector.tensor_mul(
                                gs[:, m * R:(m + 1) * R],
                                gs[:, m * R:(m + 1) * R], ps_m[:])
                        else:
                            nc.vector.tensor_mul(
                                at[:, m * R:(m + 1) * R],
                                gs[:, m * R:(m + 1) * R], ps_m[:])

                    if s == 0 and phase == 0:
                        groups = [(0, 1), (1, 2), (2, 8), (8, 16)]
                        wts = s0_wts
                    else:
                        groups = [(kb * WLOAD_K, (kb + 1) * WLOAD_K)
                                  for kb in range(KT_H // WLOAD_K)]
                        wts = []
                        for (k0, k1) in groups:
                            wt = w_pool.tile([128, (k1 - k0) * I], DTS,
                                             tag="wsl", name="wt")
                            ldma(wt[:], wgu.ap()[s, phase][:, k0 * I:k1 * I])
                            wts.append(wt)
                    # two 4-m-tile waves: expert slots hold at most 4+2
                    # PSUM banks so shared chunks can interleave
                    for wave in range(2):
                        ps = [pe_pool.tile([128, R], F32, tag="ps",
                                           name="ps") for _ in range(4)]
                        for gi, (k0, k1) in enumerate(groups):
                            wt = wts[gi]
                            for kk in range(k1 - k0):
                                k = k0 + kk
                                rx = xs[:, k * R:(k + 1) * R]
                                for mi in range(4):
                                    m = wave * 4 + mi
                                    lw = wt[:, kk * I + m * 128:
                                            kk * I + (m + 1) * 128]
                                    nc.tensor.matmul(
                                        ps[mi][:], lw, rx,
                                        start=(k == 0), stop=(k == KT_H - 1))
                        for mi in range(4):
                            gu_drain(wave * 4 + mi, ps[mi])

                WLD = min(WLOAD_K, KT_I)
                for half in range(2):
                    wts = []
                    for kb in range(KT_I // WLD):
                        wt = w_pool.tile([128, WLD * I], DTS, tag="wsl",
                                         name="wt")
                        ldma(wt[:], wd.ap()[s, half][:, kb * WLD * I:
                                                     (kb + 1) * WLD * I])
                        wts.append(wt)
                    stg = o_pool.tile([128, 8 * R], DTO, tag="ost", name="stg")
                    for wave in range(2):
                        ps = [pe_pool.tile([128, R], F32, tag="ps",
                                           name="ps") for _ in range(4)]
                        for kb in range(KT_I // WLD):
                            wt = wts[kb]
                            for kk in range(WLD):
                                k = kb * WLD + kk
                                ra = at[:, k * R:(k + 1) * R]
                                for mi in range(4):
                                    m = wave * 4 + mi
                                    lw = wt[:, kk * I + m * 128:
                                            kk * I + (m + 1) * 128]
                                    nc.tensor.matmul(
                                        ps[mi][:], lw, ra,
                                        start=(k == 0), stop=(k == KT_I - 1))
                        for mi in range(4):
                            m = wave * 4 + mi
                            nc.scalar.copy(stg[:, m * R:(m + 1) * R],
                                           ps[mi][:])
                    boff = 16 * off + half * 8 * R
                    odma(yr.ap()[:, boff:boff + 8 * R], stg[:])

            # experts carry the bulk of the DMA stream; shared-expert
            # chunks are interleaved to fill PE gaps at phase boundaries
            expert_slot(0)
            shared_weights()
            shared_chunk(0)
            shared_chunk(1)
            for s_i in range(1, n_slots):
                if s_i == 1:
                    expert_slot(1)
                    shared_chunk(2)
                elif s_i == 2:
                    expert_slot(2)
                    shared_chunk(3)
                else:
                    expert_slot(s_i)

    if _LDW_OPT:
        _fuse_ldweights(nc)
    nc.compile()
    return nc


def _get_program(slot_rows, mode):
    key = (tuple(slot_rows), mode)
    if key not in _PROGRAM_CACHE:
        _PROGRAM_CACHE[key] = _build_program(slot_rows, mode)
    return _PROGRAM_CACHE[key]


# ---------------------------------------------------------------------------
# Per-core input construction (host shard + reorder + cast)
# ---------------------------------------------------------------------------

def _pm(a):
    """[KT, 128, M] -> partition-major [128, KT*M]."""
    kt, p, m = a.shape
    return np.ascontiguousarray(a.transpose(1, 0, 2)).reshape(p, kt * m)


def _make_in_maps(x, w_gate_up, w_down, shared_gate_up, shared_down,
                  topk_ids, plan, mode):
    rtotal = plan["rtotal"]
    slot_rows = plan["slot_rows"]
    offs = plan["slot_offs"]
    expert_of = plan["expert_of"]
    np_dt = np.float32 if mode == "f32r" else ml_dtypes.bfloat16

    slot_piece = plan.get("slot_piece") or [(0, r) for r in slot_rows]
    n_slots = len(slot_rows)
    tok_of = [np.where((topk_ids == e).any(axis=1))[0] for e in range(E)]
    flat_col = np.zeros((T, TOPK), dtype=np.int64)

    xT = np.ascontiguousarray(x.T).astype(np_dt)          # [H, T]
    # weights -> [E, 2, 128, KT*I] partition-major k-slab layout
    wgu_pm = np.ascontiguousarray(
        w_gate_up.astype(np_dt).reshape(E, KT_H, 128, 2, I)
        .transpose(0, 3, 2, 1, 4)).reshape(E, 2, 128, KT_H * I)
    wd_pm = np.ascontiguousarray(
        w_down.astype(np_dt).reshape(E, KT_I, 128, 2, I)
        .transpose(0, 3, 2, 1, 4)).reshape(E, 2, 128, KT_I * I)

    xf_pm = np.concatenate(
        [_pm(xT[:, ch * CH:(ch + 1) * CH].reshape(KT_H, 128, CH))
         for ch in range(T // CH)], axis=1)

    in_maps = []
    for c in range(N_CORES):
        xg_pm = np.zeros((128, KT_H * rtotal), dtype=np_dt)
        for s in range(n_slots):
            e = expert_of[c, s]
            pj, pcap = slot_piece[s]
            toks = tok_of[e][pj * pcap:(pj + 1) * pcap]
            R = slot_rows[s]
            blk = np.zeros((H, R), dtype=np_dt)
            blk[:, :len(toks)] = xT[:, toks]
            xg_pm[:, KT_H * offs[s]:KT_H * (offs[s] + R)] = _pm(
                blk.reshape(KT_H, 128, R))
            col_base = c * rtotal + offs[s]
            for pos, t in enumerate(toks):
                for k in np.nonzero(topk_ids[t] == e)[0]:
                    flat_col[t, k] = col_base + pos
        sl = slice(c * 128, (c + 1) * 128)
        sgu_slice = np.concatenate(
            [shared_gate_up[:, sl],
             shared_gate_up[:, 1024 + c * 128:1024 + (c + 1) * 128]],
            axis=1).astype(np_dt)
        in_maps.append({
            "xg": xg_pm,
            "xf": xf_pm,
            "wgu": np.ascontiguousarray(wgu_pm[expert_of[c]]),
            "wd": np.ascontiguousarray(wd_pm[expert_of[c]]),
            "wsg": _pm(sgu_slice.reshape(KT_H, 128, 256)),
            "wsd": np.ascontiguousarray(shared_down[sl, :]).astype(np_dt),
        })
    return in_maps, flat_col


# ---------------------------------------------------------------------------
# Entry point
# ---------------------------------------------------------------------------

def kernel(hidden_states, gate_w, e_bias, w_gate_up, w_down,
           shared_gate_up, shared_down):
    global LAST_RESULTS
    mode = _mode()
    x = np.ascontiguousarray(np.asarray(hidden_states, dtype=np.float32))
    gate_w = np.asarray(gate_w, dtype=np.float32)
    e_bias = np.asarray(e_bias, dtype=np.float32)
    w_gate_up = np.asarray(w_gate_up, dtype=np.float32)
    w_down = np.asarray(w_down, dtype=np.float32)
    shared_gate_up = np.asarray(shared_gate_up, dtype=np.float32)
    shared_down = np.asarray(shared_down, dtype=np.float32)

    topk_w, topk_ids = _host_routing(x, gate_w, e_bias)
    plan = _make_plan(topk_ids)
    slot_rows = plan["slot_rows"]
    offs = plan["slot_offs"]
    rtotal = plan["rtotal"]

    nc = _get_program(slot_rows, mode)
    in_maps, flat_col = _make_in_maps(
        x, w_gate_up, w_down, shared_gate_up, shared_down,
        topk_ids, plan, mode)

    trace = bool(int(os.environ.get("KERNEL_TRACE", "0")))
    res = run_bass_kernel_spmd(
        nc, in_maps, list(range(N_CORES)), trace=trace,
        tmpdir=os.environ.get("KERNEL_TRACE_DIR") or None)
    LAST_RESULTS = res

    # decode partition-major outputs back to [H, rtotal] per core
    def decode_yr(arr):
        out = np.empty((H, rtotal), dtype=np.float32)
        for s in range(len(slot_rows)):
            R = slot_rows[s]
            for half in range(2):
                boff = 16 * offs[s] + half * 8 * R
                blk = np.asarray(arr[:, boff:boff + 8 * R], dtype=np.float32)
                out[half * 1024:(half + 1) * 1024, offs[s]:offs[s] + R] = (
                    blk.reshape(128, 8, R).transpose(1, 0, 2).reshape(1024, R))
        return out

    Y = np.concatenate(
        [decode_yr(res.results[c]["yr"]).T for c in range(N_CORES)], axis=0)
    w_flat = (topk_w * SCALE).astype(np.float32).reshape(-1)
    out = (Y[flat_col.reshape(-1)] * w_flat[:, None]).reshape(T, TOPK, H).sum(1)

    ys_sum = np.zeros((128, 16 * T), dtype=np.float32)
    for c in range(N_CORES):
        ys_sum += np.asarray(res.results[c]["ys"], dtype=np.float32)
    shared = np.empty((H, T), dtype=np.float32)
    for ch in range(T // CH):
        for half in range(2):
            boff = (ch * 2 + half) * 8 * CH
            blk = ys_sum[:, boff:boff + 8 * CH]
            shared[half * 1024:(half + 1) * 1024, ch * CH:(ch + 1) * CH] = (
                blk.reshape(128, 8, CH).transpose(1, 0, 2).reshape(1024, CH))
    out += shared.T
    return out.astype(np.float32)



# revision 17
# speedup vs baseline: 1.0764x; 1.0764x over previous
"""MegrezMoE MoE layer on 8 Trainium2 NeuronCores.

Strategy (expert-parallel, host-routed dispatch):
  - Host computes the (tiny) router: logits -> grouped top-k ids/weights,
    exactly mirroring the reference's noaux_tc selection.
  - 32 experts are assigned 4-per-core, balanced by routed-token count.
    Tokens are gathered per expert on the host (transposed: [H, rows],
    rows padded to a per-slot static capacity) so the device kernel is a
    fully static SPMD program: per expert slot, gate_up matmul ->
    silu*mul -> down matmul, streaming the expert weight bank from HBM
    exactly once per core.
  - The shared-expert MLP is tensor-parallel across the 8 cores (each
    core owns a 128-wide slice of the shared intermediate dim) and its
    partial outputs are summed on the host.
  - All device tensors use a partition-major layout ([128, ...] with
    k-tiles concatenated along the free dim) so every DMA is a plain 2D
    contiguous transfer with minimal descriptor overhead.
  - Matmuls run in bf16 with fp32 PSUM accumulation (KERNEL_DTYPE=f32r
    selects a float32r variant that keeps fp32 data in HBM).
  - Host combines: out[t] = sum_k w[t,k]*SCALE * y_col(t,k) + shared[t].

kernel() takes the full unsharded inputs, returns the full [1024, 2048]
fp32 output.
"""

import os

import ml_dtypes
import numpy as np

import concourse.bass as bass
import concourse.bass_utils as _bass_utils
import concourse.tile as tile
from concourse import bacc, mybir
from concourse.bass_utils import run_bass_kernel_spmd

# The stock compile driver pins --enable-ldw-opt=false; with plain bf16
# LDWEIGHTS (no FWL) the PE spends ~102ns per weight load, which is the
# kernel's critical path. Rewrite the flag on the walrus command line.
# walrus rejects STANDALONE InstLdweights under ldw-opt, so
# _fuse_ldweights() folds each one into its matmul (self-loading form)
# before compile.
_LDW_OPT = os.environ.get("KERNEL_LDW_OPT", "1") == "1"
if _LDW_OPT and not getattr(_bass_utils, "_ldw_opt_patched", False):
    _orig_run_command = _bass_utils.run_command

    def _run_command_ldw(cmd, **kw):
        if isinstance(cmd, list):
            cmd = ["--enable-ldw-opt=true" if c == "--enable-ldw-opt=false"
                   else c for c in cmd]
        return _orig_run_command(cmd, **kw)

    _bass_utils.run_command = _run_command_ldw
    _bass_utils._ldw_opt_patched = True

# Model dims (hardcoded per problem spec)
H = 2048
E = 32
I = 1024
TOPK = 6
NGROUP = 8
TOPKG = 4
SCALE = 2.5
T = 1024

N_CORES = 8
EPC = 4          # experts per core
KT_H = H // 128  # 16 k-tiles over hidden dim
KT_I = I // 128  # 8 k-tiles over intermediate dim
WLOAD_K = 8      # k-tiles per weight DMA
CH = 256         # shared-expert token chunk

F32 = mybir.dt.float32
F32R = mybir.dt.float32r
BF16 = mybir.dt.bfloat16

_PROGRAM_CACHE = {}
LAST_RESULTS = None  # BassKernelResults from the most recent run (for harness)


def _mode():
    return os.environ.get("KERNEL_DTYPE", "bf16")


# ---------------------------------------------------------------------------
# Host-side routing (mirrors reference._grouped_topk in fp32 numpy)
# ---------------------------------------------------------------------------

def _host_routing(x, gate_w, e_bias):
    logits = x @ gate_w                                   # [T, E] fp32
    scores = 1.0 / (1.0 + np.exp(-logits, dtype=np.float32))
    scores_choice = scores + e_bias[None, :]
    gsize = E // NGROUP
    grp = scores_choice.reshape(T, NGROUP, gsize)
    top2 = np.sort(grp, axis=-1)[:, :, -2:]
    group_scores = top2.sum(-1)                           # [T, G]
    gidx = np.argsort(-group_scores, axis=-1, kind="stable")[:, :TOPKG]
    gmask = np.zeros((T, NGROUP), bool)
    np.put_along_axis(gmask, gidx, True, axis=1)
    emask = np.repeat(gmask, gsize, axis=1)
    masked = np.where(emask, scores_choice, -np.inf)
    topk_ids = np.argsort(-masked, axis=-1, kind="stable")[:, :TOPK]
    topk_w = np.take_along_axis(scores, topk_ids, axis=1)
    topk_w = topk_w / topk_w.sum(-1, keepdims=True)
    return topk_w.astype(np.float32), topk_ids.astype(np.int64)


# ---------------------------------------------------------------------------
# Dispatch plan: expert -> (core, slot), per-slot static row capacities
# ---------------------------------------------------------------------------

def _make_plan(topk_ids):
    counts = np.bincount(topk_ids.ravel(), minlength=E)
    # slot capacity = max routed count in the slot's expert group, rounded
    # up to 8 (DMA alignment)
    padded = np.maximum(16, ((counts + 7) // 8) * 8)
    order = np.argsort(-padded, kind="stable")            # experts, big first
    expert_of = []      # per slot: experts per core
    slot_rows = []
    slot_piece = []     # (piece index, piece capacity) per slot
    for s in range(EPC):
        chunk = order[s * N_CORES:(s + 1) * N_CORES]
        cap = int(padded[chunk].max())
        if cap <= 512:
            expert_of.append(list(chunk))
            slot_rows.append(cap)
            slot_piece.append((0, cap))
        else:
            # an expert group too wide for one PSUM bank: split into
            # pieces of <=512 rows (same expert, partitioned token list)
            n_p = -(-cap // 512)
            pcap = ((-(-cap // n_p) + 7) // 8) * 8
            for j in range(n_p):
                expert_of.append(list(chunk))
                slot_rows.append(pcap)
                slot_piece.append((j, pcap))
    expert_of = np.asarray(expert_of).T                   # [N_CORES, n_slots]
    offs = np.concatenate([[0], np.cumsum(slot_rows)])
    return {
        "expert_of": expert_of,
        "slot_rows": tuple(slot_rows),
        "slot_piece": slot_piece,
        "slot_offs": offs[:-1],
        "rtotal": int(offs[-1]),
        "counts": counts,
    }


# ---------------------------------------------------------------------------
# Bass program (SPMD; one program, per-core data)
# ---------------------------------------------------------------------------

def _fuse_ldweights(nc):
    """Fold each standalone InstLdweights into the following InstMatmult
    (ldweights=True, self-loading) so walrus --enable-ldw-opt (FWL) can
    compile the program. Tile legalization always splits matmuls into
    LDW+MM pairs; walrus errors on any standalone LDW under ldw-opt.
    LDW waits move onto the matmul (or an event-semaphore right before
    it when the matmul already waits on a different semaphore)."""
    PE = mybir.EngineType.PE
    n_fused = 0
    for fn in nc.m.functions:
        for blk in fn.blocks:
            pending = None
            out = []
            changed = False
            for inst in blk.instructions:
                if isinstance(inst, mybir.InstLdweights) and inst.engine == PE:
                    assert pending is None, "LDW with no consuming matmul"
                    pending = inst
                    changed = True
                    continue
                if isinstance(inst, mybir.InstMatmult) and inst.engine == PE:
                    assert pending is not None, "matmul without its LDW"
                    ldw, pending = pending, None
                    wap, lap = inst.ins[1], ldw.ins[0]
                    assert (wap.memref, wap.offset, str(wap.ap)) == \
                           (lap.memref, lap.offset, str(lap.ap))
                    inst.ldweights = True
                    lsync = ldw.sync_info
                    lw = list(lsync.on_wait) if lsync else []
                    assert not (lsync and lsync.on_update)
                    if lw:
                        msync = inst.sync_info
                        mw = list(msync.on_wait) if msync else []
                        mu = list(msync.on_update) if msync else []
                        extra = []
                        for w in lw:
                            dup = next((x for x in mw if x.id == w.id and
                                        x.wait_mode == w.wait_mode ==
                                        "sem-ge-imm"), None)
                            if dup is not None:
                                if w.wait_value > dup.wait_value:
                                    mw[mw.index(dup)] = w
                            elif not mw:
                                mw.append(w)
                            else:
                                extra.append(w)
                        if extra:
                            ev = mybir.InstEventSemaphore(
                                name=nc.get_next_instruction_name(),
                                ins=[], outs=[])
                            ev.engine = PE
                            ev.sync_info = mybir.SyncInfo(
                                on_wait=extra, on_update=[])
                            nc.register_instruction(ev)
                            out.append(ev)
                        inst.sync_info = mybir.SyncInfo(
                            on_wait=mw, on_update=mu)
                    n_fused += 1
                out.append(inst)
            assert pending is None
            if changed:
                blk.instructions = out
    return n_fused


def _build_program(slot_rows, mode):
    rtotal = sum(slot_rows)
    f32r = mode == "f32r"
    DTD = F32 if f32r else BF16      # dram dtype of matmul operands
    DTS = F32R if f32r else BF16     # sbuf dtype of matmul operands
    DTO = F32 if f32r else BF16      # output dtype

    nc = bacc.Bacc("TRN2", target_bir_lowering=False, debug=False,
                   num_devices=N_CORES)

    # DRAM I/O, all partition-major ([128 partitions, free]):
    #   xg : slot-blocked gathered tokens; slot s at cols KT_H*off_s,
    #        k-tile k of slot s at [KT_H*off_s + k*R_s, +R_s]
    #   xf : chunk-blocked all tokens (for the shared expert)
    #   wgu/wd : per (slot, half): k-tiles concatenated along free dim
    #   yr/ys : per (slot/chunk, half): 8 output m-tiles concatenated
    n_slots = len(slot_rows)
    xg = nc.dram_tensor("xg", [128, KT_H * rtotal], DTD, kind="ExternalInput")
    xf = nc.dram_tensor("xf", [128, KT_H * T], DTD, kind="ExternalInput")
    wgu = nc.dram_tensor("wgu", [n_slots, 2, 128, KT_H * I], DTD,
                         kind="ExternalInput")
    wd = nc.dram_tensor("wd", [n_slots, 2, 128, KT_I * I], DTD,
                        kind="ExternalInput")
    wsg = nc.dram_tensor("wsg", [128, KT_H * 256], DTD, kind="ExternalInput")
    wsd = nc.dram_tensor("wsd", [128, H], DTD, kind="ExternalInput")
    nc.dram_tensor(f"cfgldw{int(_LDW_OPT)}", [1, 1], F32, kind="Internal")
    yr = nc.dram_tensor("yr", [128, 16 * rtotal], DTO, kind="ExternalOutput")
    ys = nc.dram_tensor("ys", [128, 16 * T], DTO, kind="ExternalOutput")

    # casting DMA (fp32 dram -> f32r sbuf) must go via SWDGE
    ldma = nc.gpsimd.dma_start if f32r else nc.sync.dma_start
    xdma = ldma

    slot_offs = [0]
    for R in slot_rows[:-1]:
        slot_offs.append(slot_offs[-1] + R)

    # Input loads (weights + x) all ride the Sync HWDGE queue — triggers
    # run well ahead of compute there. Output stores go on the Activation
    # HWDGE queue: their deps (the drain copies) are scalar/vector-local,
    # so a store trigger never blocks that queue's forward progress.
    sdma = ldma
    odma = nc.gpsimd.dma_start if f32r else nc.scalar.dma_start

    with tile.TileContext(nc) as tc:
        with tc.tile_pool(name="psum_e", bufs=5, space="PSUM") as pe_pool, \
             tc.tile_pool(name="psum_s", bufs=3, space="PSUM") as psh_pool, \
             tc.tile_pool(name="swg", bufs=1) as swg_pool, \
             tc.tile_pool(name="swd", bufs=1) as swd_pool, \
             tc.tile_pool(name="sxf", bufs=3) as sxf_pool, \
             tc.tile_pool(name="sact", bufs=2) as sact_pool, \
             tc.tile_pool(name="sout", bufs=2) as sout_pool, \
             tc.tile_pool(name="wsl", bufs=4) as w_pool, \
             tc.tile_pool(name="xs", bufs=3) as x_pool, \
             tc.tile_pool(name="gs", bufs=2) as g_pool, \
             tc.tile_pool(name="at", bufs=2) as a_pool, \
             tc.tile_pool(name="ost", bufs=2) as o_pool:

            wsg_sb = swg_pool.tile([128, KT_H * 256], DTS)
            wsd_sb = swd_pool.tile([128, H], DTS)

            def shared_weights():
                sdma(wsg_sb[:], wsg.ap())
                sdma(wsd_sb[:], wsd.ap())

            def shared_chunk(ch):
                xf_sb = sxf_pool.tile([128, KT_H * CH], DTS, name="xf_sb")
                sdma(xf_sb[:], xf.ap()[:, ch * KT_H * CH:(ch + 1) * KT_H * CH])
                ps_g = psh_pool.tile([128, CH], F32, tag="ps", name="ps_g")
                ps_u = psh_pool.tile([128, CH], F32, tag="ps", name="ps_u")
                for k in range(KT_H):
                    lg = wsg_sb[:, k * 256:k * 256 + 128]
                    lu = wsg_sb[:, k * 256 + 128:k * 256 + 256]
                    rx = xf_sb[:, k * CH:(k + 1) * CH]
                    nc.tensor.matmul(ps_g[:], lg, rx,
                                     start=(k == 0), stop=(k == KT_H - 1))
                    nc.tensor.matmul(ps_u[:], lu, rx,
                                     start=(k == 0), stop=(k == KT_H - 1))
                gss = sact_pool.tile([128, CH], F32, tag="sgs", name="gss")
                nc.scalar.activation(gss[:], ps_g[:],
                                     mybir.ActivationFunctionType.Sigmoid)
                nc.vector.tensor_mul(gss[:], gss[:], ps_g[:])
                a_s = sact_pool.tile([128, CH], DTS, tag="sas", name="a_s")
                nc.vector.tensor_mul(a_s[:], gss[:], ps_u[:])
                # down: 16 output m-tiles, single k (the 128-slice of I);
                # vector does the PSUM drain copies (scalar is busy with
                # the expert-slot drains and DMA triggers)
                for half in range(2):
                    stg = sout_pool.tile([128, 8 * CH], DTO, tag="sstg",
                                         name="stg")
                    for m in range(8):
                        pd = psh_pool.tile([128, CH], F32, tag="ps",
                                           name="pd")
                        lw = wsd_sb[:, (half * 8 + m) * 128:
                                    (half * 8 + m + 1) * 128]
                        nc.tensor.matmul(pd[:], lw, a_s[:],
                                         start=True, stop=True)
                        nc.vector.tensor_copy(stg[:, m * CH:(m + 1) * CH],
                                              pd[:])
                    odma(ys.ap()[:, (ch * 2 + half) * 8 * CH:
                                 (ch * 2 + half + 1) * 8 * CH],
                         stg[:])

            def expert_slot(s):
                R = slot_rows[s]
                off = slot_offs[s]
                xbase = KT_H * off
                if s == 0 and not f32r:
                    # first weight group is the very first DMA on the sync
                    # queue so the PE can start as early as possible
                    wt0 = w_pool.tile([128, I], DTS, tag="wsl", name="wt")
                    ldma(wt0[:], wgu.ap()[0, 0][:, 0:I])
                else:
                    wt0 = None
                xs = x_pool.tile([128, KT_H * R], DTS, tag="xs", name="xs")
                if s == 0:
                    # split the first x load so the PE can start early
                    sdma(xs[:, :R], xg.ap()[:, xbase:xbase + R])
                    sdma(xs[:, R:2 * R], xg.ap()[:, xbase + R:xbase + 2 * R])
                    sdma(xs[:, 2 * R:], xg.ap()[:, xbase + 2 * R:
                                                xbase + KT_H * R])
                else:
                    sdma(xs[:], xg.ap()[:, xbase:xbase + KT_H * R])

                gs = g_pool.tile([128, KT_I * R], F32, tag="gs", name="gs")
                at = a_pool.tile([128, KT_I * R], DTS, tag="at", name="at")

                for phase in range(2):  # 0 = gate, 1 = up
                    def gu_drain(m, ps_m):
                        if phase == 0:
                            nc.scalar.activation(
                                gs[:, m * R:(m + 1) * R], ps_m[:],
                                mybir.ActivationFunctionType.Sigmoid)
                            nc.vector.tensor_mul(
                                gs[:, m * R:(m + 1) * R],
                                gs[:, m * R:(m + 1) * R], ps_m[:])
                        else:
                            nc.vector.tensor_mul(
                                at[:, m * R:(m + 1) * R],
                                gs[:, m * R:(m + 1) * R], ps_m[:])

                    if s == 0 and phase == 0:
                        groups = [(0, 1), (1, 2), (2, 8), (8, 16)]
                    else:
                        groups = [(kb * WLOAD_K, (kb + 1) * WLOAD_K)
                                  for kb in range(KT_H // WLOAD_K)]
                    wts = []
                    for (k0, k1) in groups:
                        if s == 0 and phase == 0 and k0 == 0 and wt0:
                            wts.append(wt0)
                            continue
                        wt = w_pool.tile([128, (k1 - k0) * I], DTS,
                                         tag="wsl", name="wt")
                        ldma(wt[:], wgu.ap()[s, phase][:, k0 * I:k1 * I])
                        wts.append(wt)
                    # two 4-m-tile waves: expert slots hold at most 4+2
                    # PSUM banks so shared chunks can interleave
                    for wave in range(2):
                        ps = [pe_pool.tile([128, R], F32, tag="ps",
                                           name="ps") for _ in range(4)]
                        for gi, (k0, k1) in enumerate(groups):
                            wt = wts[gi]
                            for kk in range(k1 - k0):
                                k = k0 + kk
                                rx = xs[:, k * R:(k + 1) * R]
                                for mi in range(4):
                                    m = wave * 4 + mi
                                    lw = wt[:, kk * I + m * 128:
                                            kk * I + (m + 1) * 128]
                                    nc.tensor.matmul(
                                        ps[mi][:], lw, rx,
                                        start=(k == 0), stop=(k == KT_H - 1))
                        for mi in range(4):
                            gu_drain(wave * 4 + mi, ps[mi])

                WLD = min(WLOAD_K, KT_I)
                for half in range(2):
                    wts = []
                    for kb in range(KT_I // WLD):
                        wt = w_pool.tile([128, WLD * I], DTS, tag="wsl",
                                         name="wt")
                        ldma(wt[:], wd.ap()[s, half][:, kb * WLD * I:
                                                     (kb + 1) * WLD * I])
                        wts.append(wt)
                    stg = o_pool.tile([128, 8 * R], DTO, tag="ost", name="stg")
                    for wave in range(2):
                        ps = [pe_pool.tile([128, R], F32, tag="ps",
                                           name="ps") for _ in range(4)]
                        for kb in range(KT_I // WLD):
                            wt = wts[kb]
                            for kk in range(WLD):
                                k = kb * WLD + kk
                                ra = at[:, k * R:(k + 1) * R]
                                for mi in range(4):
                                    m = wave * 4 + mi
                                    lw = wt[:, kk * I + m * 128:
                                            kk * I + (m + 1) * 128]
                                    nc.tensor.matmul(
                                        ps[mi][:], lw, ra,
                                        start=(k == 0), stop=(k == KT_I - 1))
                        for mi in range(4):
                            m = wave * 4 + mi
                            nc.scalar.copy(stg[:, m * R:(m + 1) * R],
                                           ps[mi][:])
                    boff = 16 * off + half * 8 * R
                    odma(yr.ap()[:, boff:boff + 8 * R], stg[:])

            # experts carry the bulk of the DMA stream; shared-expert
            # chunks are interleaved to fill PE gaps at phase boundaries
            expert_slot(0)
            shared_weights()
            shared_chunk(0)
            shared_chunk(1)
            for s_i in range(1, n_slots):
                if s_i == 1:
                    expert_slot(1)
                    shared_chunk(2)
                elif s_i == 2:
                    expert_slot(2)
                    shared_chunk(3)
                else:
                    expert_slot(s_i)

    if _LDW_OPT:
        _fuse_ldweights(nc)
    nc.compile()
    return nc


def _get_program(slot_rows, mode):
    key = (tuple(slot_rows), mode)
    if key not in _PROGRAM_CACHE:
        _PROGRAM_CACHE[key] = _build_program(slot_rows, mode)
    return _PROGRAM_CACHE[key]


# ---------------------------------------------------------------------------
# Per-core input construction (host shard + reorder + cast)
# ---------------------------------------------------------------------------

def _pm(a):
    """[KT, 128, M] -> partition-major [128, KT*M]."""
    kt, p, m = a.shape
    return np.ascontiguousarray(a.transpose(1, 0, 2)).reshape(p, kt * m)


def _make_in_maps(x, w_gate_up, w_down, shared_gate_up, shared_down,
                  topk_ids, plan, mode):
    rtotal = plan["rtotal"]
    slot_rows = plan["slot_rows"]
    offs = plan["slot_offs"]
    expert_of = plan["expert_of"]
    np_dt = np.float32 if mode == "f32r" else ml_dtypes.bfloat16

    slot_piece = plan.get("slot_piece") or [(0, r) for r in slot_rows]
    n_slots = len(slot_rows)
    tok_of = [np.where((topk_ids == e).any(axis=1))[0] for e in range(E)]
    flat_col = np.zeros((T, TOPK), dtype=np.int64)

    xT = np.ascontiguousarray(x.T).astype(np_dt)          # [H, T]
    # weights -> [E, 2, 128, KT*I] partition-major k-slab layout
    wgu_pm = np.ascontiguousarray(
        w_gate_up.astype(np_dt).reshape(E, KT_H, 128, 2, I)
        .transpose(0, 3, 2, 1, 4)).reshape(E, 2, 128, KT_H * I)
    wd_pm = np.ascontiguousarray(
        w_down.astype(np_dt).reshape(E, KT_I, 128, 2, I)
        .transpose(0, 3, 2, 1, 4)).reshape(E, 2, 128, KT_I * I)

    xf_pm = np.concatenate(
        [_pm(xT[:, ch * CH:(ch + 1) * CH].reshape(KT_H, 128, CH))
         for ch in range(T // CH)], axis=1)

    in_maps = []
    for c in range(N_CORES):
        xg_pm = np.zeros((128, KT_H * rtotal), dtype=np_dt)
        for s in range(n_slots):
            e = expert_of[c, s]
            pj, pcap = slot_piece[s]
            toks = tok_of[e][pj * pcap:(pj + 1) * pcap]
            R = slot_rows[s]
            blk = np.zeros((H, R), dtype=np_dt)
            blk[:, :len(toks)] = xT[:, toks]
            xg_pm[:, KT_H * offs[s]:KT_H * (offs[s] + R)] = _pm(
                blk.reshape(KT_H, 128, R))
            col_base = c * rtotal + offs[s]
            for pos, t in enumerate(toks):
                for k in np.nonzero(topk_ids[t] == e)[0]:
                    flat_col[t, k] = col_base + pos
        sl = slice(c * 128, (c + 1) * 128)
        sgu_slice = np.concatenate(
            [shared_gate_up[:, sl],
             shared_gate_up[:, 1024 + c * 128:1024 + (c + 1) * 128]],
            axis=1).astype(np_dt)
        in_maps.append({
            "xg": xg_pm,
            "xf": xf_pm,
            "wgu": np.ascontiguousarray(wgu_pm[expert_of[c]]),
            "wd": np.ascontiguousarray(wd_pm[expert_of[c]]),
            "wsg": _pm(sgu_slice.reshape(KT_H, 128, 256)),
            "wsd": np.ascontiguousarray(shared_down[sl, :]).astype(np_dt),
        })
    return in_maps, flat_col


# ---------------------------------------------------------------------------
# Entry point
# ---------------------------------------------------------------------------

def kernel(hidden_states, gate_w, e_bias, w_gate_up, w_down,
           shared_gate_up, shared_down):
    global LAST_RESULTS
    mode = _mode()
    x = np.ascontiguousarray(np.asarray(hidden_states, dtype=np.float32))
    gate_w = np.asarray(gate_w, dtype=np.float32)
    e_bias = np.asarray(e_bias, dtype=np.float32)
    w_gate_up = np.asarray(w_gate_up, dtype=np.float32)
    w_down = np.asarray(w_down, dtype=np.float32)
    shared_gate_up = np.asarray(shared_gate_up, dtype=np.float32)
    shared_down = np.asarray(shared_down, dtype=np.float32)

    topk_w, topk_ids = _host_routing(x, gate_w, e_bias)
    plan = _make_plan(topk_ids)
    slot_rows = plan["slot_rows"]
    offs = plan["slot_offs"]
    rtotal = plan["rtotal"]

    nc = _get_program(slot_rows, mode)
    in_maps, flat_col = _make_in_maps(
        x, w_gate_up, w_down, shared_gate_up, shared_down,
        topk_ids, plan, mode)

    trace = bool(int(os.environ.get("KERNEL_TRACE", "0")))
    res = run_bass_kernel_spmd(
        nc, in_maps, list(range(N_CORES)), trace=trace,
        tmpdir=os.environ.get("KERNEL_TRACE_DIR") or None)
    LAST_RESULTS = res

    # decode partition-major outputs back to [H, rtotal] per core
    def decode_yr(arr):
        out = np.empty((H, rtotal), dtype=np.float32)
        for s in range(len(slot_rows)):
            R = slot_rows[s]
            for half in range(2):
                boff = 16 * offs[s] + half * 8 * R
                blk = np.asarray(arr[:, boff:boff + 8 * R], dtype=np.float32)
                out[half * 1024:(half + 1) * 1024, offs[s]:offs[s] + R] = (
                    blk.reshape(128, 8, R).transpose(1, 0, 2).reshape(1024, R))
        return out

    Y = np.concatenate(
        [decode_yr(res.results[c]["yr"]).T for c in range(N_CORES)], axis=0)
    w_flat = (topk_w * SCALE).astype(np.float32).reshape(-1)
    out = (Y[flat_col.reshape(-1)] * w_flat[:, None]).reshape(T, TOPK, H).sum(1)

    ys_sum = np.zeros((128, 16 * T), dtype=np.float32)
    for c in range(N_CORES):
        ys_sum += np.asarray(res.results[c]["ys"], dtype=np.float32)
    shared = np.empty((H, T), dtype=np.float32)
    for ch in range(T // CH):
        for half in range(2):
            boff = (ch * 2 + half) * 8 * CH
            blk = ys_sum[:, boff:boff + 8 * CH]
            shared[half * 1024:(half + 1) * 1024, ch * CH:(ch + 1) * CH] = (
                blk.reshape(128, 8, CH).transpose(1, 0, 2).reshape(1024, CH))
    out += shared.T
    return out.astype(np.float32)



# revision 47
# speedup vs baseline: 1.0918x; 1.0144x over previous
"""MegrezMoE MoE layer on 8 Trainium2 NeuronCores.

Strategy (expert-parallel, host-routed dispatch):
  - Host computes the (tiny) router: logits -> grouped top-k ids/weights,
    exactly mirroring the reference's noaux_tc selection.
  - 32 experts are assigned 4-per-core, balanced by routed-token count.
    Tokens are gathered per expert on the host (transposed: [H, rows],
    rows padded to a per-slot static capacity) so the device kernel is a
    fully static SPMD program: per expert slot, gate_up matmul ->
    silu*mul -> down matmul, streaming the expert weight bank from HBM
    exactly once per core.
  - The shared-expert MLP is tensor-parallel across the 8 cores (each
    core owns a 128-wide slice of the shared intermediate dim) and its
    partial outputs are summed on the host.
  - All device tensors use a partition-major layout ([128, ...] with
    k-tiles concatenated along the free dim) so every DMA is a plain 2D
    contiguous transfer with minimal descriptor overhead.
  - Matmuls run in bf16 with fp32 PSUM accumulation (KERNEL_DTYPE=f32r
    selects a float32r variant that keeps fp32 data in HBM).
  - Host combines: out[t] = sum_k w[t,k]*SCALE * y_col(t,k) + shared[t].

Scheduling notes (measured on HW, each worth 5-10%):
  - _fuse_ldweights() folds the standalone InstLdweights emitted by tile
    legalization into self-loading matmuls; the compiler's automatic
    fast-weight-load then halves the PE weight-load cost (~254us ->
    ~222us).
  - Expert slots accumulate in two 4-bank PSUM waves (pool psum_e) so
    the shared-expert chunks (pool psum_s) interleave with the expert
    stream instead of serializing at the end.
  - Down-projection weights use their own 2-deep pool so the next
    slot's gate_up weight DMA is not WAR-blocked behind the up phase.
  - Output stores ride the Activation HWDGE queue; all loads ride the
    Sync HWDGE queue, whose triggers run ahead of compute.

kernel() takes the full unsharded inputs, returns the full [1024, 2048]
fp32 output.
"""

import os

import ml_dtypes
import numpy as np

import concourse.bass as bass
import concourse.tile as tile
from concourse import bacc, mybir
from concourse.bass_utils import run_bass_kernel_spmd

# Model dims (hardcoded per problem spec)
H = 2048
E = 32
I = 1024
TOPK = 6
NGROUP = 8
TOPKG = 4
SCALE = 2.5
T = 1024

N_CORES = 8
EPC = 4          # experts per core
KT_H = H // 128  # 16 k-tiles over hidden dim
KT_I = I // 128  # 8 k-tiles over intermediate dim
WLOAD_K = 8      # k-tiles per weight DMA
CH = 256         # shared-expert token chunk

F32 = mybir.dt.float32
F32R = mybir.dt.float32r
BF16 = mybir.dt.bfloat16

_PROGRAM_CACHE = {}
LAST_RESULTS = None  # BassKernelResults from the most recent run (for harness)


def _mode():
    return os.environ.get("KERNEL_DTYPE", "bf16")


def _variant():
    """Scheduling-variant knob for A/B benchmarking (0 = default)."""
    return int(os.environ.get("KERNEL_VARIANT", "0"))


# ---------------------------------------------------------------------------
# Host-side routing (mirrors reference._grouped_topk in fp32 numpy)
# ---------------------------------------------------------------------------

def _host_routing(x, gate_w, e_bias):
    logits = x @ gate_w                                   # [T, E] fp32
    scores = 1.0 / (1.0 + np.exp(-logits, dtype=np.float32))
    scores_choice = scores + e_bias[None, :]
    gsize = E // NGROUP
    grp = scores_choice.reshape(T, NGROUP, gsize)
    top2 = np.sort(grp, axis=-1)[:, :, -2:]
    group_scores = top2.sum(-1)                           # [T, G]
    gidx = np.argsort(-group_scores, axis=-1, kind="stable")[:, :TOPKG]
    gmask = np.zeros((T, NGROUP), bool)
    np.put_along_axis(gmask, gidx, True, axis=1)
    emask = np.repeat(gmask, gsize, axis=1)
    masked = np.where(emask, scores_choice, -np.inf)
    topk_ids = np.argsort(-masked, axis=-1, kind="stable")[:, :TOPK]
    topk_w = np.take_along_axis(scores, topk_ids, axis=1)
    topk_w = topk_w / topk_w.sum(-1, keepdims=True)
    return topk_w.astype(np.float32), topk_ids.astype(np.int64)


# ---------------------------------------------------------------------------
# Dispatch plan: expert -> (core, slot), per-slot static row capacities
# ---------------------------------------------------------------------------

def _make_plan(topk_ids):
    counts = np.bincount(topk_ids.ravel(), minlength=E)
    # slot capacity = max routed count in the slot's expert group, rounded
    # up to 8 (DMA alignment)
    padded = np.maximum(16, ((counts + 7) // 8) * 8)
    order = np.argsort(-padded, kind="stable")            # experts, big first
    expert_of = []      # per slot: experts per core
    slot_rows = []
    slot_piece = []     # (piece index, piece capacity) per slot
    for s in range(EPC):
        chunk = order[s * N_CORES:(s + 1) * N_CORES]
        cap = int(padded[chunk].max())
        if cap <= 512:
            expert_of.append(list(chunk))
            slot_rows.append(cap)
            slot_piece.append((0, cap))
        else:
            # an expert group too wide for one PSUM bank: split into
            # pieces of <=512 rows (same expert, partitioned token list)
            n_p = -(-cap // 512)
            pcap = ((-(-cap // n_p) + 7) // 8) * 8
            for j in range(n_p):
                expert_of.append(list(chunk))
                slot_rows.append(pcap)
                slot_piece.append((j, pcap))
    expert_of = np.asarray(expert_of).T                   # [N_CORES, n_slots]
    offs = np.concatenate([[0], np.cumsum(slot_rows)])
    return {
        "expert_of": expert_of,
        "slot_rows": tuple(slot_rows),
        "slot_piece": slot_piece,
        "slot_offs": offs[:-1],
        "rtotal": int(offs[-1]),
        "counts": counts,
    }


# ---------------------------------------------------------------------------
# Bass program (SPMD; one program, per-core data)
# ---------------------------------------------------------------------------

def _fuse_ldweights(nc):
    """Fold each standalone InstLdweights into the following InstMatmult
    (ldweights=True, self-loading form). Tile legalization always splits
    matmuls into LDW+MM pairs; with the split form the PE pays ~102ns of
    unoverlapped LDWEIGHTS per matmul, while self-loading matmuls get
    the compiler's automatic fast-weight-load (~2x weight path). LDW
    waits move onto the matmul (or an event-semaphore right before it
    when the matmul already waits on a different semaphore)."""
    PE = mybir.EngineType.PE
    n_fused = 0
    for fn in nc.m.functions:
        for blk in fn.blocks:
            pending = None
            out = []
            changed = False
            for inst in blk.instructions:
                if isinstance(inst, mybir.InstLdweights) and inst.engine == PE:
                    assert pending is None, "LDW with no consuming matmul"
                    pending = inst
                    changed = True
                    continue
                if isinstance(inst, mybir.InstMatmult) and inst.engine == PE:
                    assert pending is not None, "matmul without its LDW"
                    ldw, pending = pending, None
                    wap, lap = inst.ins[1], ldw.ins[0]
                    assert (wap.memref, wap.offset, str(wap.ap)) == \
                           (lap.memref, lap.offset, str(lap.ap))
                    inst.ldweights = True
                    lsync = ldw.sync_info
                    lw = list(lsync.on_wait) if lsync else []
                    assert not (lsync and lsync.on_update)
                    if lw:
                        msync = inst.sync_info
                        mw = list(msync.on_wait) if msync else []
                        mu = list(msync.on_update) if msync else []
                        extra = []
                        for w in lw:
                            dup = next((x for x in mw if x.id == w.id and
                                        x.wait_mode == w.wait_mode ==
                                        "sem-ge-imm"), None)
                            if dup is not None:
                                if w.wait_value > dup.wait_value:
                                    mw[mw.index(dup)] = w
                            elif not mw:
                                mw.append(w)
                            else:
                                extra.append(w)
                        if extra:
                            ev = mybir.InstEventSemaphore(
                                name=nc.get_next_instruction_name(),
                                ins=[], outs=[])
                            ev.engine = PE
                            ev.sync_info = mybir.SyncInfo(
                                on_wait=extra, on_update=[])
                            nc.register_instruction(ev)
                            out.append(ev)
                        inst.sync_info = mybir.SyncInfo(
                            on_wait=mw, on_update=mu)
                    n_fused += 1
                out.append(inst)
            assert pending is None
            if changed:
                blk.instructions = out
    return n_fused


def _build_program(slot_rows, mode, variant=0):
    rtotal = sum(slot_rows)
    f32r = mode == "f32r"
    DTD = F32 if f32r else BF16      # dram dtype of matmul operands
    DTS = F32R if f32r else BF16     # sbuf dtype of matmul operands
    DTO = F32 if f32r else BF16      # output dtype

    nc = bacc.Bacc("TRN2", target_bir_lowering=False, debug=False,
                   num_devices=N_CORES)

    # DRAM I/O, all partition-major ([128 partitions, free]):
    #   xg : slot-blocked gathered tokens; slot s at cols KT_H*off_s,
    #        k-tile k of slot s at [KT_H*off_s + k*R_s, +R_s]
    #   xf : chunk-blocked all tokens (for the shared expert)
    #   wgu/wd : per (slot, half): k-tiles concatenated along free dim
    #   yr/ys : per (slot/chunk, half): 8 output m-tiles concatenated
    n_slots = len(slot_rows)
    gather = variant == 7 and not f32r
    # with on-chip gather only slot 0 ships host-gathered tokens; slots
    # 1+ are gathered out of the (SBUF-resident) xf by gpsimd
    xg_cols = KT_H * (slot_rows[0] if gather else rtotal)
    rgath = sum(r // 4 for r in slot_rows[1:])
    xg = nc.dram_tensor("xg", [128, xg_cols], DTD, kind="ExternalInput")
    xf = nc.dram_tensor("xf", [128, KT_H * T], DTD, kind="ExternalInput")
    if gather:
        xidx = nc.dram_tensor("xidx", [128, rgath], mybir.dt.uint16,
                              kind="ExternalInput")
    wgu = nc.dram_tensor("wgu", [n_slots, 2, 128, KT_H * I], DTD,
                         kind="ExternalInput")
    wd = nc.dram_tensor("wd", [n_slots, 2, 128, KT_I * I], DTD,
                        kind="ExternalInput")
    wsg = nc.dram_tensor("wsg", [128, KT_H * 256], DTD, kind="ExternalInput")
    wsd = nc.dram_tensor("wsd", [128, H], DTD, kind="ExternalInput")
    yr = nc.dram_tensor("yr", [128, 16 * rtotal], DTO, kind="ExternalOutput")
    ys = nc.dram_tensor("ys", [128, 16 * T], DTO, kind="ExternalOutput")

    # casting DMA (fp32 dram -> f32r sbuf) must go via SWDGE
    ldma = nc.gpsimd.dma_start if f32r else nc.sync.dma_start
    xdma = ldma

    slot_offs = [0]
    for R in slot_rows[:-1]:
        slot_offs.append(slot_offs[-1] + R)

    # Input loads (weights + x) all ride the Sync HWDGE queue — triggers
    # run well ahead of compute there. Output stores go on the Activation
    # HWDGE queue: their deps (the drain copies) are scalar/vector-local,
    # so a store trigger never blocks that queue's forward progress.
    sdma = ldma
    odma = nc.gpsimd.dma_start if f32r else nc.scalar.dma_start

    n_wbufs = 5 if variant == 1 else 4
    n_xbufs = 4 if variant == 5 else 3
    split_store = variant == 6

    with tile.TileContext(nc) as tc:
        with tc.tile_pool(name="psum_e", bufs=5, space="PSUM") as pe_pool, \
             tc.tile_pool(name="psum_s", bufs=3, space="PSUM") as psh_pool, \
             tc.tile_pool(name="swg", bufs=1) as swg_pool, \
             tc.tile_pool(name="swd", bufs=1) as swd_pool, \
             tc.tile_pool(name="sxf", bufs=1 if gather else 3) as sxf_pool, \
             tc.tile_pool(name="xidxp", bufs=1) as xidx_pool, \
             tc.tile_pool(name="sact", bufs=2) as sact_pool, \
             tc.tile_pool(name="sout", bufs=2) as sout_pool, \
             tc.tile_pool(name="wsl", bufs=n_wbufs) as w_pool, \
             tc.tile_pool(name="wdl", bufs=2) as wd_pool_, \
             tc.tile_pool(name="xs", bufs=n_xbufs) as x_pool, \
             tc.tile_pool(name="gs", bufs=2) as g_pool, \
             tc.tile_pool(name="at", bufs=2) as a_pool, \
             tc.tile_pool(name="ost", bufs=2) as o_pool:
            # down-projection weights get their own 2-deep pool so the
            # next slot's gate_up weight DMA is not WAR-blocked behind
            # the up phase (measured −16us vs a single 4-deep pool)
            dw_pool = w_pool if variant == 4 else wd_pool_

            wsg_sb = swg_pool.tile([128, KT_H * 256], DTS)
            wsd_sb = swd_pool.tile([128, H], DTS)
            if gather:
                xf_full = sxf_pool.tile([128, KT_H * T], DTS, name="xf_full")
                xidx_sb = xidx_pool.tile([128, rgath], mybir.dt.uint16,
                                         name="xidx_sb")

            def shared_weights():
                sdma(wsg_sb[:], wsg.ap())
                sdma(wsd_sb[:], wsd.ap())

            def shared_chunk(ch):
                if gather:
                    xf_t, xf_b = xf_full, ch * KT_H * CH
                else:
                    xf_t = sxf_pool.tile([128, KT_H * CH], DTS, name="xf_sb")
                    sdma(xf_t[:],
                         xf.ap()[:, ch * KT_H * CH:(ch + 1) * KT_H * CH])
                    xf_b = 0
                ps_g = psh_pool.tile([128, CH], F32, tag="ps", name="ps_g")
                ps_u = psh_pool.tile([128, CH], F32, tag="ps", name="ps_u")
                for k in range(KT_H):
                    lg = wsg_sb[:, k * 256:k * 256 + 128]
                    lu = wsg_sb[:, k * 256 + 128:k * 256 + 256]
                    rx = xf_t[:, xf_b + k * CH:xf_b + (k + 1) * CH]
                    nc.tensor.matmul(ps_g[:], lg, rx,
                                     start=(k == 0), stop=(k == KT_H - 1))
                    nc.tensor.matmul(ps_u[:], lu, rx,
                                     start=(k == 0), stop=(k == KT_H - 1))
                gss = sact_pool.tile([128, CH], F32, tag="sgs", name="gss")
                nc.scalar.activation(gss[:], ps_g[:],
                                     mybir.ActivationFunctionType.Sigmoid)
                nc.vector.tensor_mul(gss[:], gss[:], ps_g[:])
                a_s = sact_pool.tile([128, CH], DTS, tag="sas", name="a_s")
                nc.vector.tensor_mul(a_s[:], gss[:], ps_u[:])
                # down: 16 output m-tiles, single k (the 128-slice of I);
                # vector does the PSUM drain copies (scalar is busy with
                # the expert-slot drains and DMA triggers)
                for half in range(2):
                    stg = sout_pool.tile([128, 8 * CH], DTO, tag="sstg",
                                         name="stg")
                    for m in range(8):
                        pd = psh_pool.tile([128, CH], F32, tag="ps",
                                           name="pd")
                        lw = wsd_sb[:, (half * 8 + m) * 128:
                                    (half * 8 + m + 1) * 128]
                        nc.tensor.matmul(pd[:], lw, a_s[:],
                                         start=True, stop=True)
                        nc.vector.tensor_copy(stg[:, m * CH:(m + 1) * CH],
                                              pd[:])
                    odma(ys.ap()[:, (ch * 2 + half) * 8 * CH:
                                 (ch * 2 + half + 1) * 8 * CH],
                         stg[:])

            def expert_slot(s):
                R = slot_rows[s]
                off = slot_offs[s]
                xbase = KT_H * off
                wt0 = None
                s0_wts = None
                xs = x_pool.tile([128, KT_H * R], DTS, tag="xs", name="xs")
                if s == 0 and variant == 3 and not f32r:
                    # ramp: interleave phase-0 weight groups with the xs
                    # k-ranges they need
                    s0_wts = []
                    for (k0, k1) in [(0, 1), (1, 2), (2, 8), (8, 16)]:
                        wt = w_pool.tile([128, (k1 - k0) * I], DTS,
                                         tag="wsl", name="wt")
                        ldma(wt[:], wgu.ap()[0, 0][:, k0 * I:k1 * I])
                        s0_wts.append(wt)
                        sdma(xs[:, k0 * R:k1 * R],
                             xg.ap()[:, xbase + k0 * R:xbase + k1 * R])
                elif s == 0:
                    if not f32r:
                        # first weight group is the very first DMA on the
                        # sync queue so the PE can start early
                        wt0 = w_pool.tile([128, I], DTS, tag="wsl",
                                          name="wt")
                        ldma(wt0[:], wgu.ap()[0, 0][:, 0:I])
                    # split the first x load so the PE can start early
                    sdma(xs[:, :R], xg.ap()[:, xbase:xbase + R])
                    sdma(xs[:, R:2 * R], xg.ap()[:, xbase + R:xbase + 2 * R])
                    sdma(xs[:, 2 * R:], xg.ap()[:, xbase + 2 * R:
                                                xbase + KT_H * R])
                    if gather:
                        # piggyback the gather inputs on the sync queue
                        # right behind slot-0's x so they land before
                        # slot 1 needs them
                        sdma(xidx_sb[:], xidx.ap())
                        for ch4 in range(4):
                            sdma(xf_full[:, ch4 * 4096:(ch4 + 1) * 4096],
                                 xf.ap()[:, ch4 * 4096:(ch4 + 1) * 4096])
                elif gather:
                    # gather xs out of the resident xf, 4 k-tiles per call
                    # (ISA caps the indirect-copy dst element count); the
                    # relative index set is shared, the k-offset moves the
                    # data base
                    goff = sum(r // 4 for r in slot_rows[1:s])
                    ncol = R // 4
                    for kb4 in range(4):
                        nc.gpsimd.indirect_copy(
                            xs[:, kb4 * 4 * R:(kb4 + 1) * 4 * R],
                            xf_full[:, kb4 * 4 * CH:],
                            xidx_sb[:, goff:goff + ncol], True)
                else:
                    sdma(xs[:], xg.ap()[:, xbase:xbase + KT_H * R])

                gs = g_pool.tile([128, KT_I * R], DTS if gather else F32,
                                 tag="gs", name="gs")
                at = a_pool.tile([128, KT_I * R], DTS, tag="at", name="at")

                for phase in range(2):  # 0 = gate, 1 = up
                    def gu_drain(m, ps_m):
                        if phase == 0:
                            nc.scalar.activation(
                                gs[:, m * R:(m + 1) * R], ps_m[:],
                                mybir.ActivationFunctionType.Sigmoid)
                            nc.vector.tensor_mul(
                                gs[:, m * R:(m + 1) * R],
                                gs[:, m * R:(m + 1) * R], ps_m[:])
                        else:
                            nc.vector.tensor_mul(
                                at[:, m * R:(m + 1) * R],
                                gs[:, m * R:(m + 1) * R], ps_m[:])

                    if s == 0 and phase == 0:
                        groups = [(0, 1), (1, 2), (2, 8), (8, 16)]
                    else:
                        groups = [(kb * WLOAD_K, (kb + 1) * WLOAD_K)
                                  for kb in range(KT_H // WLOAD_K)]
                    if s == 0 and phase == 0 and s0_wts is not None:
                        wts = s0_wts
                    else:
                        wts = []
                        for (k0, k1) in groups:
                            if s == 0 and phase == 0 and k0 == 0 and wt0:
                                wts.append(wt0)
                                continue
                            wt = w_pool.tile([128, (k1 - k0) * I], DTS,
                                             tag="wsl", name="wt")
                            ldma(wt[:], wgu.ap()[s, phase][:, k0 * I:k1 * I])
                            wts.append(wt)
                    # two 4-m-tile waves: expert slots hold at most 4+2
                    # PSUM banks so shared chunks can interleave
                    for wave in range(2):
                        ps = [pe_pool.tile([128, R], F32, tag="ps",
                                           name="ps") for _ in range(4)]
                        for gi, (k0, k1) in enumerate(groups):
                            wt = wts[gi]
                            for kk in range(k1 - k0):
                                k = k0 + kk
                                rx = xs[:, k * R:(k + 1) * R]
                                for mi in range(4):
                                    m = wave * 4 + mi
                                    lw = wt[:, kk * I + m * 128:
                                            kk * I + (m + 1) * 128]
                                    nc.tensor.matmul(
                                        ps[mi][:], lw, rx,
                                        start=(k == 0), stop=(k == KT_H - 1))
                        for mi in range(4):
                            gu_drain(wave * 4 + mi, ps[mi])

                WLD = min(WLOAD_K, KT_I)
                for half in range(2):
                    wts = []
                    for kb in range(KT_I // WLD):
                        wt = dw_pool.tile([128, WLD * I], DTS, tag="wsl",
                                          name="wt")
                        ldma(wt[:], wd.ap()[s, half][:, kb * WLD * I:
                                                     (kb + 1) * WLD * I])
                        wts.append(wt)
                    stg = o_pool.tile([128, 8 * R], DTO, tag="ost", name="stg")
                    boff = 16 * off + half * 8 * R
                    for wave in range(2):
                        ps = [pe_pool.tile([128, R], F32, tag="ps",
                                           name="ps") for _ in range(4)]
                        for kb in range(KT_I // WLD):
                            wt = wts[kb]
                            for kk in range(WLD):
                                k = kb * WLD + kk
                                ra = at[:, k * R:(k + 1) * R]
                                for mi in range(4):
                                    m = wave * 4 + mi
                                    lw = wt[:, kk * I + m * 128:
                                            kk * I + (m + 1) * 128]
                                    nc.tensor.matmul(
                                        ps[mi][:], lw, ra,
                                        start=(k == 0), stop=(k == KT_I - 1))
                        for mi in range(4):
                            m = wave * 4 + mi
                            nc.scalar.copy(stg[:, m * R:(m + 1) * R],
                                           ps[mi][:])
                        if split_store:
                            w0 = wave * 4 * R
                            odma(yr.ap()[:, boff + w0:boff + w0 + 4 * R],
                                 stg[:, w0:w0 + 4 * R])
                    if not split_store:
                        odma(yr.ap()[:, boff:boff + 8 * R], stg[:])

            # experts carry the bulk of the DMA stream; shared-expert
            # chunks are interleaved to fill PE gaps at phase boundaries
            if variant == 8:
                expert_slot(0)
                shared_weights()
                shared_chunk(0)
                for s_i in range(1, n_slots):
                    expert_slot(s_i)
                    shared_chunk(s_i)
            else:
                expert_slot(0)
                shared_weights()
                shared_chunk(0)
                shared_chunk(1)
                for s_i in range(1, n_slots):
                    if s_i == 1:
                        expert_slot(1)
                        shared_chunk(2)
                    elif s_i == 2:
                        expert_slot(2)
                        shared_chunk(3)
                    else:
                        expert_slot(s_i)

    _fuse_ldweights(nc)
    nc.compile()
    return nc


def _get_program(slot_rows, mode):
    variant = _variant()
    key = (tuple(slot_rows), mode, variant)
    if key not in _PROGRAM_CACHE:
        _PROGRAM_CACHE[key] = _build_program(slot_rows, mode, variant)
    return _PROGRAM_CACHE[key]


# ---------------------------------------------------------------------------
# Per-core input construction (host shard + reorder + cast)
# ---------------------------------------------------------------------------

def _pm(a):
    """[KT, 128, M] -> partition-major [128, KT*M]."""
    kt, p, m = a.shape
    return np.ascontiguousarray(a.transpose(1, 0, 2)).reshape(p, kt * m)


def _make_in_maps(x, w_gate_up, w_down, shared_gate_up, shared_down,
                  topk_ids, plan, mode):
    rtotal = plan["rtotal"]
    slot_rows = plan["slot_rows"]
    offs = plan["slot_offs"]
    expert_of = plan["expert_of"]
    np_dt = np.float32 if mode == "f32r" else ml_dtypes.bfloat16

    slot_piece = plan.get("slot_piece") or [(0, r) for r in slot_rows]
    n_slots = len(slot_rows)
    tok_of = [np.where((topk_ids == e).any(axis=1))[0] for e in range(E)]
    flat_col = np.zeros((T, TOPK), dtype=np.int64)

    xT = np.ascontiguousarray(x.T).astype(np_dt)          # [H, T]
    # weights -> [E, 2, 128, KT*I] partition-major k-slab layout
    wgu_pm = np.ascontiguousarray(
        w_gate_up.astype(np_dt).reshape(E, KT_H, 128, 2, I)
        .transpose(0, 3, 2, 1, 4)).reshape(E, 2, 128, KT_H * I)
    wd_pm = np.ascontiguousarray(
        w_down.astype(np_dt).reshape(E, KT_I, 128, 2, I)
        .transpose(0, 3, 2, 1, 4)).reshape(E, 2, 128, KT_I * I)

    xf_pm = np.concatenate(
        [_pm(xT[:, ch * CH:(ch + 1) * CH].reshape(KT_H, 128, CH))
         for ch in range(T // CH)], axis=1)

    gather = _variant() == 7 and mode != "f32r"
    rgath = sum(r // 4 for r in slot_rows[1:])

    in_maps = []
    for c in range(N_CORES):
        xg_pm = np.zeros(
            (128, KT_H * (slot_rows[0] if gather else rtotal)), dtype=np_dt)
        xidx = np.zeros((128, max(rgath, 1)), dtype=np.uint16)
        for s in range(n_slots):
            e = expert_of[c, s]
            pj, pcap = slot_piece[s]
            toks = tok_of[e][pj * pcap:(pj + 1) * pcap]
            R = slot_rows[s]
            if gather and s > 0:
                # wrapped gather indices for one 4-k-tile batch: output
                # col i (= kt_local*R + j) has its index at
                # [i % 16, goff + i // 16], replicated over the eight
                # 16-partition groups; later batches reuse the set with
                # a shifted data base
                goff = sum(r // 4 for r in slot_rows[1:s])
                tpad = np.zeros(R, dtype=np.int64)
                tpad[:len(toks)] = toks
                base = (tpad // CH) * (KT_H * CH) + (tpad % CH)
                colidx = np.concatenate(
                    [base + kt * CH for kt in range(4)]).astype(np.uint16)
                wrapped = colidx.reshape(R // 4, 16).T   # [16, R/4]
                xidx[:, goff:goff + R // 4] = np.tile(wrapped, (8, 1))
            else:
                blk = np.zeros((H, R), dtype=np_dt)
                blk[:, :len(toks)] = xT[:, toks]
                xg_pm[:, KT_H * offs[s]:KT_H * (offs[s] + R)] = _pm(
                    blk.reshape(KT_H, 128, R))
            col_base = c * rtotal + offs[s]
            for pos, t in enumerate(toks):
                for k in np.nonzero(topk_ids[t] == e)[0]:
                    flat_col[t, k] = col_base + pos
        sl = slice(c * 128, (c + 1) * 128)
        sgu_slice = np.concatenate(
            [shared_gate_up[:, sl],
             shared_gate_up[:, 1024 + c * 128:1024 + (c + 1) * 128]],
            axis=1).astype(np_dt)
        im = {
            "xg": xg_pm,
            "xf": xf_pm,
            "wgu": np.ascontiguousarray(wgu_pm[expert_of[c]]),
            "wd": np.ascontiguousarray(wd_pm[expert_of[c]]),
            "wsg": _pm(sgu_slice.reshape(KT_H, 128, 256)),
            "wsd": np.ascontiguousarray(shared_down[sl, :]).astype(np_dt),
        }
        if gather:
            im["xidx"] = xidx
        in_maps.append(im)
    return in_maps, flat_col


# ---------------------------------------------------------------------------
# Entry point
# ---------------------------------------------------------------------------

def kernel(hidden_states, gate_w, e_bias, w_gate_up, w_down,
           shared_gate_up, shared_down):
    global LAST_RESULTS
    mode = _mode()
    x = np.ascontiguousarray(np.asarray(hidden_states, dtype=np.float32))
    gate_w = np.asarray(gate_w, dtype=np.float32)
    e_bias = np.asarray(e_bias, dtype=np.float32)
    w_gate_up = np.asarray(w_gate_up, dtype=np.float32)
    w_down = np.asarray(w_down, dtype=np.float32)
    shared_gate_up = np.asarray(shared_gate_up, dtype=np.float32)
    shared_down = np.asarray(shared_down, dtype=np.float32)

    topk_w, topk_ids = _host_routing(x, gate_w, e_bias)
    plan = _make_plan(topk_ids)
    slot_rows = plan["slot_rows"]
    offs = plan["slot_offs"]
    rtotal = plan["rtotal"]

    nc = _get_program(slot_rows, mode)
    in_maps, flat_col = _make_in_maps(
        x, w_gate_up, w_down, shared_gate_up, shared_down,
        topk_ids, plan, mode)

    trace = bool(int(os.environ.get("KERNEL_TRACE", "0")))
    res = run_bass_kernel_spmd(
        nc, in_maps, list(range(N_CORES)), trace=trace,
        tmpdir=os.environ.get("KERNEL_TRACE_DIR") or None)
    LAST_RESULTS = res

    # decode partition-major outputs back to [H, rtotal] per core
    def decode_yr(arr):
        out = np.empty((H, rtotal), dtype=np.float32)
        for s in range(len(slot_rows)):
            R = slot_rows[s]
            for half in range(2):
                boff = 16 * offs[s] + half * 8 * R
                blk = np.asarray(arr[:, boff:boff + 8 * R], dtype=np.float32)
                out[half * 1024:(half + 1) * 1024, offs[s]:offs[s] + R] = (
                    blk.reshape(128, 8, R).transpose(1, 0, 2).reshape(1024, R))
        return out

    Y = np.concatenate(
        [decode_yr(res.results[c]["yr"]).T for c in range(N_CORES)], axis=0)
    w_flat = (topk_w * SCALE).astype(np.float32).reshape(-1)
    out = (Y[flat_col.reshape(-1)] * w_flat[:, None]).reshape(T, TOPK, H).sum(1)

    ys_sum = np.zeros((128, 16 * T), dtype=np.float32)
    for c in range(N_CORES):
        ys_sum += np.asarray(res.results[c]["ys"], dtype=np.float32)
    shared = np.empty((H, T), dtype=np.float32)
    for ch in range(T // CH):
        for half in range(2):
            boff = (ch * 2 + half) * 8 * CH
            blk = ys_sum[:, boff:boff + 8 * CH]
            shared[half * 1024:(half + 1) * 1024, ch * CH:(ch + 1) * CH] = (
                blk.reshape(128, 8, CH).transpose(1, 0, 2).reshape(1024, CH))
    out += shared.T
    return out.astype(np.float32)



# revision 51
# speedup vs baseline: 1.1030x; 1.0102x over previous
"""MegrezMoE MoE layer on 8 Trainium2 NeuronCores.

Strategy (expert-parallel, host-routed dispatch):
  - Host computes the (tiny) router: logits -> grouped top-k ids/weights,
    exactly mirroring the reference's noaux_tc selection.
  - 32 experts are assigned 4-per-core, balanced by routed-token count.
    Tokens are gathered per expert on the host (transposed: [H, rows],
    rows padded to a per-slot static capacity) so the device kernel is a
    fully static SPMD program: per expert slot, gate_up matmul ->
    silu*mul -> down matmul, streaming the expert weight bank from HBM
    exactly once per core.
  - The shared-expert MLP is tensor-parallel across the 8 cores (each
    core owns a 128-wide slice of the shared intermediate dim) and its
    partial outputs are summed on the host.
  - All device tensors use a partition-major layout ([128, ...] with
    k-tiles concatenated along the free dim) so every DMA is a plain 2D
    contiguous transfer with minimal descriptor overhead.
  - Matmuls run in bf16 with fp32 PSUM accumulation (KERNEL_DTYPE=f32r
    selects a float32r variant that keeps fp32 data in HBM).
  - Host combines: out[t] = sum_k w[t,k]*SCALE * y_col(t,k) + shared[t].

Scheduling notes (measured on HW, each worth 5-10%):
  - _fuse_ldweights() folds the standalone InstLdweights emitted by tile
    legalization into self-loading matmuls; the compiler's automatic
    fast-weight-load then halves the PE weight-load cost (~254us ->
    ~222us).
  - Expert slots accumulate in two 4-bank PSUM waves (pool psum_e) so
    the shared-expert chunks (pool psum_s) interleave with the expert
    stream instead of serializing at the end.
  - Down-projection weights use their own 2-deep pool so the next
    slot's gate_up weight DMA is not WAR-blocked behind the up phase.
  - Output stores ride the Activation HWDGE queue; all loads ride the
    Sync HWDGE queue, whose triggers run ahead of compute.

kernel() takes the full unsharded inputs, returns the full [1024, 2048]
fp32 output.
"""

import os

import ml_dtypes
import numpy as np

import concourse.bass as bass
import concourse.tile as tile
from concourse import bacc, mybir
from concourse.bass_utils import run_bass_kernel_spmd

# Model dims (hardcoded per problem spec)
H = 2048
E = 32
I = 1024
TOPK = 6
NGROUP = 8
TOPKG = 4
SCALE = 2.5
T = 1024

N_CORES = 8
EPC = 4          # experts per core
KT_H = H // 128  # 16 k-tiles over hidden dim
KT_I = I // 128  # 8 k-tiles over intermediate dim
WLOAD_K = 8      # k-tiles per weight DMA
CH = 256         # shared-expert token chunk

F32 = mybir.dt.float32
F32R = mybir.dt.float32r
BF16 = mybir.dt.bfloat16

_PROGRAM_CACHE = {}
LAST_RESULTS = None  # BassKernelResults from the most recent run (for harness)


def _mode():
    return os.environ.get("KERNEL_DTYPE", "bf16")


def _variant():
    """Scheduling-variant knob for A/B benchmarking (0 = default)."""
    return int(os.environ.get("KERNEL_VARIANT", "0"))


# ---------------------------------------------------------------------------
# Host-side routing (mirrors reference._grouped_topk in fp32 numpy)
# ---------------------------------------------------------------------------

def _host_routing(x, gate_w, e_bias):
    logits = x @ gate_w                                   # [T, E] fp32
    scores = 1.0 / (1.0 + np.exp(-logits, dtype=np.float32))
    scores_choice = scores + e_bias[None, :]
    gsize = E // NGROUP
    grp = scores_choice.reshape(T, NGROUP, gsize)
    top2 = np.sort(grp, axis=-1)[:, :, -2:]
    group_scores = top2.sum(-1)                           # [T, G]
    gidx = np.argsort(-group_scores, axis=-1, kind="stable")[:, :TOPKG]
    gmask = np.zeros((T, NGROUP), bool)
    np.put_along_axis(gmask, gidx, True, axis=1)
    emask = np.repeat(gmask, gsize, axis=1)
    masked = np.where(emask, scores_choice, -np.inf)
    topk_ids = np.argsort(-masked, axis=-1, kind="stable")[:, :TOPK]
    topk_w = np.take_along_axis(scores, topk_ids, axis=1)
    topk_w = topk_w / topk_w.sum(-1, keepdims=True)
    return topk_w.astype(np.float32), topk_ids.astype(np.int64)


# ---------------------------------------------------------------------------
# Dispatch plan: expert -> (core, slot), per-slot static row capacities
# ---------------------------------------------------------------------------

def _make_plan(topk_ids):
    counts = np.bincount(topk_ids.ravel(), minlength=E)
    # slot capacity = max routed count in the slot's expert group, rounded
    # up to 8 (DMA alignment)
    padded = np.maximum(16, ((counts + 7) // 8) * 8)
    order = np.argsort(-padded, kind="stable")            # experts, big first
    expert_of = []      # per slot: experts per core
    slot_rows = []
    slot_piece = []     # (piece index, piece capacity) per slot
    for s in range(EPC):
        chunk = order[s * N_CORES:(s + 1) * N_CORES]
        cap = int(padded[chunk].max())
        if cap <= 512:
            expert_of.append(list(chunk))
            slot_rows.append(cap)
            slot_piece.append((0, cap))
        else:
            # an expert group too wide for one PSUM bank: split into
            # pieces of <=512 rows (same expert, partitioned token list)
            n_p = -(-cap // 512)
            pcap = ((-(-cap // n_p) + 7) // 8) * 8
            for j in range(n_p):
                expert_of.append(list(chunk))
                slot_rows.append(pcap)
                slot_piece.append((j, pcap))
    expert_of = np.asarray(expert_of).T                   # [N_CORES, n_slots]
    offs = np.concatenate([[0], np.cumsum(slot_rows)])
    return {
        "expert_of": expert_of,
        "slot_rows": tuple(slot_rows),
        "slot_piece": slot_piece,
        "slot_offs": offs[:-1],
        "rtotal": int(offs[-1]),
        "counts": counts,
    }


# ---------------------------------------------------------------------------
# Bass program (SPMD; one program, per-core data)
# ---------------------------------------------------------------------------

def _fuse_ldweights(nc):
    """Fold each standalone InstLdweights into the following InstMatmult
    (ldweights=True, self-loading form). Tile legalization always splits
    matmuls into LDW+MM pairs; with the split form the PE pays ~102ns of
    unoverlapped LDWEIGHTS per matmul, while self-loading matmuls get
    the compiler's automatic fast-weight-load (~2x weight path). LDW
    waits move onto the matmul (or an event-semaphore right before it
    when the matmul already waits on a different semaphore)."""
    PE = mybir.EngineType.PE
    n_fused = 0
    for fn in nc.m.functions:
        for blk in fn.blocks:
            pending = None
            out = []
            changed = False
            for inst in blk.instructions:
                if isinstance(inst, mybir.InstLdweights) and inst.engine == PE:
                    assert pending is None, "LDW with no consuming matmul"
                    pending = inst
                    changed = True
                    continue
                if isinstance(inst, mybir.InstMatmult) and inst.engine == PE:
                    assert pending is not None, "matmul without its LDW"
                    ldw, pending = pending, None
                    wap, lap = inst.ins[1], ldw.ins[0]
                    assert (wap.memref, wap.offset, str(wap.ap)) == \
                           (lap.memref, lap.offset, str(lap.ap))
                    inst.ldweights = True
                    lsync = ldw.sync_info
                    lw = list(lsync.on_wait) if lsync else []
                    assert not (lsync and lsync.on_update)
                    if lw:
                        msync = inst.sync_info
                        mw = list(msync.on_wait) if msync else []
                        mu = list(msync.on_update) if msync else []
                        extra = []
                        for w in lw:
                            dup = next((x for x in mw if x.id == w.id and
                                        x.wait_mode == w.wait_mode ==
                                        "sem-ge-imm"), None)
                            if dup is not None:
                                if w.wait_value > dup.wait_value:
                                    mw[mw.index(dup)] = w
                            elif not mw:
                                mw.append(w)
                            else:
                                extra.append(w)
                        if extra:
                            ev = mybir.InstEventSemaphore(
                                name=nc.get_next_instruction_name(),
                                ins=[], outs=[])
                            ev.engine = PE
                            ev.sync_info = mybir.SyncInfo(
                                on_wait=extra, on_update=[])
                            nc.register_instruction(ev)
                            out.append(ev)
                        inst.sync_info = mybir.SyncInfo(
                            on_wait=mw, on_update=mu)
                    n_fused += 1
                out.append(inst)
            assert pending is None
            if changed:
                blk.instructions = out
    return n_fused


def _build_program(slot_rows, mode, variant=0):
    rtotal = sum(slot_rows)
    f32r = mode == "f32r"
    DTD = F32 if f32r else BF16      # dram dtype of matmul operands
    DTS = F32R if f32r else BF16     # sbuf dtype of matmul operands
    DTO = F32 if f32r else BF16      # output dtype

    nc = bacc.Bacc("TRN2", target_bir_lowering=False, debug=False,
                   num_devices=N_CORES)

    # DRAM I/O, all partition-major ([128 partitions, free]):
    #   xg : slot-blocked gathered tokens; slot s at cols KT_H*off_s,
    #        k-tile k of slot s at [KT_H*off_s + k*R_s, +R_s]
    #   xf : chunk-blocked all tokens (for the shared expert)
    #   wgu/wd : per (slot, half): k-tiles concatenated along free dim
    #   yr/ys : per (slot/chunk, half): 8 output m-tiles concatenated
    n_slots = len(slot_rows)
    gather = variant == 7 and not f32r
    # with on-chip gather only slot 0 ships host-gathered tokens; slots
    # 1+ are gathered out of the (SBUF-resident) xf by gpsimd
    xg_cols = KT_H * (slot_rows[0] if gather else rtotal)
    rgath = sum(r // 4 for r in slot_rows[1:])
    xg = nc.dram_tensor("xg", [128, xg_cols], DTD, kind="ExternalInput")
    xf = nc.dram_tensor("xf", [128, KT_H * T], DTD, kind="ExternalInput")
    if gather:
        xidx = nc.dram_tensor("xidx", [128, rgath], mybir.dt.uint16,
                              kind="ExternalInput")
    wgu = nc.dram_tensor("wgu", [n_slots, 2, 128, KT_H * I], DTD,
                         kind="ExternalInput")
    wd = nc.dram_tensor("wd", [n_slots, 2, 128, KT_I * I], DTD,
                        kind="ExternalInput")
    wsg = nc.dram_tensor("wsg", [128, KT_H * 256], DTD, kind="ExternalInput")
    wsd = nc.dram_tensor("wsd", [128, H], DTD, kind="ExternalInput")
    yr = nc.dram_tensor("yr", [128, 16 * rtotal], DTO, kind="ExternalOutput")
    ys = nc.dram_tensor("ys", [128, 16 * T], DTO, kind="ExternalOutput")

    # casting DMA (fp32 dram -> f32r sbuf) must go via SWDGE
    ldma = nc.gpsimd.dma_start if f32r else nc.sync.dma_start
    xdma = ldma

    slot_offs = [0]
    for R in slot_rows[:-1]:
        slot_offs.append(slot_offs[-1] + R)

    # Input loads (weights + x) all ride the Sync HWDGE queue — triggers
    # run well ahead of compute there. Output stores go on the Activation
    # HWDGE queue: their deps (the drain copies) are scalar/vector-local,
    # so a store trigger never blocks that queue's forward progress.
    sdma = ldma
    odma = nc.gpsimd.dma_start if f32r else nc.scalar.dma_start

    n_wbufs = 5 if variant == 1 else 4
    n_xbufs = 4 if variant == 5 else 3
    split_store = variant == 6

    with tile.TileContext(nc) as tc:
        with tc.tile_pool(name="psum_e", bufs=5, space="PSUM") as pe_pool, \
             tc.tile_pool(name="psum_s", bufs=3, space="PSUM") as psh_pool, \
             tc.tile_pool(name="swg", bufs=1) as swg_pool, \
             tc.tile_pool(name="swd", bufs=1) as swd_pool, \
             tc.tile_pool(name="sxf", bufs=1 if gather else 3) as sxf_pool, \
             tc.tile_pool(name="xidxp", bufs=1) as xidx_pool, \
             tc.tile_pool(name="sact", bufs=2) as sact_pool, \
             tc.tile_pool(name="sout", bufs=2) as sout_pool, \
             tc.tile_pool(name="wsl", bufs=n_wbufs) as w_pool, \
             tc.tile_pool(name="wdl", bufs=2) as wd_pool_, \
             tc.tile_pool(name="xs", bufs=n_xbufs) as x_pool, \
             tc.tile_pool(name="gs", bufs=2) as g_pool, \
             tc.tile_pool(name="at", bufs=2) as a_pool, \
             tc.tile_pool(name="ost", bufs=2) as o_pool:
            # down-projection weights get their own 2-deep pool so the
            # next slot's gate_up weight DMA is not WAR-blocked behind
            # the up phase (measured −16us vs a single 4-deep pool)
            dw_pool = w_pool if variant == 4 else wd_pool_

            wsg_sb = swg_pool.tile([128, KT_H * 256], DTS)
            wsd_sb = swd_pool.tile([128, H], DTS)
            if gather:
                xf_full = sxf_pool.tile([128, KT_H * T], DTS, name="xf_full")
                xidx_sb = xidx_pool.tile([128, rgath], mybir.dt.uint16,
                                         name="xidx_sb")

            def shared_weights():
                sdma(wsg_sb[:], wsg.ap())
                sdma(wsd_sb[:], wsd.ap())

            def shared_chunk(ch):
                if gather:
                    xf_t, xf_b = xf_full, ch * KT_H * CH
                else:
                    xf_t = sxf_pool.tile([128, KT_H * CH], DTS, name="xf_sb")
                    sdma(xf_t[:],
                         xf.ap()[:, ch * KT_H * CH:(ch + 1) * KT_H * CH])
                    xf_b = 0
                ps_g = psh_pool.tile([128, CH], F32, tag="ps", name="ps_g")
                ps_u = psh_pool.tile([128, CH], F32, tag="ps", name="ps_u")
                for k in range(KT_H):
                    lg = wsg_sb[:, k * 256:k * 256 + 128]
                    lu = wsg_sb[:, k * 256 + 128:k * 256 + 256]
                    rx = xf_t[:, xf_b + k * CH:xf_b + (k + 1) * CH]
                    nc.tensor.matmul(ps_g[:], lg, rx,
                                     start=(k == 0), stop=(k == KT_H - 1))
                    nc.tensor.matmul(ps_u[:], lu, rx,
                                     start=(k == 0), stop=(k == KT_H - 1))
                gss = sact_pool.tile([128, CH], F32, tag="sgs", name="gss")
                nc.scalar.activation(gss[:], ps_g[:],
                                     mybir.ActivationFunctionType.Sigmoid)
                nc.vector.tensor_mul(gss[:], gss[:], ps_g[:])
                a_s = sact_pool.tile([128, CH], DTS, tag="sas", name="a_s")
                nc.vector.tensor_mul(a_s[:], gss[:], ps_u[:])
                # down: 16 output m-tiles, single k (the 128-slice of I);
                # vector does the PSUM drain copies (scalar is busy with
                # the expert-slot drains and DMA triggers)
                for half in range(2):
                    stg = sout_pool.tile([128, 8 * CH], DTO, tag="sstg",
                                         name="stg")
                    for m in range(8):
                        pd = psh_pool.tile([128, CH], F32, tag="ps",
                                           name="pd")
                        lw = wsd_sb[:, (half * 8 + m) * 128:
                                    (half * 8 + m + 1) * 128]
                        nc.tensor.matmul(pd[:], lw, a_s[:],
                                         start=True, stop=True)
                        nc.vector.tensor_copy(stg[:, m * CH:(m + 1) * CH],
                                              pd[:])
                    odma(ys.ap()[:, (ch * 2 + half) * 8 * CH:
                                 (ch * 2 + half + 1) * 8 * CH],
                         stg[:])

            def expert_slot(s):
                R = slot_rows[s]
                off = slot_offs[s]
                xbase = KT_H * off
                wt0 = None
                s0_wts = None
                xs = x_pool.tile([128, KT_H * R], DTS, tag="xs", name="xs")
                if s == 0 and variant in (9, 10) and not f32r:
                    # ramp: weights stream on the sync queue, x pieces on
                    # the scalar queue in k-dependency order, so both
                    # transfer chains run in parallel from t=0
                    s0_wts = []
                    for (k0, k1) in [(0, 1), (1, 2), (2, 8), (8, 16)]:
                        wt = w_pool.tile([128, (k1 - k0) * I], DTS,
                                         tag="wsl", name="wt")
                        ldma(wt[:], wgu.ap()[0, 0][:, k0 * I:k1 * I])
                        s0_wts.append(wt)
                    odma(xs[:, :R], xg.ap()[:, xbase:xbase + R])
                    odma(xs[:, R:2 * R], xg.ap()[:, xbase + R:xbase + 2 * R])
                    odma(xs[:, 2 * R:], xg.ap()[:, xbase + 2 * R:
                                                xbase + KT_H * R])
                elif s == 0 and variant == 3 and not f32r:
                    # ramp: interleave phase-0 weight groups with the xs
                    # k-ranges they need
                    s0_wts = []
                    for (k0, k1) in [(0, 1), (1, 2), (2, 8), (8, 16)]:
                        wt = w_pool.tile([128, (k1 - k0) * I], DTS,
                                         tag="wsl", name="wt")
                        ldma(wt[:], wgu.ap()[0, 0][:, k0 * I:k1 * I])
                        s0_wts.append(wt)
                        sdma(xs[:, k0 * R:k1 * R],
                             xg.ap()[:, xbase + k0 * R:xbase + k1 * R])
                elif s == 0:
                    if not f32r:
                        # first weight group is the very first DMA on the
                        # sync queue so the PE can start early
                        wt0 = w_pool.tile([128, I], DTS, tag="wsl",
                                          name="wt")
                        ldma(wt0[:], wgu.ap()[0, 0][:, 0:I])
                    # split the first x load so the PE can start early
                    sdma(xs[:, :R], xg.ap()[:, xbase:xbase + R])
                    sdma(xs[:, R:2 * R], xg.ap()[:, xbase + R:xbase + 2 * R])
                    sdma(xs[:, 2 * R:], xg.ap()[:, xbase + 2 * R:
                                                xbase + KT_H * R])
                    if gather:
                        # piggyback the gather inputs on the sync queue
                        # right behind slot-0's x so they land before
                        # slot 1 needs them
                        sdma(xidx_sb[:], xidx.ap())
                        for ch4 in range(4):
                            sdma(xf_full[:, ch4 * 4096:(ch4 + 1) * 4096],
                                 xf.ap()[:, ch4 * 4096:(ch4 + 1) * 4096])
                elif gather:
                    # gather xs out of the resident xf, 4 k-tiles per call
                    # (ISA caps the indirect-copy dst element count); the
                    # relative index set is shared, the k-offset moves the
                    # data base
                    goff = sum(r // 4 for r in slot_rows[1:s])
                    ncol = R // 4
                    for kb4 in range(4):
                        nc.gpsimd.indirect_copy(
                            xs[:, kb4 * 4 * R:(kb4 + 1) * 4 * R],
                            xf_full[:, kb4 * 4 * CH:],
                            xidx_sb[:, goff:goff + ncol], True)
                else:
                    sdma(xs[:], xg.ap()[:, xbase:xbase + KT_H * R])

                gs = g_pool.tile([128, KT_I * R], DTS if gather else F32,
                                 tag="gs", name="gs")
                at = a_pool.tile([128, KT_I * R], DTS, tag="at", name="at")

                for phase in range(2):  # 0 = gate, 1 = up
                    def gu_drain(m, ps_m):
                        if phase == 0:
                            nc.scalar.activation(
                                gs[:, m * R:(m + 1) * R], ps_m[:],
                                mybir.ActivationFunctionType.Sigmoid)
                            nc.vector.tensor_mul(
                                gs[:, m * R:(m + 1) * R],
                                gs[:, m * R:(m + 1) * R], ps_m[:])
                        else:
                            nc.vector.tensor_mul(
                                at[:, m * R:(m + 1) * R],
                                gs[:, m * R:(m + 1) * R], ps_m[:])

                    if s == 0 and phase == 0:
                        groups = [(0, 1), (1, 2), (2, 8), (8, 16)]
                    else:
                        groups = [(kb * WLOAD_K, (kb + 1) * WLOAD_K)
                                  for kb in range(KT_H // WLOAD_K)]
                    if s == 0 and phase == 0 and s0_wts is not None:
                        wts = s0_wts
                    else:
                        wts = []
                        for (k0, k1) in groups:
                            if s == 0 and phase == 0 and k0 == 0 and wt0:
                                wts.append(wt0)
                                continue
                            wt = w_pool.tile([128, (k1 - k0) * I], DTS,
                                             tag="wsl", name="wt")
                            ldma(wt[:], wgu.ap()[s, phase][:, k0 * I:k1 * I])
                            wts.append(wt)
                    if s == 0 and phase == 0 and variant == 9:  # not 10
                        # nothing else competes for PSUM this early, so
                        # run a single 8-bank wave (5 expert + 3 shared
                        # banks): twice the PE work per arrived k-tile,
                        # halving the DMA-latency-bound ramp idle
                        ps8 = ([pe_pool.tile([128, R], F32, tag="ps",
                                             name="ps") for _ in range(5)] +
                               [psh_pool.tile([128, R], F32, tag="ps",
                                              name="ps") for _ in range(3)])
                        for gi, (k0, k1) in enumerate(groups):
                            wt = wts[gi]
                            for kk in range(k1 - k0):
                                k = k0 + kk
                                rx = xs[:, k * R:(k + 1) * R]
                                for m in range(8):
                                    lw = wt[:, kk * I + m * 128:
                                            kk * I + (m + 1) * 128]
                                    nc.tensor.matmul(
                                        ps8[m][:], lw, rx,
                                        start=(k == 0), stop=(k == KT_H - 1))
                        for m in range(8):
                            gu_drain(m, ps8[m])
                        continue
                    # two 4-m-tile waves: expert slots hold at most 4+2
                    # PSUM banks so shared chunks can interleave
                    for wave in range(2):
                        ps = [pe_pool.tile([128, R], F32, tag="ps",
                                           name="ps") for _ in range(4)]
                        for gi, (k0, k1) in enumerate(groups):
                            wt = wts[gi]
                            for kk in range(k1 - k0):
                                k = k0 + kk
                                rx = xs[:, k * R:(k + 1) * R]
                                for mi in range(4):
                                    m = wave * 4 + mi
                                    lw = wt[:, kk * I + m * 128:
                                            kk * I + (m + 1) * 128]
                                    nc.tensor.matmul(
                                        ps[mi][:], lw, rx,
                                        start=(k == 0), stop=(k == KT_H - 1))
                        for mi in range(4):
                            gu_drain(wave * 4 + mi, ps[mi])

                WLD = min(WLOAD_K, KT_I)
                for half in range(2):
                    wts = []
                    for kb in range(KT_I // WLD):
                        wt = dw_pool.tile([128, WLD * I], DTS, tag="wsl",
                                          name="wt")
                        ldma(wt[:], wd.ap()[s, half][:, kb * WLD * I:
                                                     (kb + 1) * WLD * I])
                        wts.append(wt)
                    stg = o_pool.tile([128, 8 * R], DTO, tag="ost", name="stg")
                    boff = 16 * off + half * 8 * R
                    for wave in range(2):
                        ps = [pe_pool.tile([128, R], F32, tag="ps",
                                           name="ps") for _ in range(4)]
                        for kb in range(KT_I // WLD):
                            wt = wts[kb]
                            for kk in range(WLD):
                                k = kb * WLD + kk
                                ra = at[:, k * R:(k + 1) * R]
                                for mi in range(4):
                                    m = wave * 4 + mi
                                    lw = wt[:, kk * I + m * 128:
                                            kk * I + (m + 1) * 128]
                                    nc.tensor.matmul(
                                        ps[mi][:], lw, ra,
                                        start=(k == 0), stop=(k == KT_I - 1))
                        for mi in range(4):
                            m = wave * 4 + mi
                            nc.scalar.copy(stg[:, m * R:(m + 1) * R],
                                           ps[mi][:])
                        if split_store:
                            w0 = wave * 4 * R
                            odma(yr.ap()[:, boff + w0:boff + w0 + 4 * R],
                                 stg[:, w0:w0 + 4 * R])
                    if not split_store:
                        odma(yr.ap()[:, boff:boff + 8 * R], stg[:])

            # experts carry the bulk of the DMA stream; shared-expert
            # chunks are interleaved to fill PE gaps at phase boundaries
            if variant == 8:
                expert_slot(0)
                shared_weights()
                shared_chunk(0)
                for s_i in range(1, n_slots):
                    expert_slot(s_i)
                    shared_chunk(s_i)
            else:
                expert_slot(0)
                shared_weights()
                shared_chunk(0)
                shared_chunk(1)
                for s_i in range(1, n_slots):
                    if s_i == 1:
                        expert_slot(1)
                        shared_chunk(2)
                    elif s_i == 2:
                        expert_slot(2)
                        shared_chunk(3)
                    else:
                        expert_slot(s_i)

    _fuse_ldweights(nc)
    nc.compile()
    return nc


def _get_program(slot_rows, mode):
    variant = _variant()
    key = (tuple(slot_rows), mode, variant)
    if key not in _PROGRAM_CACHE:
        _PROGRAM_CACHE[key] = _build_program(slot_rows, mode, variant)
    return _PROGRAM_CACHE[key]


# ---------------------------------------------------------------------------
# Per-core input construction (host shard + reorder + cast)
# ---------------------------------------------------------------------------

def _pm(a):
    """[KT, 128, M] -> partition-major [128, KT*M]."""
    kt, p, m = a.shape
    return np.ascontiguousarray(a.transpose(1, 0, 2)).reshape(p, kt * m)


def _make_in_maps(x, w_gate_up, w_down, shared_gate_up, shared_down,
                  topk_ids, plan, mode):
    rtotal = plan["rtotal"]
    slot_rows = plan["slot_rows"]
    offs = plan["slot_offs"]
    expert_of = plan["expert_of"]
    np_dt = np.float32 if mode == "f32r" else ml_dtypes.bfloat16

    slot_piece = plan.get("slot_piece") or [(0, r) for r in slot_rows]
    n_slots = len(slot_rows)
    tok_of = [np.where((topk_ids == e).any(axis=1))[0] for e in range(E)]
    flat_col = np.zeros((T, TOPK), dtype=np.int64)

    xT = np.ascontiguousarray(x.T).astype(np_dt)          # [H, T]
    # weights -> [E, 2, 128, KT*I] partition-major k-slab layout
    wgu_pm = np.ascontiguousarray(
        w_gate_up.astype(np_dt).reshape(E, KT_H, 128, 2, I)
        .transpose(0, 3, 2, 1, 4)).reshape(E, 2, 128, KT_H * I)
    wd_pm = np.ascontiguousarray(
        w_down.astype(np_dt).reshape(E, KT_I, 128, 2, I)
        .transpose(0, 3, 2, 1, 4)).reshape(E, 2, 128, KT_I * I)

    xf_pm = np.concatenate(
        [_pm(xT[:, ch * CH:(ch + 1) * CH].reshape(KT_H, 128, CH))
         for ch in range(T // CH)], axis=1)

    gather = _variant() == 7 and mode != "f32r"
    rgath = sum(r // 4 for r in slot_rows[1:])

    in_maps = []
    for c in range(N_CORES):
        xg_pm = np.zeros(
            (128, KT_H * (slot_rows[0] if gather else rtotal)), dtype=np_dt)
        xidx = np.zeros((128, max(rgath, 1)), dtype=np.uint16)
        for s in range(n_slots):
            e = expert_of[c, s]
            pj, pcap = slot_piece[s]
            toks = tok_of[e][pj * pcap:(pj + 1) * pcap]
            R = slot_rows[s]
            if gather and s > 0:
                # wrapped gather indices for one 4-k-tile batch: output
                # col i (= kt_local*R + j) has its index at
                # [i % 16, goff + i // 16], replicated over the eight
                # 16-partition groups; later batches reuse the set with
                # a shifted data base
                goff = sum(r // 4 for r in slot_rows[1:s])
                tpad = np.zeros(R, dtype=np.int64)
                tpad[:len(toks)] = toks
                base = (tpad // CH) * (KT_H * CH) + (tpad % CH)
                colidx = np.concatenate(
                    [base + kt * CH for kt in range(4)]).astype(np.uint16)
                wrapped = colidx.reshape(R // 4, 16).T   # [16, R/4]
                xidx[:, goff:goff + R // 4] = np.tile(wrapped, (8, 1))
            else:
                blk = np.zeros((H, R), dtype=np_dt)
                blk[:, :len(toks)] = xT[:, toks]
                xg_pm[:, KT_H * offs[s]:KT_H * (offs[s] + R)] = _pm(
                    blk.reshape(KT_H, 128, R))
            col_base = c * rtotal + offs[s]
            for pos, t in enumerate(toks):
                for k in np.nonzero(topk_ids[t] == e)[0]:
                    flat_col[t, k] = col_base + pos
        sl = slice(c * 128, (c + 1) * 128)
        sgu_slice = np.concatenate(
            [shared_gate_up[:, sl],
             shared_gate_up[:, 1024 + c * 128:1024 + (c + 1) * 128]],
            axis=1).astype(np_dt)
        im = {
            "xg": xg_pm,
            "xf": xf_pm,
            "wgu": np.ascontiguousarray(wgu_pm[expert_of[c]]),
            "wd": np.ascontiguousarray(wd_pm[expert_of[c]]),
            "wsg": _pm(sgu_slice.reshape(KT_H, 128, 256)),
            "wsd": np.ascontiguousarray(shared_down[sl, :]).astype(np_dt),
        }
        if gather:
            im["xidx"] = xidx
        in_maps.append(im)
    return in_maps, flat_col


# ---------------------------------------------------------------------------
# Entry point
# ---------------------------------------------------------------------------

def kernel(hidden_states, gate_w, e_bias, w_gate_up, w_down,
           shared_gate_up, shared_down):
    global LAST_RESULTS
    mode = _mode()
    x = np.ascontiguousarray(np.asarray(hidden_states, dtype=np.float32))
    gate_w = np.asarray(gate_w, dtype=np.float32)
    e_bias = np.asarray(e_bias, dtype=np.float32)
    w_gate_up = np.asarray(w_gate_up, dtype=np.float32)
    w_down = np.asarray(w_down, dtype=np.float32)
    shared_gate_up = np.asarray(shared_gate_up, dtype=np.float32)
    shared_down = np.asarray(shared_down, dtype=np.float32)

    topk_w, topk_ids = _host_routing(x, gate_w, e_bias)
    plan = _make_plan(topk_ids)
    slot_rows = plan["slot_rows"]
    offs = plan["slot_offs"]
    rtotal = plan["rtotal"]

    nc = _get_program(slot_rows, mode)
    in_maps, flat_col = _make_in_maps(
        x, w_gate_up, w_down, shared_gate_up, shared_down,
        topk_ids, plan, mode)

    trace = bool(int(os.environ.get("KERNEL_TRACE", "0")))
    res = run_bass_kernel_spmd(
        nc, in_maps, list(range(N_CORES)), trace=trace,
        tmpdir=os.environ.get("KERNEL_TRACE_DIR") or None)
    LAST_RESULTS = res

    # decode partition-major outputs back to [H, rtotal] per core
    def decode_yr(arr):
        out = np.empty((H, rtotal), dtype=np.float32)
        for s in range(len(slot_rows)):
            R = slot_rows[s]
            for half in range(2):
                boff = 16 * offs[s] + half * 8 * R
                blk = np.asarray(arr[:, boff:boff + 8 * R], dtype=np.float32)
                out[half * 1024:(half + 1) * 1024, offs[s]:offs[s] + R] = (
                    blk.reshape(128, 8, R).transpose(1, 0, 2).reshape(1024, R))
        return out

    Y = np.concatenate(
        [decode_yr(res.results[c]["yr"]).T for c in range(N_CORES)], axis=0)
    w_flat = (topk_w * SCALE).astype(np.float32).reshape(-1)
    out = (Y[flat_col.reshape(-1)] * w_flat[:, None]).reshape(T, TOPK, H).sum(1)

    ys_sum = np.zeros((128, 16 * T), dtype=np.float32)
    for c in range(N_CORES):
        ys_sum += np.asarray(res.results[c]["ys"], dtype=np.float32)
    shared = np.empty((H, T), dtype=np.float32)
    for ch in range(T // CH):
        for half in range(2):
            boff = (ch * 2 + half) * 8 * CH
            blk = ys_sum[:, boff:boff + 8 * CH]
            shared[half * 1024:(half + 1) * 1024, ch * CH:(ch + 1) * CH] = (
                blk.reshape(128, 8, CH).transpose(1, 0, 2).reshape(1024, CH))
    out += shared.T
    return out.astype(np.float32)



# revision 56
# speedup vs baseline: 1.1348x; 1.0289x over previous
"""MegrezMoE MoE layer on 8 Trainium2 NeuronCores.

Strategy (expert-parallel, host-routed dispatch):
  - Host computes the (tiny) router: logits -> grouped top-k ids/weights,
    exactly mirroring the reference's noaux_tc selection.
  - 32 experts are assigned 4-per-core, balanced by routed-token count.
    Tokens are gathered per expert on the host (transposed: [H, rows],
    rows padded to a per-slot static capacity) so the device kernel is a
    fully static SPMD program: per expert slot, gate_up matmul ->
    silu*mul -> down matmul, streaming the expert weight bank from HBM
    exactly once per core.
  - The shared-expert MLP is tensor-parallel across the 8 cores (each
    core owns a 128-wide slice of the shared intermediate dim) and its
    partial outputs are summed on the host.
  - All device tensors use a partition-major layout ([128, ...] with
    k-tiles concatenated along the free dim) so every DMA is a plain 2D
    contiguous transfer with minimal descriptor overhead.
  - Matmuls run in bf16 with fp32 PSUM accumulation (KERNEL_DTYPE=f32r
    selects a float32r variant that keeps fp32 data in HBM).
  - Host combines: out[t] = sum_k w[t,k]*SCALE * y_col(t,k) + shared[t].

Scheduling notes (measured on HW, each worth 5-10%):
  - _fuse_ldweights() folds the standalone InstLdweights emitted by tile
    legalization into self-loading matmuls; the compiler's automatic
    fast-weight-load then halves the PE weight-load cost (~254us ->
    ~222us).
  - Expert slots accumulate in two 4-bank PSUM waves (pool psum_e) so
    the shared-expert chunks (pool psum_s) interleave with the expert
    stream instead of serializing at the end.
  - Down-projection weights use their own 2-deep pool so the next
    slot's gate_up weight DMA is not WAR-blocked behind the up phase.
  - Output stores ride the Activation HWDGE queue; all loads ride the
    Sync HWDGE queue, whose triggers run ahead of compute.

kernel() takes the full unsharded inputs, returns the full [1024, 2048]
fp32 output.
"""

import os

import ml_dtypes
import numpy as np

import concourse.bass as bass
import concourse.tile as tile
from concourse import bacc, mybir
from concourse.bass_utils import run_bass_kernel_spmd

# Model dims (hardcoded per problem spec)
H = 2048
E = 32
I = 1024
TOPK = 6
NGROUP = 8
TOPKG = 4
SCALE = 2.5
T = 1024

N_CORES = 8
EPC = 4          # experts per core
KT_H = H // 128  # 16 k-tiles over hidden dim
KT_I = I // 128  # 8 k-tiles over intermediate dim
WLOAD_K = 8      # k-tiles per weight DMA
CH = 256         # shared-expert token chunk

F32 = mybir.dt.float32
F32R = mybir.dt.float32r
BF16 = mybir.dt.bfloat16

_PROGRAM_CACHE = {}
LAST_RESULTS = None  # BassKernelResults from the most recent run (for harness)


def _mode():
    return os.environ.get("KERNEL_DTYPE", "bf16")


def _variant():
    """Scheduling-variant knob for A/B benchmarking (0 = default)."""
    return int(os.environ.get("KERNEL_VARIANT", "0"))


# ---------------------------------------------------------------------------
# Host-side routing (mirrors reference._grouped_topk in fp32 numpy)
# ---------------------------------------------------------------------------

def _host_routing(x, gate_w, e_bias):
    logits = x @ gate_w                                   # [T, E] fp32
    scores = 1.0 / (1.0 + np.exp(-logits, dtype=np.float32))
    scores_choice = scores + e_bias[None, :]
    gsize = E // NGROUP
    grp = scores_choice.reshape(T, NGROUP, gsize)
    top2 = np.sort(grp, axis=-1)[:, :, -2:]
    group_scores = top2.sum(-1)                           # [T, G]
    gidx = np.argsort(-group_scores, axis=-1, kind="stable")[:, :TOPKG]
    gmask = np.zeros((T, NGROUP), bool)
    np.put_along_axis(gmask, gidx, True, axis=1)
    emask = np.repeat(gmask, gsize, axis=1)
    masked = np.where(emask, scores_choice, -np.inf)
    topk_ids = np.argsort(-masked, axis=-1, kind="stable")[:, :TOPK]
    topk_w = np.take_along_axis(scores, topk_ids, axis=1)
    topk_w = topk_w / topk_w.sum(-1, keepdims=True)
    return topk_w.astype(np.float32), topk_ids.astype(np.int64)


# ---------------------------------------------------------------------------
# Dispatch plan: expert -> (core, slot), per-slot static row capacities
# ---------------------------------------------------------------------------

def _make_plan(topk_ids):
    counts = np.bincount(topk_ids.ravel(), minlength=E)
    # slot capacity = max routed count in the slot's expert group, rounded
    # up to 8 (DMA alignment)
    padded = np.maximum(16, ((counts + 7) // 8) * 8)
    order = np.argsort(-padded, kind="stable")            # experts, big first
    expert_of = []      # per slot: experts per core
    slot_rows = []
    slot_piece = []     # (piece index, piece capacity) per slot
    for s in range(EPC):
        chunk = order[s * N_CORES:(s + 1) * N_CORES]
        cap = int(padded[chunk].max())
        if cap <= 512:
            expert_of.append(list(chunk))
            slot_rows.append(cap)
            slot_piece.append((0, cap))
        else:
            # an expert group too wide for one PSUM bank: split into
            # pieces of <=512 rows (same expert, partitioned token list)
            n_p = -(-cap // 512)
            pcap = ((-(-cap // n_p) + 7) // 8) * 8
            for j in range(n_p):
                expert_of.append(list(chunk))
                slot_rows.append(pcap)
                slot_piece.append((j, pcap))
    expert_of = np.asarray(expert_of).T                   # [N_CORES, n_slots]
    offs = np.concatenate([[0], np.cumsum(slot_rows)])
    return {
        "expert_of": expert_of,
        "slot_rows": tuple(slot_rows),
        "slot_piece": slot_piece,
        "slot_offs": offs[:-1],
        "rtotal": int(offs[-1]),
        "counts": counts,
    }


# ---------------------------------------------------------------------------
# Bass program (SPMD; one program, per-core data)
# ---------------------------------------------------------------------------

def _fuse_ldweights(nc):
    """Fold each standalone InstLdweights into the following InstMatmult
    (ldweights=True, self-loading form). Tile legalization always splits
    matmuls into LDW+MM pairs; with the split form the PE pays ~102ns of
    unoverlapped LDWEIGHTS per matmul, while self-loading matmuls get
    the compiler's automatic fast-weight-load (~2x weight path). LDW
    waits move onto the matmul (or an event-semaphore right before it
    when the matmul already waits on a different semaphore)."""
    PE = mybir.EngineType.PE
    n_fused = 0
    for fn in nc.m.functions:
        for blk in fn.blocks:
            pending = None
            out = []
            changed = False
            for inst in blk.instructions:
                if isinstance(inst, mybir.InstLdweights) and inst.engine == PE:
                    assert pending is None, "LDW with no consuming matmul"
                    pending = inst
                    changed = True
                    continue
                if isinstance(inst, mybir.InstMatmult) and inst.engine == PE:
                    assert pending is not None, "matmul without its LDW"
                    ldw, pending = pending, None
                    wap, lap = inst.ins[1], ldw.ins[0]
                    assert (wap.memref, wap.offset, str(wap.ap)) == \
                           (lap.memref, lap.offset, str(lap.ap))
                    inst.ldweights = True
                    lsync = ldw.sync_info
                    lw = list(lsync.on_wait) if lsync else []
                    assert not (lsync and lsync.on_update)
                    if lw:
                        msync = inst.sync_info
                        mw = list(msync.on_wait) if msync else []
                        mu = list(msync.on_update) if msync else []
                        extra = []
                        for w in lw:
                            dup = next((x for x in mw if x.id == w.id and
                                        x.wait_mode == w.wait_mode ==
                                        "sem-ge-imm"), None)
                            if dup is not None:
                                if w.wait_value > dup.wait_value:
                                    mw[mw.index(dup)] = w
                            elif not mw:
                                mw.append(w)
                            else:
                                extra.append(w)
                        if extra:
                            ev = mybir.InstEventSemaphore(
                                name=nc.get_next_instruction_name(),
                                ins=[], outs=[])
                            ev.engine = PE
                            ev.sync_info = mybir.SyncInfo(
                                on_wait=extra, on_update=[])
                            nc.register_instruction(ev)
                            out.append(ev)
                        inst.sync_info = mybir.SyncInfo(
                            on_wait=mw, on_update=mu)
                    n_fused += 1
                out.append(inst)
            assert pending is None
            if changed:
                blk.instructions = out
    return n_fused


def _build_program(slot_rows, mode, variant=0):
    rtotal = sum(slot_rows)
    f32r = mode == "f32r"
    DTD = F32 if f32r else BF16      # dram dtype of matmul operands
    DTS = F32R if f32r else BF16     # sbuf dtype of matmul operands
    DTO = F32 if f32r else BF16      # output dtype

    nc = bacc.Bacc("TRN2", target_bir_lowering=False, debug=False,
                   num_devices=N_CORES)

    # DRAM I/O, all partition-major ([128 partitions, free]):
    #   xg : slot-blocked gathered tokens; slot s at cols KT_H*off_s,
    #        k-tile k of slot s at [KT_H*off_s + k*R_s, +R_s]
    #   xf : chunk-blocked all tokens (for the shared expert)
    #   wgu/wd : per (slot, half): k-tiles concatenated along free dim
    #   yr/ys : per (slot/chunk, half): 8 output m-tiles concatenated
    n_slots = len(slot_rows)
    gather = variant == 7 and not f32r
    # with on-chip gather only slot 0 ships host-gathered tokens; slots
    # 1+ are gathered out of the (SBUF-resident) xf by gpsimd
    xg_cols = KT_H * (slot_rows[0] if gather else rtotal)
    rgath = sum(r // 4 for r in slot_rows[1:])
    xg = nc.dram_tensor("xg", [128, xg_cols], DTD, kind="ExternalInput")
    xf = nc.dram_tensor("xf", [128, KT_H * T], DTD, kind="ExternalInput")
    if gather:
        xidx = nc.dram_tensor("xidx", [128, rgath], mybir.dt.uint16,
                              kind="ExternalInput")
    wgu = nc.dram_tensor("wgu", [n_slots, 2, 128, KT_H * I], DTD,
                         kind="ExternalInput")
    wd = nc.dram_tensor("wd", [n_slots, 2, 128, KT_I * I], DTD,
                        kind="ExternalInput")
    wsg = nc.dram_tensor("wsg", [128, KT_H * 256], DTD, kind="ExternalInput")
    wsd = nc.dram_tensor("wsd", [128, H], DTD, kind="ExternalInput")
    yr = nc.dram_tensor("yr", [128, 16 * rtotal], DTO, kind="ExternalOutput")
    ys = nc.dram_tensor("ys", [128, 16 * T], DTO, kind="ExternalOutput")

    # casting DMA (fp32 dram -> f32r sbuf) must go via SWDGE
    ldma = nc.gpsimd.dma_start if f32r else nc.sync.dma_start
    xdma = ldma

    slot_offs = [0]
    for R in slot_rows[:-1]:
        slot_offs.append(slot_offs[-1] + R)

    # Input loads (weights + x) all ride the Sync HWDGE queue — triggers
    # run well ahead of compute there. Output stores go on the Activation
    # HWDGE queue: their deps (the drain copies) are scalar/vector-local,
    # so a store trigger never blocks that queue's forward progress.
    sdma = ldma
    odma = nc.gpsimd.dma_start if f32r else nc.scalar.dma_start

    # default: 4-k-tile weight DMA groups with a 6-deep pool. Finer
    # arrival granularity engages more DMA rings per phase and the
    # deeper prefetch rides out device throttle phases (median 210us vs
    # 217us for 8-k-tile groups / 4 bufs, at ~3us cost in the best case)
    n_wbufs = 5 if variant == 1 else (4 if variant == 13 else 6)
    n_xbufs = 4 if variant == 5 else 3
    split_store = variant == 6
    n_pe_bufs, n_psh_bufs = (6, 2) if variant == 11 else (5, 3)
    wk = WLOAD_K if variant == 13 else 4

    with tile.TileContext(nc) as tc:
        with tc.tile_pool(name="psum_e", bufs=n_pe_bufs,
                          space="PSUM") as pe_pool, \
             tc.tile_pool(name="psum_s", bufs=n_psh_bufs,
                          space="PSUM") as psh_pool, \
             tc.tile_pool(name="swg", bufs=1) as swg_pool, \
             tc.tile_pool(name="swd", bufs=1) as swd_pool, \
             tc.tile_pool(name="sxf", bufs=1 if gather else 3) as sxf_pool, \
             tc.tile_pool(name="xidxp", bufs=1) as xidx_pool, \
             tc.tile_pool(name="sact", bufs=2) as sact_pool, \
             tc.tile_pool(name="sout", bufs=2) as sout_pool, \
             tc.tile_pool(name="wsl", bufs=n_wbufs) as w_pool, \
             tc.tile_pool(name="wdl", bufs=2) as wd_pool_, \
             tc.tile_pool(name="xs", bufs=n_xbufs) as x_pool, \
             tc.tile_pool(name="gs", bufs=2) as g_pool, \
             tc.tile_pool(name="at", bufs=2) as a_pool, \
             tc.tile_pool(name="ost", bufs=2) as o_pool:
            # down-projection weights get their own 2-deep pool so the
            # next slot's gate_up weight DMA is not WAR-blocked behind
            # the up phase (measured −16us vs a single 4-deep pool)
            dw_pool = w_pool if variant == 4 else wd_pool_

            wsg_sb = swg_pool.tile([128, KT_H * 256], DTS)
            wsd_sb = swd_pool.tile([128, H], DTS)
            if gather:
                xf_full = sxf_pool.tile([128, KT_H * T], DTS, name="xf_full")
                xidx_sb = xidx_pool.tile([128, rgath], mybir.dt.uint16,
                                         name="xidx_sb")

            def shared_weights():
                sdma(wsg_sb[:], wsg.ap())
                sdma(wsd_sb[:], wsd.ap())

            def shared_chunk(ch):
                if gather:
                    xf_t, xf_b = xf_full, ch * KT_H * CH
                else:
                    xf_t = sxf_pool.tile([128, KT_H * CH], DTS, name="xf_sb")
                    sdma(xf_t[:],
                         xf.ap()[:, ch * KT_H * CH:(ch + 1) * KT_H * CH])
                    xf_b = 0
                ps_g = psh_pool.tile([128, CH], F32, tag="ps", name="ps_g")
                ps_u = psh_pool.tile([128, CH], F32, tag="ps", name="ps_u")
                for k in range(KT_H):
                    lg = wsg_sb[:, k * 256:k * 256 + 128]
                    lu = wsg_sb[:, k * 256 + 128:k * 256 + 256]
                    rx = xf_t[:, xf_b + k * CH:xf_b + (k + 1) * CH]
                    nc.tensor.matmul(ps_g[:], lg, rx,
                                     start=(k == 0), stop=(k == KT_H - 1))
                    nc.tensor.matmul(ps_u[:], lu, rx,
                                     start=(k == 0), stop=(k == KT_H - 1))
                gss = sact_pool.tile([128, CH], F32, tag="sgs", name="gss")
                nc.scalar.activation(gss[:], ps_g[:],
                                     mybir.ActivationFunctionType.Sigmoid)
                nc.vector.tensor_mul(gss[:], gss[:], ps_g[:])
                a_s = sact_pool.tile([128, CH], DTS, tag="sas", name="a_s")
                nc.vector.tensor_mul(a_s[:], gss[:], ps_u[:])
                # down: 16 output m-tiles, single k (the 128-slice of I);
                # vector does the PSUM drain copies (scalar is busy with
                # the expert-slot drains and DMA triggers)
                for half in range(2):
                    stg = sout_pool.tile([128, 8 * CH], DTO, tag="sstg",
                                         name="stg")
                    for m in range(8):
                        pd = psh_pool.tile([128, CH], F32, tag="ps",
                                           name="pd")
                        lw = wsd_sb[:, (half * 8 + m) * 128:
                                    (half * 8 + m + 1) * 128]
                        nc.tensor.matmul(pd[:], lw, a_s[:],
                                         start=True, stop=True)
                        nc.vector.tensor_copy(stg[:, m * CH:(m + 1) * CH],
                                              pd[:])
                    odma(ys.ap()[:, (ch * 2 + half) * 8 * CH:
                                 (ch * 2 + half + 1) * 8 * CH],
                         stg[:])

            def expert_slot(s):
                R = slot_rows[s]
                off = slot_offs[s]
                xbase = KT_H * off
                wt0 = None
                s0_wts = None
                xs = x_pool.tile([128, KT_H * R], DTS, tag="xs", name="xs")
                if s == 0 and variant in (9, 10) and not f32r:
                    # ramp: weights stream on the sync queue, x pieces on
                    # the scalar queue in k-dependency order, so both
                    # transfer chains run in parallel from t=0
                    s0_wts = []
                    for (k0, k1) in [(0, 1), (1, 2), (2, 8), (8, 16)]:
                        wt = w_pool.tile([128, (k1 - k0) * I], DTS,
                                         tag="wsl", name="wt")
                        ldma(wt[:], wgu.ap()[0, 0][:, k0 * I:k1 * I])
                        s0_wts.append(wt)
                    odma(xs[:, :R], xg.ap()[:, xbase:xbase + R])
                    odma(xs[:, R:2 * R], xg.ap()[:, xbase + R:xbase + 2 * R])
                    odma(xs[:, 2 * R:], xg.ap()[:, xbase + 2 * R:
                                                xbase + KT_H * R])
                elif s == 0 and variant == 3 and not f32r:
                    # ramp: interleave phase-0 weight groups with the xs
                    # k-ranges they need
                    s0_wts = []
                    for (k0, k1) in [(0, 1), (1, 2), (2, 8), (8, 16)]:
                        wt = w_pool.tile([128, (k1 - k0) * I], DTS,
                                         tag="wsl", name="wt")
                        ldma(wt[:], wgu.ap()[0, 0][:, k0 * I:k1 * I])
                        s0_wts.append(wt)
                        sdma(xs[:, k0 * R:k1 * R],
                             xg.ap()[:, xbase + k0 * R:xbase + k1 * R])
                elif s == 0:
                    if not f32r:
                        # first weight group is the very first DMA on the
                        # sync queue so the PE can start early
                        wt0 = w_pool.tile([128, I], DTS, tag="wsl",
                                          name="wt")
                        ldma(wt0[:], wgu.ap()[0, 0][:, 0:I])
                    # split the first x load so the PE can start early
                    sdma(xs[:, :R], xg.ap()[:, xbase:xbase + R])
                    sdma(xs[:, R:2 * R], xg.ap()[:, xbase + R:xbase + 2 * R])
                    sdma(xs[:, 2 * R:], xg.ap()[:, xbase + 2 * R:
                                                xbase + KT_H * R])
                    if gather:
                        # piggyback the gather inputs on the sync queue
                        # right behind slot-0's x so they land before
                        # slot 1 needs them
                        sdma(xidx_sb[:], xidx.ap())
                        for ch4 in range(4):
                            sdma(xf_full[:, ch4 * 4096:(ch4 + 1) * 4096],
                                 xf.ap()[:, ch4 * 4096:(ch4 + 1) * 4096])
                elif gather:
                    # gather xs out of the resident xf, 4 k-tiles per call
                    # (ISA caps the indirect-copy dst element count); the
                    # relative index set is shared, the k-offset moves the
                    # data base
                    goff = sum(r // 4 for r in slot_rows[1:s])
                    ncol = R // 4
                    for kb4 in range(4):
                        nc.gpsimd.indirect_copy(
                            xs[:, kb4 * 4 * R:(kb4 + 1) * 4 * R],
                            xf_full[:, kb4 * 4 * CH:],
                            xidx_sb[:, goff:goff + ncol], True)
                else:
                    sdma(xs[:], xg.ap()[:, xbase:xbase + KT_H * R])

                gs = g_pool.tile([128, KT_I * R], DTS if gather else F32,
                                 tag="gs", name="gs")
                at = a_pool.tile([128, KT_I * R], DTS, tag="at", name="at")

                for phase in range(2):  # 0 = gate, 1 = up
                    def gu_drain(m, ps_m):
                        if phase == 0:
                            nc.scalar.activation(
                                gs[:, m * R:(m + 1) * R], ps_m[:],
                                mybir.ActivationFunctionType.Sigmoid)
                            nc.vector.tensor_mul(
                                gs[:, m * R:(m + 1) * R],
                                gs[:, m * R:(m + 1) * R], ps_m[:])
                        else:
                            nc.vector.tensor_mul(
                                at[:, m * R:(m + 1) * R],
                                gs[:, m * R:(m + 1) * R], ps_m[:])

                    if s == 0 and phase == 0:
                        if variant == 13:
                            groups = [(0, 1), (1, 2), (2, 8), (8, 16)]
                        else:
                            groups = [(0, 1), (1, 2), (2, 4), (4, 8),
                                      (8, 12), (12, 16)]
                    else:
                        groups = [(kb * wk, (kb + 1) * wk)
                                  for kb in range(KT_H // wk)]
                    if s == 0 and phase == 0 and s0_wts is not None:
                        wts = s0_wts
                    else:
                        wts = []
                        for (k0, k1) in groups:
                            if s == 0 and phase == 0 and k0 == 0 and wt0:
                                wts.append(wt0)
                                continue
                            wt = w_pool.tile([128, (k1 - k0) * I], DTS,
                                             tag="wsl", name="wt")
                            ldma(wt[:], wgu.ap()[s, phase][:, k0 * I:k1 * I])
                            wts.append(wt)
                    if s == 0 and phase == 0 and variant == 9:  # not 10
                        # nothing else competes for PSUM this early, so
                        # run a single 8-bank wave (5 expert + 3 shared
                        # banks): twice the PE work per arrived k-tile,
                        # halving the DMA-latency-bound ramp idle
                        ps8 = ([pe_pool.tile([128, R], F32, tag="ps",
                                             name="ps") for _ in range(5)] +
                               [psh_pool.tile([128, R], F32, tag="ps",
                                              name="ps") for _ in range(3)])
                        for gi, (k0, k1) in enumerate(groups):
                            wt = wts[gi]
                            for kk in range(k1 - k0):
                                k = k0 + kk
                                rx = xs[:, k * R:(k + 1) * R]
                                for m in range(8):
                                    lw = wt[:, kk * I + m * 128:
                                            kk * I + (m + 1) * 128]
                                    nc.tensor.matmul(
                                        ps8[m][:], lw, rx,
                                        start=(k == 0), stop=(k == KT_H - 1))
                        for m in range(8):
                            gu_drain(m, ps8[m])
                        continue
                    # two 4-m-tile waves: expert slots hold at most 4+2
                    # PSUM banks so shared chunks can interleave
                    for wave in range(2):
                        ps = [pe_pool.tile([128, R], F32, tag="ps",
                                           name="ps") for _ in range(4)]
                        for gi, (k0, k1) in enumerate(groups):
                            wt = wts[gi]
                            for kk in range(k1 - k0):
                                k = k0 + kk
                                rx = xs[:, k * R:(k + 1) * R]
                                for mi in range(4):
                                    m = wave * 4 + mi
                                    lw = wt[:, kk * I + m * 128:
                                            kk * I + (m + 1) * 128]
                                    nc.tensor.matmul(
                                        ps[mi][:], lw, rx,
                                        start=(k == 0), stop=(k == KT_H - 1))
                        for mi in range(4):
                            gu_drain(wave * 4 + mi, ps[mi])

                WLD = min(WLOAD_K, KT_I)
                for half in range(2):
                    wts = []
                    for kb in range(KT_I // WLD):
                        wt = dw_pool.tile([128, WLD * I], DTS, tag="wsl",
                                          name="wt")
                        ldma(wt[:], wd.ap()[s, half][:, kb * WLD * I:
                                                     (kb + 1) * WLD * I])
                        wts.append(wt)
                    stg = o_pool.tile([128, 8 * R], DTO, tag="ost", name="stg")
                    boff = 16 * off + half * 8 * R
                    for wave in range(2):
                        ps = [pe_pool.tile([128, R], F32, tag="ps",
                                           name="ps") for _ in range(4)]
                        for kb in range(KT_I // WLD):
                            wt = wts[kb]
                            for kk in range(WLD):
                                k = kb * WLD + kk
                                ra = at[:, k * R:(k + 1) * R]
                                for mi in range(4):
                                    m = wave * 4 + mi
                                    lw = wt[:, kk * I + m * 128:
                                            kk * I + (m + 1) * 128]
                                    nc.tensor.matmul(
                                        ps[mi][:], lw, ra,
                                        start=(k == 0), stop=(k == KT_I - 1))
                        for mi in range(4):
                            m = wave * 4 + mi
                            nc.scalar.copy(stg[:, m * R:(m + 1) * R],
                                           ps[mi][:])
                        if split_store:
                            w0 = wave * 4 * R
                            odma(yr.ap()[:, boff + w0:boff + w0 + 4 * R],
                                 stg[:, w0:w0 + 4 * R])
                    if not split_store:
                        odma(yr.ap()[:, boff:boff + 8 * R], stg[:])

            # experts carry the bulk of the DMA stream; shared-expert
            # chunks are interleaved to fill PE gaps at phase boundaries
            if variant == 8:
                expert_slot(0)
                shared_weights()
                shared_chunk(0)
                for s_i in range(1, n_slots):
                    expert_slot(s_i)
                    shared_chunk(s_i)
            else:
                expert_slot(0)
                shared_weights()
                shared_chunk(0)
                shared_chunk(1)
                for s_i in range(1, n_slots):
                    if s_i == 1:
                        expert_slot(1)
                        shared_chunk(2)
                    elif s_i == 2:
                        expert_slot(2)
                        shared_chunk(3)
                    else:
                        expert_slot(s_i)

    _fuse_ldweights(nc)
    nc.compile()
    return nc


def _get_program(slot_rows, mode):
    variant = _variant()
    key = (tuple(slot_rows), mode, variant)
    if key not in _PROGRAM_CACHE:
        _PROGRAM_CACHE[key] = _build_program(slot_rows, mode, variant)
    return _PROGRAM_CACHE[key]


# ---------------------------------------------------------------------------
# Per-core input construction (host shard + reorder + cast)
# ---------------------------------------------------------------------------

def _pm(a):
    """[KT, 128, M] -> partition-major [128, KT*M]."""
    kt, p, m = a.shape
    return np.ascontiguousarray(a.transpose(1, 0, 2)).reshape(p, kt * m)


def _make_in_maps(x, w_gate_up, w_down, shared_gate_up, shared_down,
                  topk_ids, plan, mode):
    rtotal = plan["rtotal"]
    slot_rows = plan["slot_rows"]
    offs = plan["slot_offs"]
    expert_of = plan["expert_of"]
    np_dt = np.float32 if mode == "f32r" else ml_dtypes.bfloat16

    slot_piece = plan.get("slot_piece") or [(0, r) for r in slot_rows]
    n_slots = len(slot_rows)
    tok_of = [np.where((topk_ids == e).any(axis=1))[0] for e in range(E)]
    flat_col = np.zeros((T, TOPK), dtype=np.int64)

    xT = np.ascontiguousarray(x.T).astype(np_dt)          # [H, T]
    # weights -> [E, 2, 128, KT*I] partition-major k-slab layout
    wgu_pm = np.ascontiguousarray(
        w_gate_up.astype(np_dt).reshape(E, KT_H, 128, 2, I)
        .transpose(0, 3, 2, 1, 4)).reshape(E, 2, 128, KT_H * I)
    wd_pm = np.ascontiguousarray(
        w_down.astype(np_dt).reshape(E, KT_I, 128, 2, I)
        .transpose(0, 3, 2, 1, 4)).reshape(E, 2, 128, KT_I * I)

    xf_pm = np.concatenate(
        [_pm(xT[:, ch * CH:(ch + 1) * CH].reshape(KT_H, 128, CH))
         for ch in range(T // CH)], axis=1)

    gather = _variant() == 7 and mode != "f32r"
    rgath = sum(r // 4 for r in slot_rows[1:])

    in_maps = []
    for c in range(N_CORES):
        xg_pm = np.zeros(
            (128, KT_H * (slot_rows[0] if gather else rtotal)), dtype=np_dt)
        xidx = np.zeros((128, max(rgath, 1)), dtype=np.uint16)
        for s in range(n_slots):
            e = expert_of[c, s]
            pj, pcap = slot_piece[s]
            toks = tok_of[e][pj * pcap:(pj + 1) * pcap]
            R = slot_rows[s]
            if gather and s > 0:
                # wrapped gather indices for one 4-k-tile batch: output
                # col i (= kt_local*R + j) has its index at
                # [i % 16, goff + i // 16], replicated over the eight
                # 16-partition groups; later batches reuse the set with
                # a shifted data base
                goff = sum(r // 4 for r in slot_rows[1:s])
                tpad = np.zeros(R, dtype=np.int64)
                tpad[:len(toks)] = toks
                base = (tpad // CH) * (KT_H * CH) + (tpad % CH)
                colidx = np.concatenate(
                    [base + kt * CH for kt in range(4)]).astype(np.uint16)
                wrapped = colidx.reshape(R // 4, 16).T   # [16, R/4]
                xidx[:, goff:goff + R // 4] = np.tile(wrapped, (8, 1))
            else:
                blk = np.zeros((H, R), dtype=np_dt)
                blk[:, :len(toks)] = xT[:, toks]
                xg_pm[:, KT_H * offs[s]:KT_H * (offs[s] + R)] = _pm(
                    blk.reshape(KT_H, 128, R))
            col_base = c * rtotal + offs[s]
            for pos, t in enumerate(toks):
                for k in np.nonzero(topk_ids[t] == e)[0]:
                    flat_col[t, k] = col_base + pos
        sl = slice(c * 128, (c + 1) * 128)
        sgu_slice = np.concatenate(
            [shared_gate_up[:, sl],
             shared_gate_up[:, 1024 + c * 128:1024 + (c + 1) * 128]],
            axis=1).astype(np_dt)
        im = {
            "xg": xg_pm,
            "xf": xf_pm,
            "wgu": np.ascontiguousarray(wgu_pm[expert_of[c]]),
            "wd": np.ascontiguousarray(wd_pm[expert_of[c]]),
            "wsg": _pm(sgu_slice.reshape(KT_H, 128, 256)),
            "wsd": np.ascontiguousarray(shared_down[sl, :]).astype(np_dt),
        }
        if gather:
            im["xidx"] = xidx
        in_maps.append(im)
    return in_maps, flat_col


# ---------------------------------------------------------------------------
# Entry point
# ---------------------------------------------------------------------------

def kernel(hidden_states, gate_w, e_bias, w_gate_up, w_down,
           shared_gate_up, shared_down):
    global LAST_RESULTS
    mode = _mode()
    x = np.ascontiguousarray(np.asarray(hidden_states, dtype=np.float32))
    gate_w = np.asarray(gate_w, dtype=np.float32)
    e_bias = np.asarray(e_bias, dtype=np.float32)
    w_gate_up = np.asarray(w_gate_up, dtype=np.float32)
    w_down = np.asarray(w_down, dtype=np.float32)
    shared_gate_up = np.asarray(shared_gate_up, dtype=np.float32)
    shared_down = np.asarray(shared_down, dtype=np.float32)

    topk_w, topk_ids = _host_routing(x, gate_w, e_bias)
    plan = _make_plan(topk_ids)
    slot_rows = plan["slot_rows"]
    offs = plan["slot_offs"]
    rtotal = plan["rtotal"]

    nc = _get_program(slot_rows, mode)
    in_maps, flat_col = _make_in_maps(
        x, w_gate_up, w_down, shared_gate_up, shared_down,
        topk_ids, plan, mode)

    trace = bool(int(os.environ.get("KERNEL_TRACE", "0")))
    res = run_bass_kernel_spmd(
        nc, in_maps, list(range(N_CORES)), trace=trace,
        tmpdir=os.environ.get("KERNEL_TRACE_DIR") or None)
    LAST_RESULTS = res

    # decode partition-major outputs back to [H, rtotal] per core
    def decode_yr(arr):
        out = np.empty((H, rtotal), dtype=np.float32)
        for s in range(len(slot_rows)):
            R = slot_rows[s]
            for half in range(2):
                boff = 16 * offs[s] + half * 8 * R
                blk = np.asarray(arr[:, boff:boff + 8 * R], dtype=np.float32)
                out[half * 1024:(half + 1) * 1024, offs[s]:offs[s] + R] = (
                    blk.reshape(128, 8, R).transpose(1, 0, 2).reshape(1024, R))
        return out

    Y = np.concatenate(
        [decode_yr(res.results[c]["yr"]).T for c in range(N_CORES)], axis=0)
    w_flat = (topk_w * SCALE).astype(np.float32).reshape(-1)
    out = (Y[flat_col.reshape(-1)] * w_flat[:, None]).reshape(T, TOPK, H).sum(1)

    ys_sum = np.zeros((128, 16 * T), dtype=np.float32)
    for c in range(N_CORES):
        ys_sum += np.asarray(res.results[c]["ys"], dtype=np.float32)
    shared = np.empty((H, T), dtype=np.float32)
    for ch in range(T // CH):
        for half in range(2):
            boff = (ch * 2 + half) * 8 * CH
            blk = ys_sum[:, boff:boff + 8 * CH]
            shared[half * 1024:(half + 1) * 1024, ch * CH:(ch + 1) * CH] = (
                blk.reshape(128, 8, CH).transpose(1, 0, 2).reshape(1024, CH))
    out += shared.T
    return out.astype(np.float32)



# revision 59
# speedup vs baseline: 1.1454x; 1.0093x over previous
"""MegrezMoE MoE layer on 8 Trainium2 NeuronCores.

Strategy (expert-parallel, host-routed dispatch):
  - Host computes the (tiny) router: logits -> grouped top-k ids/weights,
    exactly mirroring the reference's noaux_tc selection.
  - 32 experts are assigned 4-per-core, balanced by routed-token count.
    Tokens are gathered per expert on the host (transposed: [H, rows],
    rows padded to a per-slot static capacity) so the device kernel is a
    fully static SPMD program: per expert slot, gate_up matmul ->
    silu*mul -> down matmul, streaming the expert weight bank from HBM
    exactly once per core.
  - The shared-expert MLP is tensor-parallel across the 8 cores (each
    core owns a 128-wide slice of the shared intermediate dim) and its
    partial outputs are summed on the host.
  - All device tensors use a partition-major layout ([128, ...] with
    k-tiles concatenated along the free dim) so every DMA is a plain 2D
    contiguous transfer with minimal descriptor overhead.
  - Matmuls run in bf16 with fp32 PSUM accumulation (KERNEL_DTYPE=f32r
    selects a float32r variant that keeps fp32 data in HBM).
  - Host combines: out[t] = sum_k w[t,k]*SCALE * y_col(t,k) + shared[t].

Scheduling notes (measured on HW, each worth 5-10%):
  - _fuse_ldweights() folds the standalone InstLdweights emitted by tile
    legalization into self-loading matmuls; the compiler's automatic
    fast-weight-load then halves the PE weight-load cost (~254us ->
    ~222us).
  - Expert slots accumulate in two 4-bank PSUM waves (pool psum_e) so
    the shared-expert chunks (pool psum_s) interleave with the expert
    stream instead of serializing at the end.
  - Down-projection weights use their own 2-deep pool so the next
    slot's gate_up weight DMA is not WAR-blocked behind the up phase.
  - Output stores ride the Activation HWDGE queue; all loads ride the
    Sync HWDGE queue, whose triggers run ahead of compute.

kernel() takes the full unsharded inputs, returns the full [1024, 2048]
fp32 output.
"""

import os

import ml_dtypes
import numpy as np

import concourse.bass as bass
import concourse.tile as tile
from concourse import bacc, mybir
from concourse.bass_utils import run_bass_kernel_spmd

# Model dims (hardcoded per problem spec)
H = 2048
E = 32
I = 1024
TOPK = 6
NGROUP = 8
TOPKG = 4
SCALE = 2.5
T = 1024

N_CORES = 8
EPC = 4          # experts per core
KT_H = H // 128  # 16 k-tiles over hidden dim
KT_I = I // 128  # 8 k-tiles over intermediate dim
WLOAD_K = 8      # k-tiles per weight DMA
CH = 256         # shared-expert token chunk

F32 = mybir.dt.float32
F32R = mybir.dt.float32r
BF16 = mybir.dt.bfloat16

_PROGRAM_CACHE = {}
LAST_RESULTS = None  # BassKernelResults from the most recent run (for harness)


def _mode():
    return os.environ.get("KERNEL_DTYPE", "bf16")


def _variant():
    """Scheduling-variant knob for A/B benchmarking (0 = default)."""
    return int(os.environ.get("KERNEL_VARIANT", "0"))


# ---------------------------------------------------------------------------
# Host-side routing (mirrors reference._grouped_topk in fp32 numpy)
# ---------------------------------------------------------------------------

def _host_routing(x, gate_w, e_bias):
    logits = x @ gate_w                                   # [T, E] fp32
    scores = 1.0 / (1.0 + np.exp(-logits, dtype=np.float32))
    scores_choice = scores + e_bias[None, :]
    gsize = E // NGROUP
    grp = scores_choice.reshape(T, NGROUP, gsize)
    top2 = np.sort(grp, axis=-1)[:, :, -2:]
    group_scores = top2.sum(-1)                           # [T, G]
    gidx = np.argsort(-group_scores, axis=-1, kind="stable")[:, :TOPKG]
    gmask = np.zeros((T, NGROUP), bool)
    np.put_along_axis(gmask, gidx, True, axis=1)
    emask = np.repeat(gmask, gsize, axis=1)
    masked = np.where(emask, scores_choice, -np.inf)
    topk_ids = np.argsort(-masked, axis=-1, kind="stable")[:, :TOPK]
    topk_w = np.take_along_axis(scores, topk_ids, axis=1)
    topk_w = topk_w / topk_w.sum(-1, keepdims=True)
    return topk_w.astype(np.float32), topk_ids.astype(np.int64)


# ---------------------------------------------------------------------------
# Dispatch plan: expert -> (core, slot), per-slot static row capacities
# ---------------------------------------------------------------------------

def _make_plan(topk_ids):
    counts = np.bincount(topk_ids.ravel(), minlength=E)
    # slot capacity = max routed count in the slot's expert group, rounded
    # up to 8 (DMA alignment)
    padded = np.maximum(16, ((counts + 7) // 8) * 8)
    order = np.argsort(-padded, kind="stable")            # experts, big first
    expert_of = []      # per slot: experts per core
    slot_rows = []
    slot_piece = []     # (piece index, piece capacity) per slot
    for s in range(EPC):
        chunk = order[s * N_CORES:(s + 1) * N_CORES]
        cap = int(padded[chunk].max())
        if cap <= 512:
            expert_of.append(list(chunk))
            slot_rows.append(cap)
            slot_piece.append((0, cap))
        else:
            # an expert group too wide for one PSUM bank: split into
            # pieces of <=512 rows (same expert, partitioned token list)
            n_p = -(-cap // 512)
            pcap = ((-(-cap // n_p) + 7) // 8) * 8
            for j in range(n_p):
                expert_of.append(list(chunk))
                slot_rows.append(pcap)
                slot_piece.append((j, pcap))
    expert_of = np.asarray(expert_of).T                   # [N_CORES, n_slots]
    offs = np.concatenate([[0], np.cumsum(slot_rows)])
    return {
        "expert_of": expert_of,
        "slot_rows": tuple(slot_rows),
        "slot_piece": slot_piece,
        "slot_offs": offs[:-1],
        "rtotal": int(offs[-1]),
        "counts": counts,
    }


# ---------------------------------------------------------------------------
# Bass program (SPMD; one program, per-core data)
# ---------------------------------------------------------------------------

def _fuse_ldweights(nc):
    """Fold each standalone InstLdweights into the following InstMatmult
    (ldweights=True, self-loading form). Tile legalization always splits
    matmuls into LDW+MM pairs; with the split form the PE pays ~102ns of
    unoverlapped LDWEIGHTS per matmul, while self-loading matmuls get
    the compiler's automatic fast-weight-load (~2x weight path). LDW
    waits move onto the matmul (or an event-semaphore right before it
    when the matmul already waits on a different semaphore)."""
    PE = mybir.EngineType.PE
    n_fused = 0
    for fn in nc.m.functions:
        for blk in fn.blocks:
            pending = None
            out = []
            changed = False
            for inst in blk.instructions:
                if isinstance(inst, mybir.InstLdweights) and inst.engine == PE:
                    assert pending is None, "LDW with no consuming matmul"
                    pending = inst
                    changed = True
                    continue
                if isinstance(inst, mybir.InstMatmult) and inst.engine == PE:
                    assert pending is not None, "matmul without its LDW"
                    ldw, pending = pending, None
                    wap, lap = inst.ins[1], ldw.ins[0]
                    assert (wap.memref, wap.offset, str(wap.ap)) == \
                           (lap.memref, lap.offset, str(lap.ap))
                    inst.ldweights = True
                    lsync = ldw.sync_info
                    lw = list(lsync.on_wait) if lsync else []
                    assert not (lsync and lsync.on_update)
                    if lw:
                        msync = inst.sync_info
                        mw = list(msync.on_wait) if msync else []
                        mu = list(msync.on_update) if msync else []
                        extra = []
                        for w in lw:
                            dup = next((x for x in mw if x.id == w.id and
                                        x.wait_mode == w.wait_mode ==
                                        "sem-ge-imm"), None)
                            if dup is not None:
                                if w.wait_value > dup.wait_value:
                                    mw[mw.index(dup)] = w
                            elif not mw:
                                mw.append(w)
                            else:
                                extra.append(w)
                        if extra:
                            ev = mybir.InstEventSemaphore(
                                name=nc.get_next_instruction_name(),
                                ins=[], outs=[])
                            ev.engine = PE
                            ev.sync_info = mybir.SyncInfo(
                                on_wait=extra, on_update=[])
                            nc.register_instruction(ev)
                            out.append(ev)
                        inst.sync_info = mybir.SyncInfo(
                            on_wait=mw, on_update=mu)
                    n_fused += 1
                out.append(inst)
            assert pending is None
            if changed:
                blk.instructions = out
    return n_fused


def _build_program(slot_rows, mode, variant=0):
    rtotal = sum(slot_rows)
    f32r = mode == "f32r"
    DTD = F32 if f32r else BF16      # dram dtype of matmul operands
    DTS = F32R if f32r else BF16     # sbuf dtype of matmul operands
    DTO = F32 if f32r else BF16      # output dtype

    nc = bacc.Bacc("TRN2", target_bir_lowering=False, debug=False,
                   num_devices=N_CORES)

    # DRAM I/O, all partition-major ([128 partitions, free]):
    #   xg : slot-blocked gathered tokens; slot s at cols KT_H*off_s,
    #        k-tile k of slot s at [KT_H*off_s + k*R_s, +R_s]
    #   xf : chunk-blocked all tokens (for the shared expert)
    #   wgu/wd : per (slot, half): k-tiles concatenated along free dim
    #   yr/ys : per (slot/chunk, half): 8 output m-tiles concatenated
    n_slots = len(slot_rows)
    gather = variant == 7 and not f32r
    # with on-chip gather only slot 0 ships host-gathered tokens; slots
    # 1+ are gathered out of the (SBUF-resident) xf by gpsimd
    xg_cols = KT_H * (slot_rows[0] if gather else rtotal)
    rgath = sum(r // 4 for r in slot_rows[1:])
    xg = nc.dram_tensor("xg", [128, xg_cols], DTD, kind="ExternalInput")
    xf = nc.dram_tensor("xf", [128, KT_H * T], DTD, kind="ExternalInput")
    if gather:
        xidx = nc.dram_tensor("xidx", [128, rgath], mybir.dt.uint16,
                              kind="ExternalInput")
    wgu = nc.dram_tensor("wgu", [n_slots, 2, 128, KT_H * I], DTD,
                         kind="ExternalInput")
    wd = nc.dram_tensor("wd", [n_slots, 2, 128, KT_I * I], DTD,
                        kind="ExternalInput")
    wsg = nc.dram_tensor("wsg", [128, KT_H * 256], DTD, kind="ExternalInput")
    wsd = nc.dram_tensor("wsd", [128, H], DTD, kind="ExternalInput")
    yr = nc.dram_tensor("yr", [128, 16 * rtotal], DTO, kind="ExternalOutput")
    ys = nc.dram_tensor("ys", [128, 16 * T], DTO, kind="ExternalOutput")

    # casting DMA (fp32 dram -> f32r sbuf) must go via SWDGE
    ldma = nc.gpsimd.dma_start if f32r else nc.sync.dma_start
    xdma = ldma

    slot_offs = [0]
    for R in slot_rows[:-1]:
        slot_offs.append(slot_offs[-1] + R)

    # Input loads (weights + x) all ride the Sync HWDGE queue — triggers
    # run well ahead of compute there. Output stores go on the Activation
    # HWDGE queue: their deps (the drain copies) are scalar/vector-local,
    # so a store trigger never blocks that queue's forward progress.
    sdma = ldma
    odma = nc.gpsimd.dma_start if f32r else nc.scalar.dma_start

    # default: 4-k-tile weight DMA groups with an 8-deep pool. Finer
    # arrival granularity engages more DMA rings per phase, and the
    # deep (4MB) prefetch rides out device throttle phases: measured
    # 200us while the 6-deep variant showed 218us in the same rounds;
    # 2-k-tile groups (10-deep) regress to 226us on trigger overhead
    n_wbufs = {1: 5, 13: 4, 14: 10, 16: 6}.get(variant, 8)
    n_xbufs = 4 if variant == 5 else 3
    split_store = variant == 6
    n_pe_bufs, n_psh_bufs = (6, 2) if variant == 11 else (5, 3)
    wk = {13: WLOAD_K, 14: 2}.get(variant, 4)

    with tile.TileContext(nc) as tc:
        with tc.tile_pool(name="psum_e", bufs=n_pe_bufs,
                          space="PSUM") as pe_pool, \
             tc.tile_pool(name="psum_s", bufs=n_psh_bufs,
                          space="PSUM") as psh_pool, \
             tc.tile_pool(name="swg", bufs=1) as swg_pool, \
             tc.tile_pool(name="swd", bufs=1) as swd_pool, \
             tc.tile_pool(name="sxf", bufs=1 if gather else 3) as sxf_pool, \
             tc.tile_pool(name="xidxp", bufs=1) as xidx_pool, \
             tc.tile_pool(name="sact", bufs=2) as sact_pool, \
             tc.tile_pool(name="sout", bufs=2) as sout_pool, \
             tc.tile_pool(name="wsl", bufs=n_wbufs) as w_pool, \
             tc.tile_pool(name="wdl", bufs=2) as wd_pool_, \
             tc.tile_pool(name="xs", bufs=n_xbufs) as x_pool, \
             tc.tile_pool(name="gs", bufs=2) as g_pool, \
             tc.tile_pool(name="at", bufs=2) as a_pool, \
             tc.tile_pool(name="ost", bufs=2) as o_pool:
            # down-projection weights get their own 2-deep pool so the
            # next slot's gate_up weight DMA is not WAR-blocked behind
            # the up phase (measured −16us vs a single 4-deep pool)
            dw_pool = w_pool if variant == 4 else wd_pool_

            wsg_sb = swg_pool.tile([128, KT_H * 256], DTS)
            wsd_sb = swd_pool.tile([128, H], DTS)
            if gather:
                xf_full = sxf_pool.tile([128, KT_H * T], DTS, name="xf_full")
                xidx_sb = xidx_pool.tile([128, rgath], mybir.dt.uint16,
                                         name="xidx_sb")

            def shared_weights():
                sdma(wsg_sb[:], wsg.ap())
                sdma(wsd_sb[:], wsd.ap())

            def shared_chunk(ch):
                if gather:
                    xf_t, xf_b = xf_full, ch * KT_H * CH
                else:
                    xf_t = sxf_pool.tile([128, KT_H * CH], DTS, name="xf_sb")
                    sdma(xf_t[:],
                         xf.ap()[:, ch * KT_H * CH:(ch + 1) * KT_H * CH])
                    xf_b = 0
                ps_g = psh_pool.tile([128, CH], F32, tag="ps", name="ps_g")
                ps_u = psh_pool.tile([128, CH], F32, tag="ps", name="ps_u")
                for k in range(KT_H):
                    lg = wsg_sb[:, k * 256:k * 256 + 128]
                    lu = wsg_sb[:, k * 256 + 128:k * 256 + 256]
                    rx = xf_t[:, xf_b + k * CH:xf_b + (k + 1) * CH]
                    nc.tensor.matmul(ps_g[:], lg, rx,
                                     start=(k == 0), stop=(k == KT_H - 1))
                    nc.tensor.matmul(ps_u[:], lu, rx,
                                     start=(k == 0), stop=(k == KT_H - 1))
                gss = sact_pool.tile([128, CH], F32, tag="sgs", name="gss")
                nc.scalar.activation(gss[:], ps_g[:],
                                     mybir.ActivationFunctionType.Sigmoid)
                nc.vector.tensor_mul(gss[:], gss[:], ps_g[:])
                a_s = sact_pool.tile([128, CH], DTS, tag="sas", name="a_s")
                nc.vector.tensor_mul(a_s[:], gss[:], ps_u[:])
                # down: 16 output m-tiles, single k (the 128-slice of I);
                # vector does the PSUM drain copies (scalar is busy with
                # the expert-slot drains and DMA triggers)
                for half in range(2):
                    stg = sout_pool.tile([128, 8 * CH], DTO, tag="sstg",
                                         name="stg")
                    for m in range(8):
                        pd = psh_pool.tile([128, CH], F32, tag="ps",
                                           name="pd")
                        lw = wsd_sb[:, (half * 8 + m) * 128:
                                    (half * 8 + m + 1) * 128]
                        nc.tensor.matmul(pd[:], lw, a_s[:],
                                         start=True, stop=True)
                        nc.vector.tensor_copy(stg[:, m * CH:(m + 1) * CH],
                                              pd[:])
                    odma(ys.ap()[:, (ch * 2 + half) * 8 * CH:
                                 (ch * 2 + half + 1) * 8 * CH],
                         stg[:])

            def expert_slot(s):
                R = slot_rows[s]
                off = slot_offs[s]
                xbase = KT_H * off
                wt0 = None
                s0_wts = None
                xs = x_pool.tile([128, KT_H * R], DTS, tag="xs", name="xs")
                if s == 0 and variant in (9, 10) and not f32r:
                    # ramp: weights stream on the sync queue, x pieces on
                    # the scalar queue in k-dependency order, so both
                    # transfer chains run in parallel from t=0
                    s0_wts = []
                    for (k0, k1) in [(0, 1), (1, 2), (2, 8), (8, 16)]:
                        wt = w_pool.tile([128, (k1 - k0) * I], DTS,
                                         tag="wsl", name="wt")
                        ldma(wt[:], wgu.ap()[0, 0][:, k0 * I:k1 * I])
                        s0_wts.append(wt)
                    odma(xs[:, :R], xg.ap()[:, xbase:xbase + R])
                    odma(xs[:, R:2 * R], xg.ap()[:, xbase + R:xbase + 2 * R])
                    odma(xs[:, 2 * R:], xg.ap()[:, xbase + 2 * R:
                                                xbase + KT_H * R])
                elif s == 0 and variant == 3 and not f32r:
                    # ramp: interleave phase-0 weight groups with the xs
                    # k-ranges they need
                    s0_wts = []
                    for (k0, k1) in [(0, 1), (1, 2), (2, 8), (8, 16)]:
                        wt = w_pool.tile([128, (k1 - k0) * I], DTS,
                                         tag="wsl", name="wt")
                        ldma(wt[:], wgu.ap()[0, 0][:, k0 * I:k1 * I])
                        s0_wts.append(wt)
                        sdma(xs[:, k0 * R:k1 * R],
                             xg.ap()[:, xbase + k0 * R:xbase + k1 * R])
                elif s == 0:
                    if not f32r:
                        # first weight group is the very first DMA on the
                        # sync queue so the PE can start early
                        wt0 = w_pool.tile([128, I], DTS, tag="wsl",
                                          name="wt")
                        ldma(wt0[:], wgu.ap()[0, 0][:, 0:I])
                    # split the first x load so the PE can start early
                    sdma(xs[:, :R], xg.ap()[:, xbase:xbase + R])
                    sdma(xs[:, R:2 * R], xg.ap()[:, xbase + R:xbase + 2 * R])
                    sdma(xs[:, 2 * R:], xg.ap()[:, xbase + 2 * R:
                                                xbase + KT_H * R])
                    if gather:
                        # piggyback the gather inputs on the sync queue
                        # right behind slot-0's x so they land before
                        # slot 1 needs them
                        sdma(xidx_sb[:], xidx.ap())
                        for ch4 in range(4):
                            sdma(xf_full[:, ch4 * 4096:(ch4 + 1) * 4096],
                                 xf.ap()[:, ch4 * 4096:(ch4 + 1) * 4096])
                elif gather:
                    # gather xs out of the resident xf, 4 k-tiles per call
                    # (ISA caps the indirect-copy dst element count); the
                    # relative index set is shared, the k-offset moves the
                    # data base
                    goff = sum(r // 4 for r in slot_rows[1:s])
                    ncol = R // 4
                    for kb4 in range(4):
                        nc.gpsimd.indirect_copy(
                            xs[:, kb4 * 4 * R:(kb4 + 1) * 4 * R],
                            xf_full[:, kb4 * 4 * CH:],
                            xidx_sb[:, goff:goff + ncol], True)
                else:
                    sdma(xs[:], xg.ap()[:, xbase:xbase + KT_H * R])

                gs = g_pool.tile([128, KT_I * R], DTS if gather else F32,
                                 tag="gs", name="gs")
                at = a_pool.tile([128, KT_I * R], DTS, tag="at", name="at")

                for phase in range(2):  # 0 = gate, 1 = up
                    def gu_drain(m, ps_m):
                        if phase == 0:
                            nc.scalar.activation(
                                gs[:, m * R:(m + 1) * R], ps_m[:],
                                mybir.ActivationFunctionType.Sigmoid)
                            nc.vector.tensor_mul(
                                gs[:, m * R:(m + 1) * R],
                                gs[:, m * R:(m + 1) * R], ps_m[:])
                        else:
                            nc.vector.tensor_mul(
                                at[:, m * R:(m + 1) * R],
                                gs[:, m * R:(m + 1) * R], ps_m[:])

                    if s == 0 and phase == 0:
                        if variant == 13:
                            groups = [(0, 1), (1, 2), (2, 8), (8, 16)]
                        elif variant == 14:
                            # keep every ramp group <= 2 k-tiles so the
                            # pool slab stays at the 2-k-tile size
                            groups = [(0, 1), (1, 2)] + [
                                (k, k + 2) for k in range(2, 16, 2)]
                        else:
                            groups = [(0, 1), (1, 2), (2, 4), (4, 8),
                                      (8, 12), (12, 16)]
                    else:
                        groups = [(kb * wk, (kb + 1) * wk)
                                  for kb in range(KT_H // wk)]
                    if s == 0 and phase == 0 and s0_wts is not None:
                        wts = s0_wts
                    else:
                        wts = []
                        for (k0, k1) in groups:
                            if s == 0 and phase == 0 and k0 == 0 and wt0:
                                wts.append(wt0)
                                continue
                            wt = w_pool.tile([128, (k1 - k0) * I], DTS,
                                             tag="wsl", name="wt")
                            ldma(wt[:], wgu.ap()[s, phase][:, k0 * I:k1 * I])
                            wts.append(wt)
                    if s == 0 and phase == 0 and variant == 9:  # not 10
                        # nothing else competes for PSUM this early, so
                        # run a single 8-bank wave (5 expert + 3 shared
                        # banks): twice the PE work per arrived k-tile,
                        # halving the DMA-latency-bound ramp idle
                        ps8 = ([pe_pool.tile([128, R], F32, tag="ps",
                                             name="ps") for _ in range(5)] +
                               [psh_pool.tile([128, R], F32, tag="ps",
                                              name="ps") for _ in range(3)])
                        for gi, (k0, k1) in enumerate(groups):
                            wt = wts[gi]
                            for kk in range(k1 - k0):
                                k = k0 + kk
                                rx = xs[:, k * R:(k + 1) * R]
                                for m in range(8):
                                    lw = wt[:, kk * I + m * 128:
                                            kk * I + (m + 1) * 128]
                                    nc.tensor.matmul(
                                        ps8[m][:], lw, rx,
                                        start=(k == 0), stop=(k == KT_H - 1))
                        for m in range(8):
                            gu_drain(m, ps8[m])
                        continue
                    # two 4-m-tile waves: expert slots hold at most 4+2
                    # PSUM banks so shared chunks can interleave
                    for wave in range(2):
                        ps = [pe_pool.tile([128, R], F32, tag="ps",
                                           name="ps") for _ in range(4)]
                        for gi, (k0, k1) in enumerate(groups):
                            wt = wts[gi]
                            for kk in range(k1 - k0):
                                k = k0 + kk
                                rx = xs[:, k * R:(k + 1) * R]
                                for mi in range(4):
                                    m = wave * 4 + mi
                                    lw = wt[:, kk * I + m * 128:
                                            kk * I + (m + 1) * 128]
                                    nc.tensor.matmul(
                                        ps[mi][:], lw, rx,
                                        start=(k == 0), stop=(k == KT_H - 1))
                        for mi in range(4):
                            gu_drain(wave * 4 + mi, ps[mi])

                WLD = min(WLOAD_K, KT_I)
                for half in range(2):
                    wts = []
                    for kb in range(KT_I // WLD):
                        wt = dw_pool.tile([128, WLD * I], DTS, tag="wsl",
                                          name="wt")
                        ldma(wt[:], wd.ap()[s, half][:, kb * WLD * I:
                                                     (kb + 1) * WLD * I])
                        wts.append(wt)
                    stg = o_pool.tile([128, 8 * R], DTO, tag="ost", name="stg")
                    boff = 16 * off + half * 8 * R
                    for wave in range(2):
                        ps = [pe_pool.tile([128, R], F32, tag="ps",
                                           name="ps") for _ in range(4)]
                        for kb in range(KT_I // WLD):
                            wt = wts[kb]
                            for kk in range(WLD):
                                k = kb * WLD + kk
                                ra = at[:, k * R:(k + 1) * R]
                                for mi in range(4):
                                    m = wave * 4 + mi
                                    lw = wt[:, kk * I + m * 128:
                                            kk * I + (m + 1) * 128]
                                    nc.tensor.matmul(
                                        ps[mi][:], lw, ra,
                                        start=(k == 0), stop=(k == KT_I - 1))
                        for mi in range(4):
                            m = wave * 4 + mi
                            nc.scalar.copy(stg[:, m * R:(m + 1) * R],
                                           ps[mi][:])
                        if split_store:
                            w0 = wave * 4 * R
                            odma(yr.ap()[:, boff + w0:boff + w0 + 4 * R],
                                 stg[:, w0:w0 + 4 * R])
                    if not split_store:
                        odma(yr.ap()[:, boff:boff + 8 * R], stg[:])

            # experts carry the bulk of the DMA stream; shared-expert
            # chunks are interleaved to fill PE gaps at phase boundaries
            if variant == 8:
                expert_slot(0)
                shared_weights()
                shared_chunk(0)
                for s_i in range(1, n_slots):
                    expert_slot(s_i)
                    shared_chunk(s_i)
            else:
                expert_slot(0)
                shared_weights()
                shared_chunk(0)
                shared_chunk(1)
                for s_i in range(1, n_slots):
                    if s_i == 1:
                        expert_slot(1)
                        shared_chunk(2)
                    elif s_i == 2:
                        expert_slot(2)
                        shared_chunk(3)
                    else:
                        expert_slot(s_i)

    _fuse_ldweights(nc)
    nc.compile()
    return nc


def _get_program(slot_rows, mode):
    variant = _variant()
    key = (tuple(slot_rows), mode, variant)
    if key not in _PROGRAM_CACHE:
        _PROGRAM_CACHE[key] = _build_program(slot_rows, mode, variant)
    return _PROGRAM_CACHE[key]


# ---------------------------------------------------------------------------
# Per-core input construction (host shard + reorder + cast)
# ---------------------------------------------------------------------------

def _pm(a):
    """[KT, 128, M] -> partition-major [128, KT*M]."""
    kt, p, m = a.shape
    return np.ascontiguousarray(a.transpose(1, 0, 2)).reshape(p, kt * m)


def _make_in_maps(x, w_gate_up, w_down, shared_gate_up, shared_down,
                  topk_ids, plan, mode):
    rtotal = plan["rtotal"]
    slot_rows = plan["slot_rows"]
    offs = plan["slot_offs"]
    expert_of = plan["expert_of"]
    np_dt = np.float32 if mode == "f32r" else ml_dtypes.bfloat16

    slot_piece = plan.get("slot_piece") or [(0, r) for r in slot_rows]
    n_slots = len(slot_rows)
    tok_of = [np.where((topk_ids == e).any(axis=1))[0] for e in range(E)]
    flat_col = np.zeros((T, TOPK), dtype=np.int64)

    xT = np.ascontiguousarray(x.T).astype(np_dt)          # [H, T]
    # weights -> [E, 2, 128, KT*I] partition-major k-slab layout
    wgu_pm = np.ascontiguousarray(
        w_gate_up.astype(np_dt).reshape(E, KT_H, 128, 2, I)
        .transpose(0, 3, 2, 1, 4)).reshape(E, 2, 128, KT_H * I)
    wd_pm = np.ascontiguousarray(
        w_down.astype(np_dt).reshape(E, KT_I, 128, 2, I)
        .transpose(0, 3, 2, 1, 4)).reshape(E, 2, 128, KT_I * I)

    xf_pm = np.concatenate(
        [_pm(xT[:, ch * CH:(ch + 1) * CH].reshape(KT_H, 128, CH))
         for ch in range(T // CH)], axis=1)

    gather = _variant() == 7 and mode != "f32r"
    rgath = sum(r // 4 for r in slot_rows[1:])

    in_maps = []
    for c in range(N_CORES):
        xg_pm = np.zeros(
            (128, KT_H * (slot_rows[0] if gather else rtotal)), dtype=np_dt)
        xidx = np.zeros((128, max(rgath, 1)), dtype=np.uint16)
        for s in range(n_slots):
            e = expert_of[c, s]
            pj, pcap = slot_piece[s]
            toks = tok_of[e][pj * pcap:(pj + 1) * pcap]
            R = slot_rows[s]
            if gather and s > 0:
                # wrapped gather indices for one 4-k-tile batch: output
                # col i (= kt_local*R + j) has its index at
                # [i % 16, goff + i // 16], replicated over the eight
                # 16-partition groups; later batches reuse the set with
                # a shifted data base
                goff = sum(r // 4 for r in slot_rows[1:s])
                tpad = np.zeros(R, dtype=np.int64)
                tpad[:len(toks)] = toks
                base = (tpad // CH) * (KT_H * CH) + (tpad % CH)
                colidx = np.concatenate(
                    [base + kt * CH for kt in range(4)]).astype(np.uint16)
                wrapped = colidx.reshape(R // 4, 16).T   # [16, R/4]
                xidx[:, goff:goff + R // 4] = np.tile(wrapped, (8, 1))
            else:
                blk = np.zeros((H, R), dtype=np_dt)
                blk[:, :len(toks)] = xT[:, toks]
                xg_pm[:, KT_H * offs[s]:KT_H * (offs[s] + R)] = _pm(
                    blk.reshape(KT_H, 128, R))
            col_base = c * rtotal + offs[s]
            for pos, t in enumerate(toks):
                for k in np.nonzero(topk_ids[t] == e)[0]:
                    flat_col[t, k] = col_base + pos
        sl = slice(c * 128, (c + 1) * 128)
        sgu_slice = np.concatenate(
            [shared_gate_up[:, sl],
             shared_gate_up[:, 1024 + c * 128:1024 + (c + 1) * 128]],
            axis=1).astype(np_dt)
        im = {
            "xg": xg_pm,
            "xf": xf_pm,
            "wgu": np.ascontiguousarray(wgu_pm[expert_of[c]]),
            "wd": np.ascontiguousarray(wd_pm[expert_of[c]]),
            "wsg": _pm(sgu_slice.reshape(KT_H, 128, 256)),
            "wsd": np.ascontiguousarray(shared_down[sl, :]).astype(np_dt),
        }
        if gather:
            im["xidx"] = xidx
        in_maps.append(im)
    return in_maps, flat_col


# ---------------------------------------------------------------------------
# Entry point
# ---------------------------------------------------------------------------

def kernel(hidden_states, gate_w, e_bias, w_gate_up, w_down,
           shared_gate_up, shared_down):
    global LAST_RESULTS
    mode = _mode()
    x = np.ascontiguousarray(np.asarray(hidden_states, dtype=np.float32))
    gate_w = np.asarray(gate_w, dtype=np.float32)
    e_bias = np.asarray(e_bias, dtype=np.float32)
    w_gate_up = np.asarray(w_gate_up, dtype=np.float32)
    w_down = np.asarray(w_down, dtype=np.float32)
    shared_gate_up = np.asarray(shared_gate_up, dtype=np.float32)
    shared_down = np.asarray(shared_down, dtype=np.float32)

    topk_w, topk_ids = _host_routing(x, gate_w, e_bias)
    plan = _make_plan(topk_ids)
    slot_rows = plan["slot_rows"]
    offs = plan["slot_offs"]
    rtotal = plan["rtotal"]

    nc = _get_program(slot_rows, mode)
    in_maps, flat_col = _make_in_maps(
        x, w_gate_up, w_down, shared_gate_up, shared_down,
        topk_ids, plan, mode)

    trace = bool(int(os.environ.get("KERNEL_TRACE", "0")))
    res = run_bass_kernel_spmd(
        nc, in_maps, list(range(N_CORES)), trace=trace,
        tmpdir=os.environ.get("KERNEL_TRACE_DIR") or None)
    LAST_RESULTS = res

    # decode partition-major outputs back to [H, rtotal] per core
    def decode_yr(arr):
        out = np.empty((H, rtotal), dtype=np.float32)
        for s in range(len(slot_rows)):
            R = slot_rows[s]
            for half in range(2):
                boff = 16 * offs[s] + half * 8 * R
                blk = np.asarray(arr[:, boff:boff + 8 * R], dtype=np.float32)
                out[half * 1024:(half + 1) * 1024, offs[s]:offs[s] + R] = (
                    blk.reshape(128, 8, R).transpose(1, 0, 2).reshape(1024, R))
        return out

    Y = np.concatenate(
        [decode_yr(res.results[c]["yr"]).T for c in range(N_CORES)], axis=0)
    w_flat = (topk_w * SCALE).astype(np.float32).reshape(-1)
    out = (Y[flat_col.reshape(-1)] * w_flat[:, None]).reshape(T, TOPK, H).sum(1)

    ys_sum = np.zeros((128, 16 * T), dtype=np.float32)
    for c in range(N_CORES):
        ys_sum += np.asarray(res.results[c]["ys"], dtype=np.float32)
    shared = np.empty((H, T), dtype=np.float32)
    for ch in range(T // CH):
        for half in range(2):
            boff = (ch * 2 + half) * 8 * CH
            blk = ys_sum[:, boff:boff + 8 * CH]
            shared[half * 1024:(half + 1) * 1024, ch * CH:(ch + 1) * CH] = (
                blk.reshape(128, 8, CH).transpose(1, 0, 2).reshape(1024, CH))
    out += shared.T
    return out.astype(np.float32)



# revision 63
# speedup vs baseline: 1.1690x; 1.0207x over previous
"""MegrezMoE MoE layer on 8 Trainium2 NeuronCores.

Strategy (expert-parallel, host-routed dispatch):
  - Host computes the (tiny) router: logits -> grouped top-k ids/weights,
    exactly mirroring the reference's noaux_tc selection.
  - 32 experts are assigned 4-per-core, balanced by routed-token count.
    Tokens are gathered per expert on the host (transposed: [H, rows],
    rows padded to a per-slot static capacity) so the device kernel is a
    fully static SPMD program: per expert slot, gate_up matmul ->
    silu*mul -> down matmul, streaming the expert weight bank from HBM
    exactly once per core.
  - The shared-expert MLP is tensor-parallel across the 8 cores (each
    core owns a 128-wide slice of the shared intermediate dim) and its
    partial outputs are summed on the host.
  - All device tensors use a partition-major layout ([128, ...] with
    k-tiles concatenated along the free dim) so every DMA is a plain 2D
    contiguous transfer with minimal descriptor overhead.
  - Matmuls run in bf16 with fp32 PSUM accumulation (KERNEL_DTYPE=f32r
    selects a float32r variant that keeps fp32 data in HBM).
  - Host combines: out[t] = sum_k w[t,k]*SCALE * y_col(t,k) + shared[t].

Scheduling notes (measured on HW, each worth 5-10%):
  - _fuse_ldweights() folds the standalone InstLdweights emitted by tile
    legalization into self-loading matmuls; the compiler's automatic
    fast-weight-load then halves the PE weight-load cost (~254us ->
    ~222us).
  - Expert slots accumulate in two 4-bank PSUM waves (pool psum_e) so
    the shared-expert chunks (pool psum_s) interleave with the expert
    stream instead of serializing at the end.
  - Down-projection weights use their own 2-deep pool so the next
    slot's gate_up weight DMA is not WAR-blocked behind the up phase.
  - Output stores ride the Activation HWDGE queue; all loads ride the
    Sync HWDGE queue, whose triggers run ahead of compute.

kernel() takes the full unsharded inputs, returns the full [1024, 2048]
fp32 output.
"""

import os

import ml_dtypes
import numpy as np

import concourse.bass as bass
import concourse.tile as tile
from concourse import bacc, mybir
from concourse.bass_utils import run_bass_kernel_spmd

# Model dims (hardcoded per problem spec)
H = 2048
E = 32
I = 1024
TOPK = 6
NGROUP = 8
TOPKG = 4
SCALE = 2.5
T = 1024

N_CORES = 8
EPC = 4          # experts per core
KT_H = H // 128  # 16 k-tiles over hidden dim
KT_I = I // 128  # 8 k-tiles over intermediate dim
WLOAD_K = 8      # k-tiles per weight DMA
CH = 256         # shared-expert token chunk

F32 = mybir.dt.float32
F32R = mybir.dt.float32r
BF16 = mybir.dt.bfloat16

_PROGRAM_CACHE = {}
LAST_RESULTS = None  # BassKernelResults from the most recent run (for harness)


def _mode():
    return os.environ.get("KERNEL_DTYPE", "bf16")


def _variant():
    """Scheduling-variant knob for A/B benchmarking (0 = default)."""
    return int(os.environ.get("KERNEL_VARIANT", "0"))


# ---------------------------------------------------------------------------
# Host-side routing (mirrors reference._grouped_topk in fp32 numpy)
# ---------------------------------------------------------------------------

def _host_routing(x, gate_w, e_bias):
    logits = x @ gate_w                                   # [T, E] fp32
    scores = 1.0 / (1.0 + np.exp(-logits, dtype=np.float32))
    scores_choice = scores + e_bias[None, :]
    gsize = E // NGROUP
    grp = scores_choice.reshape(T, NGROUP, gsize)
    top2 = np.sort(grp, axis=-1)[:, :, -2:]
    group_scores = top2.sum(-1)                           # [T, G]
    gidx = np.argsort(-group_scores, axis=-1, kind="stable")[:, :TOPKG]
    gmask = np.zeros((T, NGROUP), bool)
    np.put_along_axis(gmask, gidx, True, axis=1)
    emask = np.repeat(gmask, gsize, axis=1)
    masked = np.where(emask, scores_choice, -np.inf)
    topk_ids = np.argsort(-masked, axis=-1, kind="stable")[:, :TOPK]
    topk_w = np.take_along_axis(scores, topk_ids, axis=1)
    topk_w = topk_w / topk_w.sum(-1, keepdims=True)
    return topk_w.astype(np.float32), topk_ids.astype(np.int64)


# ---------------------------------------------------------------------------
# Dispatch plan: expert -> (core, slot), per-slot static row capacities
# ---------------------------------------------------------------------------

def _make_plan(topk_ids):
    counts = np.bincount(topk_ids.ravel(), minlength=E)
    # slot capacity = max routed count in the slot's expert group, rounded
    # up to 8 (DMA alignment)
    padded = np.maximum(16, ((counts + 7) // 8) * 8)
    order = np.argsort(-padded, kind="stable")            # experts, big first
    expert_of = []      # per slot: experts per core
    slot_rows = []
    slot_piece = []     # (piece index, piece capacity) per slot
    for s in range(EPC):
        chunk = order[s * N_CORES:(s + 1) * N_CORES]
        cap = int(padded[chunk].max())
        if cap <= 512:
            expert_of.append(list(chunk))
            slot_rows.append(cap)
            slot_piece.append((0, cap))
        else:
            # an expert group too wide for one PSUM bank: split into
            # pieces of <=512 rows (same expert, partitioned token list)
            n_p = -(-cap // 512)
            pcap = ((-(-cap // n_p) + 7) // 8) * 8
            for j in range(n_p):
                expert_of.append(list(chunk))
                slot_rows.append(pcap)
                slot_piece.append((j, pcap))
    expert_of = np.asarray(expert_of).T                   # [N_CORES, n_slots]
    offs = np.concatenate([[0], np.cumsum(slot_rows)])
    return {
        "expert_of": expert_of,
        "slot_rows": tuple(slot_rows),
        "slot_piece": slot_piece,
        "slot_offs": offs[:-1],
        "rtotal": int(offs[-1]),
        "counts": counts,
    }


# ---------------------------------------------------------------------------
# Bass program (SPMD; one program, per-core data)
# ---------------------------------------------------------------------------

def _fuse_ldweights(nc):
    """Fold each standalone InstLdweights into the following InstMatmult
    (ldweights=True, self-loading form). Tile legalization always splits
    matmuls into LDW+MM pairs; with the split form the PE pays ~102ns of
    unoverlapped LDWEIGHTS per matmul, while self-loading matmuls get
    the compiler's automatic fast-weight-load (~2x weight path). LDW
    waits move onto the matmul (or an event-semaphore right before it
    when the matmul already waits on a different semaphore)."""
    PE = mybir.EngineType.PE
    n_fused = 0
    for fn in nc.m.functions:
        for blk in fn.blocks:
            pending = None
            out = []
            changed = False
            for inst in blk.instructions:
                if isinstance(inst, mybir.InstLdweights) and inst.engine == PE:
                    assert pending is None, "LDW with no consuming matmul"
                    pending = inst
                    changed = True
                    continue
                if isinstance(inst, mybir.InstMatmult) and inst.engine == PE:
                    assert pending is not None, "matmul without its LDW"
                    ldw, pending = pending, None
                    wap, lap = inst.ins[1], ldw.ins[0]
                    assert (wap.memref, wap.offset, str(wap.ap)) == \
                           (lap.memref, lap.offset, str(lap.ap))
                    inst.ldweights = True
                    lsync = ldw.sync_info
                    lw = list(lsync.on_wait) if lsync else []
                    assert not (lsync and lsync.on_update)
                    if lw:
                        msync = inst.sync_info
                        mw = list(msync.on_wait) if msync else []
                        mu = list(msync.on_update) if msync else []
                        extra = []
                        for w in lw:
                            dup = next((x for x in mw if x.id == w.id and
                                        x.wait_mode == w.wait_mode ==
                                        "sem-ge-imm"), None)
                            if dup is not None:
                                if w.wait_value > dup.wait_value:
                                    mw[mw.index(dup)] = w
                            elif not mw:
                                mw.append(w)
                            else:
                                extra.append(w)
                        if extra:
                            ev = mybir.InstEventSemaphore(
                                name=nc.get_next_instruction_name(),
                                ins=[], outs=[])
                            ev.engine = PE
                            ev.sync_info = mybir.SyncInfo(
                                on_wait=extra, on_update=[])
                            nc.register_instruction(ev)
                            out.append(ev)
                        inst.sync_info = mybir.SyncInfo(
                            on_wait=mw, on_update=mu)
                    n_fused += 1
                out.append(inst)
            assert pending is None
            if changed:
                blk.instructions = out
    return n_fused


def _build_program(slot_rows, mode, variant=0):
    rtotal = sum(slot_rows)
    f32r = mode == "f32r"
    DTD = F32 if f32r else BF16      # dram dtype of matmul operands
    DTS = F32R if f32r else BF16     # sbuf dtype of matmul operands
    DTO = F32 if f32r else BF16      # output dtype

    nc = bacc.Bacc("TRN2", target_bir_lowering=False, debug=False,
                   num_devices=N_CORES)

    # DRAM I/O, all partition-major ([128 partitions, free]):
    #   xg : slot-blocked gathered tokens; slot s at cols KT_H*off_s,
    #        k-tile k of slot s at [KT_H*off_s + k*R_s, +R_s]
    #   xf : chunk-blocked all tokens (for the shared expert)
    #   wgu/wd : per (slot, half): k-tiles concatenated along free dim
    #   yr/ys : per (slot/chunk, half): 8 output m-tiles concatenated
    n_slots = len(slot_rows)
    gather = variant == 7 and not f32r
    # with on-chip gather only slot 0 ships host-gathered tokens; slots
    # 1+ are gathered out of the (SBUF-resident) xf by gpsimd
    xg_cols = KT_H * (slot_rows[0] if gather else rtotal)
    rgath = sum(r // 4 for r in slot_rows[1:])
    xg = nc.dram_tensor("xg", [128, xg_cols], DTD, kind="ExternalInput")
    xf = nc.dram_tensor("xf", [128, KT_H * T], DTD, kind="ExternalInput")
    if gather:
        xidx = nc.dram_tensor("xidx", [128, rgath], mybir.dt.uint16,
                              kind="ExternalInput")
    wgu = nc.dram_tensor("wgu", [n_slots, 2, 128, KT_H * I], DTD,
                         kind="ExternalInput")
    wd = nc.dram_tensor("wd", [n_slots, 2, 128, KT_I * I], DTD,
                        kind="ExternalInput")
    wsg = nc.dram_tensor("wsg", [128, KT_H * 256], DTD, kind="ExternalInput")
    wsd = nc.dram_tensor("wsd", [128, H], DTD, kind="ExternalInput")
    yr = nc.dram_tensor("yr", [128, 16 * rtotal], DTO, kind="ExternalOutput")
    ys = nc.dram_tensor("ys", [128, 16 * T], DTO, kind="ExternalOutput")

    # casting DMA (fp32 dram -> f32r sbuf) must go via SWDGE
    ldma = nc.gpsimd.dma_start if f32r else nc.sync.dma_start
    xdma = ldma

    slot_offs = [0]
    for R in slot_rows[:-1]:
        slot_offs.append(slot_offs[-1] + R)

    # Input loads (weights + x) all ride the Sync HWDGE queue — triggers
    # run well ahead of compute there. Output stores go on the Activation
    # HWDGE queue: their deps (the drain copies) are scalar/vector-local,
    # so a store trigger never blocks that queue's forward progress.
    sdma = ldma
    odma = nc.gpsimd.dma_start if f32r else nc.scalar.dma_start

    # default: 4-k-tile weight DMA groups with an 8-deep pool. Finer
    # arrival granularity engages more DMA rings per phase, and the
    # deep (4MB) prefetch rides out device throttle phases: measured
    # 200us while the 6-deep variant showed 218us in the same rounds;
    # 2-k-tile groups (10-deep) regress to 226us on trigger overhead
    n_wbufs = {1: 5, 13: 4, 14: 10, 16: 6}.get(variant, 8)
    n_xbufs = 4 if variant == 5 else 3
    split_store = variant == 6
    n_pe_bufs, n_psh_bufs = (6, 2) if variant == 11 else (5, 3)
    wk = {13: WLOAD_K, 14: 2}.get(variant, 4)
    # down weights use the same finer-group/deeper-pool recipe as
    # gate_up (4-k-tile groups, 4-deep): measured −2us across device
    # states vs one 8-k-tile DMA per half from a 2-deep pool
    wld = min(WLOAD_K, KT_I) if variant == 19 else 4
    n_wd_bufs = 2 if variant == 19 else 4

    with tile.TileContext(nc) as tc:
        with tc.tile_pool(name="psum_e", bufs=n_pe_bufs,
                          space="PSUM") as pe_pool, \
             tc.tile_pool(name="psum_s", bufs=n_psh_bufs,
                          space="PSUM") as psh_pool, \
             tc.tile_pool(name="swg", bufs=1) as swg_pool, \
             tc.tile_pool(name="swd", bufs=1) as swd_pool, \
             tc.tile_pool(name="sxf", bufs=1 if gather else 3) as sxf_pool, \
             tc.tile_pool(name="xidxp", bufs=1) as xidx_pool, \
             tc.tile_pool(name="sact", bufs=2) as sact_pool, \
             tc.tile_pool(name="sout", bufs=2) as sout_pool, \
             tc.tile_pool(name="wsl", bufs=n_wbufs) as w_pool, \
             tc.tile_pool(name="wdl", bufs=n_wd_bufs) as wd_pool_, \
             tc.tile_pool(name="xs", bufs=n_xbufs) as x_pool, \
             tc.tile_pool(name="gs", bufs=2) as g_pool, \
             tc.tile_pool(name="at", bufs=2) as a_pool, \
             tc.tile_pool(name="ost", bufs=2) as o_pool:
            # down-projection weights get their own 2-deep pool so the
            # next slot's gate_up weight DMA is not WAR-blocked behind
            # the up phase (measured −16us vs a single 4-deep pool)
            dw_pool = w_pool if variant == 4 else wd_pool_

            wsg_sb = swg_pool.tile([128, KT_H * 256], DTS)
            wsd_sb = swd_pool.tile([128, H], DTS)
            if gather:
                xf_full = sxf_pool.tile([128, KT_H * T], DTS, name="xf_full")
                xidx_sb = xidx_pool.tile([128, rgath], mybir.dt.uint16,
                                         name="xidx_sb")

            def shared_weights():
                sdma(wsg_sb[:], wsg.ap())
                sdma(wsd_sb[:], wsd.ap())

            def shared_chunk(ch):
                if gather:
                    xf_t, xf_b = xf_full, ch * KT_H * CH
                else:
                    xf_t = sxf_pool.tile([128, KT_H * CH], DTS, name="xf_sb")
                    sdma(xf_t[:],
                         xf.ap()[:, ch * KT_H * CH:(ch + 1) * KT_H * CH])
                    xf_b = 0
                ps_g = psh_pool.tile([128, CH], F32, tag="ps", name="ps_g")
                ps_u = psh_pool.tile([128, CH], F32, tag="ps", name="ps_u")
                for k in range(KT_H):
                    lg = wsg_sb[:, k * 256:k * 256 + 128]
                    lu = wsg_sb[:, k * 256 + 128:k * 256 + 256]
                    rx = xf_t[:, xf_b + k * CH:xf_b + (k + 1) * CH]
                    nc.tensor.matmul(ps_g[:], lg, rx,
                                     start=(k == 0), stop=(k == KT_H - 1))
                    nc.tensor.matmul(ps_u[:], lu, rx,
                                     start=(k == 0), stop=(k == KT_H - 1))
                gss = sact_pool.tile([128, CH], F32, tag="sgs", name="gss")
                nc.scalar.activation(gss[:], ps_g[:],
                                     mybir.ActivationFunctionType.Sigmoid)
                nc.vector.tensor_mul(gss[:], gss[:], ps_g[:])
                a_s = sact_pool.tile([128, CH], DTS, tag="sas", name="a_s")
                nc.vector.tensor_mul(a_s[:], gss[:], ps_u[:])
                # down: 16 output m-tiles, single k (the 128-slice of I);
                # vector does the PSUM drain copies (scalar is busy with
                # the expert-slot drains and DMA triggers)
                for half in range(2):
                    stg = sout_pool.tile([128, 8 * CH], DTO, tag="sstg",
                                         name="stg")
                    for m in range(8):
                        pd = psh_pool.tile([128, CH], F32, tag="ps",
                                           name="pd")
                        lw = wsd_sb[:, (half * 8 + m) * 128:
                                    (half * 8 + m + 1) * 128]
                        nc.tensor.matmul(pd[:], lw, a_s[:],
                                         start=True, stop=True)
                        nc.vector.tensor_copy(stg[:, m * CH:(m + 1) * CH],
                                              pd[:])
                    odma(ys.ap()[:, (ch * 2 + half) * 8 * CH:
                                 (ch * 2 + half + 1) * 8 * CH],
                         stg[:])

            def expert_slot(s):
                R = slot_rows[s]
                off = slot_offs[s]
                xbase = KT_H * off
                wt0 = None
                s0_wts = None
                xs = x_pool.tile([128, KT_H * R], DTS, tag="xs", name="xs")
                if s == 0 and variant in (9, 10) and not f32r:
                    # ramp: weights stream on the sync queue, x pieces on
                    # the scalar queue in k-dependency order, so both
                    # transfer chains run in parallel from t=0
                    s0_wts = []
                    for (k0, k1) in [(0, 1), (1, 2), (2, 8), (8, 16)]:
                        wt = w_pool.tile([128, (k1 - k0) * I], DTS,
                                         tag="wsl", name="wt")
                        ldma(wt[:], wgu.ap()[0, 0][:, k0 * I:k1 * I])
                        s0_wts.append(wt)
                    odma(xs[:, :R], xg.ap()[:, xbase:xbase + R])
                    odma(xs[:, R:2 * R], xg.ap()[:, xbase + R:xbase + 2 * R])
                    odma(xs[:, 2 * R:], xg.ap()[:, xbase + 2 * R:
                                                xbase + KT_H * R])
                elif s == 0 and variant == 3 and not f32r:
                    # ramp: interleave phase-0 weight groups with the xs
                    # k-ranges they need
                    s0_wts = []
                    for (k0, k1) in [(0, 1), (1, 2), (2, 8), (8, 16)]:
                        wt = w_pool.tile([128, (k1 - k0) * I], DTS,
                                         tag="wsl", name="wt")
                        ldma(wt[:], wgu.ap()[0, 0][:, k0 * I:k1 * I])
                        s0_wts.append(wt)
                        sdma(xs[:, k0 * R:k1 * R],
                             xg.ap()[:, xbase + k0 * R:xbase + k1 * R])
                elif s == 0:
                    if not f32r:
                        # first weight group is the very first DMA on the
                        # sync queue so the PE can start early
                        wt0 = w_pool.tile([128, I], DTS, tag="wsl",
                                          name="wt")
                        ldma(wt0[:], wgu.ap()[0, 0][:, 0:I])
                    # split the first x load so the PE can start early
                    sdma(xs[:, :R], xg.ap()[:, xbase:xbase + R])
                    sdma(xs[:, R:2 * R], xg.ap()[:, xbase + R:xbase + 2 * R])
                    sdma(xs[:, 2 * R:], xg.ap()[:, xbase + 2 * R:
                                                xbase + KT_H * R])
                    if gather:
                        # piggyback the gather inputs on the sync queue
                        # right behind slot-0's x so they land before
                        # slot 1 needs them
                        sdma(xidx_sb[:], xidx.ap())
                        for ch4 in range(4):
                            sdma(xf_full[:, ch4 * 4096:(ch4 + 1) * 4096],
                                 xf.ap()[:, ch4 * 4096:(ch4 + 1) * 4096])
                elif gather:
                    # gather xs out of the resident xf, 4 k-tiles per call
                    # (ISA caps the indirect-copy dst element count); the
                    # relative index set is shared, the k-offset moves the
                    # data base
                    goff = sum(r // 4 for r in slot_rows[1:s])
                    ncol = R // 4
                    for kb4 in range(4):
                        nc.gpsimd.indirect_copy(
                            xs[:, kb4 * 4 * R:(kb4 + 1) * 4 * R],
                            xf_full[:, kb4 * 4 * CH:],
                            xidx_sb[:, goff:goff + ncol], True)
                else:
                    sdma(xs[:], xg.ap()[:, xbase:xbase + KT_H * R])

                gs = g_pool.tile([128, KT_I * R], DTS if gather else F32,
                                 tag="gs", name="gs")
                at = a_pool.tile([128, KT_I * R], DTS, tag="at", name="at")

                for phase in range(2):  # 0 = gate, 1 = up
                    def gu_drain(m, ps_m):
                        if phase == 0:
                            nc.scalar.activation(
                                gs[:, m * R:(m + 1) * R], ps_m[:],
                                mybir.ActivationFunctionType.Sigmoid)
                            nc.vector.tensor_mul(
                                gs[:, m * R:(m + 1) * R],
                                gs[:, m * R:(m + 1) * R], ps_m[:])
                        else:
                            nc.vector.tensor_mul(
                                at[:, m * R:(m + 1) * R],
                                gs[:, m * R:(m + 1) * R], ps_m[:])

                    if s == 0 and phase == 0:
                        if variant == 13:
                            groups = [(0, 1), (1, 2), (2, 8), (8, 16)]
                        elif variant == 14:
                            # keep every ramp group <= 2 k-tiles so the
                            # pool slab stays at the 2-k-tile size
                            groups = [(0, 1), (1, 2)] + [
                                (k, k + 2) for k in range(2, 16, 2)]
                        else:
                            groups = [(0, 1), (1, 2), (2, 4), (4, 8),
                                      (8, 12), (12, 16)]
                    else:
                        groups = [(kb * wk, (kb + 1) * wk)
                                  for kb in range(KT_H // wk)]
                    if s == 0 and phase == 0 and s0_wts is not None:
                        wts = s0_wts
                    else:
                        wts = []
                        for (k0, k1) in groups:
                            if s == 0 and phase == 0 and k0 == 0 and wt0:
                                wts.append(wt0)
                                continue
                            wt = w_pool.tile([128, (k1 - k0) * I], DTS,
                                             tag="wsl", name="wt")
                            ldma(wt[:], wgu.ap()[s, phase][:, k0 * I:k1 * I])
                            wts.append(wt)
                    if s == 0 and phase == 0 and variant == 9:  # not 10
                        # nothing else competes for PSUM this early, so
                        # run a single 8-bank wave (5 expert + 3 shared
                        # banks): twice the PE work per arrived k-tile,
                        # halving the DMA-latency-bound ramp idle
                        ps8 = ([pe_pool.tile([128, R], F32, tag="ps",
                                             name="ps") for _ in range(5)] +
                               [psh_pool.tile([128, R], F32, tag="ps",
                                              name="ps") for _ in range(3)])
                        for gi, (k0, k1) in enumerate(groups):
                            wt = wts[gi]
                            for kk in range(k1 - k0):
                                k = k0 + kk
                                rx = xs[:, k * R:(k + 1) * R]
                                for m in range(8):
                                    lw = wt[:, kk * I + m * 128:
                                            kk * I + (m + 1) * 128]
                                    nc.tensor.matmul(
                                        ps8[m][:], lw, rx,
                                        start=(k == 0), stop=(k == KT_H - 1))
                        for m in range(8):
                            gu_drain(m, ps8[m])
                        continue
                    # two 4-m-tile waves: expert slots hold at most 4+2
                    # PSUM banks so shared chunks can interleave
                    for wave in range(2):
                        ps = [pe_pool.tile([128, R], F32, tag="ps",
                                           name="ps") for _ in range(4)]
                        for gi, (k0, k1) in enumerate(groups):
                            wt = wts[gi]
                            for kk in range(k1 - k0):
                                k = k0 + kk
                                rx = xs[:, k * R:(k + 1) * R]
                                for mi in range(4):
                                    m = wave * 4 + mi
                                    lw = wt[:, kk * I + m * 128:
                                            kk * I + (m + 1) * 128]
                                    nc.tensor.matmul(
                                        ps[mi][:], lw, rx,
                                        start=(k == 0), stop=(k == KT_H - 1))
                        for mi in range(4):
                            gu_drain(wave * 4 + mi, ps[mi])

                WLD = wld
                for half in range(2):
                    wts = []
                    for kb in range(KT_I // WLD):
                        wt = dw_pool.tile([128, WLD * I], DTS, tag="wsl",
                                          name="wt")
                        ldma(wt[:], wd.ap()[s, half][:, kb * WLD * I:
                                                     (kb + 1) * WLD * I])
                        wts.append(wt)
                    stg = o_pool.tile([128, 8 * R], DTO, tag="ost", name="stg")
                    boff = 16 * off + half * 8 * R
                    for wave in range(2):
                        ps = [pe_pool.tile([128, R], F32, tag="ps",
                                           name="ps") for _ in range(4)]
                        for kb in range(KT_I // WLD):
                            wt = wts[kb]
                            for kk in range(WLD):
                                k = kb * WLD + kk
                                ra = at[:, k * R:(k + 1) * R]
                                for mi in range(4):
                                    m = wave * 4 + mi
                                    lw = wt[:, kk * I + m * 128:
                                            kk * I + (m + 1) * 128]
                                    nc.tensor.matmul(
                                        ps[mi][:], lw, ra,
                                        start=(k == 0), stop=(k == KT_I - 1))
                        for mi in range(4):
                            m = wave * 4 + mi
                            nc.scalar.copy(stg[:, m * R:(m + 1) * R],
                                           ps[mi][:])
                        if split_store:
                            w0 = wave * 4 * R
                            odma(yr.ap()[:, boff + w0:boff + w0 + 4 * R],
                                 stg[:, w0:w0 + 4 * R])
                    if not split_store:
                        odma(yr.ap()[:, boff:boff + 8 * R], stg[:])

            # experts carry the bulk of the DMA stream; shared-expert
            # chunks are interleaved to fill PE gaps at phase boundaries
            if variant == 8:
                expert_slot(0)
                shared_weights()
                shared_chunk(0)
                for s_i in range(1, n_slots):
                    expert_slot(s_i)
                    shared_chunk(s_i)
            else:
                expert_slot(0)
                shared_weights()
                shared_chunk(0)
                shared_chunk(1)
                for s_i in range(1, n_slots):
                    if s_i == 1:
                        expert_slot(1)
                        shared_chunk(2)
                    elif s_i == 2:
                        expert_slot(2)
                        shared_chunk(3)
                    else:
                        expert_slot(s_i)

    _fuse_ldweights(nc)
    nc.compile()
    return nc


def _get_program(slot_rows, mode):
    variant = _variant()
    key = (tuple(slot_rows), mode, variant)
    if key not in _PROGRAM_CACHE:
        _PROGRAM_CACHE[key] = _build_program(slot_rows, mode, variant)
    return _PROGRAM_CACHE[key]


# ---------------------------------------------------------------------------
# Per-core input construction (host shard + reorder + cast)
# ---------------------------------------------------------------------------

def _pm(a):
    """[KT, 128, M] -> partition-major [128, KT*M]."""
    kt, p, m = a.shape
    return np.ascontiguousarray(a.transpose(1, 0, 2)).reshape(p, kt * m)


def _make_in_maps(x, w_gate_up, w_down, shared_gate_up, shared_down,
                  topk_ids, plan, mode):
    rtotal = plan["rtotal"]
    slot_rows = plan["slot_rows"]
    offs = plan["slot_offs"]
    expert_of = plan["expert_of"]
    np_dt = np.float32 if mode == "f32r" else ml_dtypes.bfloat16

    slot_piece = plan.get("slot_piece") or [(0, r) for r in slot_rows]
    n_slots = len(slot_rows)
    tok_of = [np.where((topk_ids == e).any(axis=1))[0] for e in range(E)]
    flat_col = np.zeros((T, TOPK), dtype=np.int64)

    xT = np.ascontiguousarray(x.T).astype(np_dt)          # [H, T]
    # weights -> [E, 2, 128, KT*I] partition-major k-slab layout
    wgu_pm = np.ascontiguousarray(
        w_gate_up.astype(np_dt).reshape(E, KT_H, 128, 2, I)
        .transpose(0, 3, 2, 1, 4)).reshape(E, 2, 128, KT_H * I)
    wd_pm = np.ascontiguousarray(
        w_down.astype(np_dt).reshape(E, KT_I, 128, 2, I)
        .transpose(0, 3, 2, 1, 4)).reshape(E, 2, 128, KT_I * I)

    xf_pm = np.concatenate(
        [_pm(xT[:, ch * CH:(ch + 1) * CH].reshape(KT_H, 128, CH))
         for ch in range(T // CH)], axis=1)

    gather = _variant() == 7 and mode != "f32r"
    rgath = sum(r // 4 for r in slot_rows[1:])

    in_maps = []
    for c in range(N_CORES):
        xg_pm = np.zeros(
            (128, KT_H * (slot_rows[0] if gather else rtotal)), dtype=np_dt)
        xidx = np.zeros((128, max(rgath, 1)), dtype=np.uint16)
        for s in range(n_slots):
            e = expert_of[c, s]
            pj, pcap = slot_piece[s]
            toks = tok_of[e][pj * pcap:(pj + 1) * pcap]
            R = slot_rows[s]
            if gather and s > 0:
                # wrapped gather indices for one 4-k-tile batch: output
                # col i (= kt_local*R + j) has its index at
                # [i % 16, goff + i // 16], replicated over the eight
                # 16-partition groups; later batches reuse the set with
                # a shifted data base
                goff = sum(r // 4 for r in slot_rows[1:s])
                tpad = np.zeros(R, dtype=np.int64)
                tpad[:len(toks)] = toks
                base = (tpad // CH) * (KT_H * CH) + (tpad % CH)
                colidx = np.concatenate(
                    [base + kt * CH for kt in range(4)]).astype(np.uint16)
                wrapped = colidx.reshape(R // 4, 16).T   # [16, R/4]
                xidx[:, goff:goff + R // 4] = np.tile(wrapped, (8, 1))
            else:
                blk = np.zeros((H, R), dtype=np_dt)
                blk[:, :len(toks)] = xT[:, toks]
                xg_pm[:, KT_H * offs[s]:KT_H * (offs[s] + R)] = _pm(
                    blk.reshape(KT_H, 128, R))
            col_base = c * rtotal + offs[s]
            for pos, t in enumerate(toks):
                for k in np.nonzero(topk_ids[t] == e)[0]:
                    flat_col[t, k] = col_base + pos
        sl = slice(c * 128, (c + 1) * 128)
        sgu_slice = np.concatenate(
            [shared_gate_up[:, sl],
             shared_gate_up[:, 1024 + c * 128:1024 + (c + 1) * 128]],
            axis=1).astype(np_dt)
        im = {
            "xg": xg_pm,
            "xf": xf_pm,
            "wgu": np.ascontiguousarray(wgu_pm[expert_of[c]]),
            "wd": np.ascontiguousarray(wd_pm[expert_of[c]]),
            "wsg": _pm(sgu_slice.reshape(KT_H, 128, 256)),
            "wsd": np.ascontiguousarray(shared_down[sl, :]).astype(np_dt),
        }
        if gather:
            im["xidx"] = xidx
        in_maps.append(im)
    return in_maps, flat_col


# ---------------------------------------------------------------------------
# Entry point
# ---------------------------------------------------------------------------

def kernel(hidden_states, gate_w, e_bias, w_gate_up, w_down,
           shared_gate_up, shared_down):
    global LAST_RESULTS
    mode = _mode()
    x = np.ascontiguousarray(np.asarray(hidden_states, dtype=np.float32))
    gate_w = np.asarray(gate_w, dtype=np.float32)
    e_bias = np.asarray(e_bias, dtype=np.float32)
    w_gate_up = np.asarray(w_gate_up, dtype=np.float32)
    w_down = np.asarray(w_down, dtype=np.float32)
    shared_gate_up = np.asarray(shared_gate_up, dtype=np.float32)
    shared_down = np.asarray(shared_down, dtype=np.float32)

    topk_w, topk_ids = _host_routing(x, gate_w, e_bias)
    plan = _make_plan(topk_ids)
    slot_rows = plan["slot_rows"]
    offs = plan["slot_offs"]
    rtotal = plan["rtotal"]

    nc = _get_program(slot_rows, mode)
    in_maps, flat_col = _make_in_maps(
        x, w_gate_up, w_down, shared_gate_up, shared_down,
        topk_ids, plan, mode)

    trace = bool(int(os.environ.get("KERNEL_TRACE", "0")))
    res = run_bass_kernel_spmd(
        nc, in_maps, list(range(N_CORES)), trace=trace,
        tmpdir=os.environ.get("KERNEL_TRACE_DIR") or None)
    LAST_RESULTS = res

    # decode partition-major outputs back to [H, rtotal] per core
    def decode_yr(arr):
        out = np.empty((H, rtotal), dtype=np.float32)
        for s in range(len(slot_rows)):
            R = slot_rows[s]
            for half in range(2):
                boff = 16 * offs[s] + half * 8 * R
                blk = np.asarray(arr[:, boff:boff + 8 * R], dtype=np.float32)
                out[half * 1024:(half + 1) * 1024, offs[s]:offs[s] + R] = (
                    blk.reshape(128, 8, R).transpose(1, 0, 2).reshape(1024, R))
        return out

    Y = np.concatenate(
        [decode_yr(res.results[c]["yr"]).T for c in range(N_CORES)], axis=0)
    w_flat = (topk_w * SCALE).astype(np.float32).reshape(-1)
    out = (Y[flat_col.reshape(-1)] * w_flat[:, None]).reshape(T, TOPK, H).sum(1)

    ys_sum = np.zeros((128, 16 * T), dtype=np.float32)
    for c in range(N_CORES):
        ys_sum += np.asarray(res.results[c]["ys"], dtype=np.float32)
    shared = np.empty((H, T), dtype=np.float32)
    for ch in range(T // CH):
        for half in range(2):
            boff = (ch * 2 + half) * 8 * CH
            blk = ys_sum[:, boff:boff + 8 * CH]
            shared[half * 1024:(half + 1) * 1024, ch * CH:(ch + 1) * CH] = (
                blk.reshape(128, 8, CH).transpose(1, 0, 2).reshape(1024, CH))
    out += shared.T
    return out.astype(np.float32)

